# revision 3
# baseline (speedup 1.0000x reference)
"""Canny edge detector on 8 Trainium2 NeuronCores — v2.

Changes vs v1 baseline (437 us):
 - Host repacks the input so each SBUF partition's 26-col window is
   contiguous in HBM: input DMA drops from ~104B descriptors (17k+/tile)
   to one ~14KB descriptor per partition.  Output likewise goes to a
   per-partition-contiguous DRAM buffer and is re-assembled on host.
 - Engine redistribution: the three RGB channels are independent until
   the magnitude/orientation accumulation, so channel 1's whole conv
   chain runs on the GpSimd (Pool) engine (which only supports
   add/sub/mult tensor_tensor + tensor_scalar, so its gaussian uses
   Scalar-engine scale-folds instead of scalar_tensor_tensor) while
   channels 0/2 run on Vector (DVE).  Squares/Sqrt/Abs run on the
   Scalar (ACT) engine.  v1 had Vector 92.6% busy, Scalar 15%,
   GpSimd 2.6%.
 - In-place identical-AP outputs reduce SBUF tags so the full N=128 row
   chunk still fits with a double-buffered shared img tile.

Layout (unchanged): 2048 rows sharded over 8 cores (256 rows each,
5-row halo), columns-on-partitions: partition p owns output columns
[16p,16p+16) and holds a 26-wide input window so every stencil is a
free-dim AP offset.
"""

import numpy as np

_COMPILED = {}

H = 2048
W = 2048
HALO = 5
ROWS_PER_CORE = H // 8            # 256
SHARD_ROWS = ROWS_PER_CORE + 2 * HALO   # 266
N_CHUNK = 128                     # output rows per chunk
CHUNKS = [(r, r + N_CHUNK) for r in range(0, ROWS_PER_CORE, N_CHUNK)]


def _build(low, high):
    import concourse.bass as bass
    import concourse.bacc as bacc
    import concourse.mybir as mybir
    from concourse.tile import TileContext

    f32 = mybir.dt.float32
    f16 = mybir.dt.float16
    u8 = mybir.dt.uint8
    Alu = mybir.AluOpType
    Act = mybir.ActivationFunctionType

    g5 = np.exp(-0.5 * (np.arange(5) - 2.0) ** 2).astype(np.float32)
    ga = float(g5[0])
    gb = float(g5[1])
    gab = float(np.float32(ga) / np.float32(gb))
    t1c = float(np.float32(np.tan(np.deg2rad(np.float64(22.5)))))
    t2c = float(np.float32(np.tan(np.deg2rad(np.float64(67.5)))))
    lowx = float(np.nextafter(np.float32(low), np.float32(0.0)))

    nc = bacc.Bacc()
    # packed input: [channel][partition][row][26] — per-partition contiguous
    x = nc.dram_tensor("x", [3, 128, SHARD_ROWS, 26], f32, kind="ExternalInput")
    # packed output: [partition][row][16]
    out = nc.dram_tensor("out", [128, ROWS_PER_CORE, 16], f32, kind="ExternalOutput")

    CH_ROW = SHARD_ROWS * 26          # per-partition elems per channel

    with TileContext(nc) as tc:
        with tc.tile_pool(name="io", bufs=2) as iop, tc.tile_pool(
            name="pl", bufs=1
        ) as pool:
            for (r0, r1) in CHUNKS:
                N = r1 - r0
                R = N + 10          # img/bh rows
                RV = N + 6          # vb rows
                RT = N + 4          # t/g rows
                RN = N + 2          # nms rows

                gpl = pool.tile([128, RT, 20], f32, tag="g")
                sgx = pool.tile([128, RT, 20], f32, tag="sgx")
                sgy = pool.tile([128, RT, 20], f32, tag="sgy")

                def dma_img(c):
                    img = iop.tile([128, R, 26], f32, tag="img", name=f"img{c}")
                    src = bass.AP(
                        x, c * 128 * CH_ROW + r0 * 26,
                        [[CH_ROW, 128], [26, R], [1, 26]],
                    )
                    nc.sync.dma_start(out=img[:], in_=src)
                    return img

                # ---- channel 1 on GpSimd (add/sub/mult TT only) + ACT ----
                img = dma_img(1)
                uA = pool.tile([128, R, 22], f32, tag="uA")
                uB = pool.tile([128, R, 22], f32, tag="uB")
                uC = pool.tile([128, R, 22], f32, tag="uC")
                uD = pool.tile([128, RV, 22], f32, tag="uD")
                P = nc.gpsimd
                # horizontal gaussian: bh = gb*(s1 + (ga/gb)*s2) + x0
                P.tensor_tensor(uA[:], img[:, :, 1:23], img[:, :, 3:25], Alu.add)
                P.tensor_tensor(uB[:], img[:, :, 0:22], img[:, :, 4:26], Alu.add)
                nc.scalar.mul(uB[:], uB[:], gab)
                P.tensor_tensor(uA[:], uA[:], uB[:], Alu.add)
                nc.scalar.mul(uA[:], uA[:], gb)
                P.tensor_tensor(uC[:], uA[:], img[:, :, 2:24], Alu.add)   # bh
                # vertical gaussian
                P.tensor_tensor(uA[:, 0:RV, :], uC[:, 1:RV + 1, :], uC[:, 3:RV + 3, :], Alu.add)
                P.tensor_tensor(uB[:, 0:RV, :], uC[:, 0:RV, :], uC[:, 4:RV + 4, :], Alu.add)
                nc.scalar.mul(uB[:, 0:RV, :], uB[:, 0:RV, :], gab)
                P.tensor_tensor(uA[:, 0:RV, :], uA[:, 0:RV, :], uB[:, 0:RV, :], Alu.add)
                nc.scalar.mul(uA[:, 0:RV, :], uA[:, 0:RV, :], gb)
                P.tensor_tensor(uD[:], uA[:, 0:RV, :], uC[:, 2:RV + 2, :], Alu.add)  # vb
                # vertical sobel: t1 = pairsums, t2 = diff
                P.tensor_tensor(uA[:, 0:RT + 1, :], uD[:, 0:RT + 1, :], uD[:, 1:RT + 2, :], Alu.add)  # p
                P.tensor_tensor(uB[:, 0:RT, :], uA[:, 0:RT, :], uA[:, 1:RT + 1, :], Alu.add)  # t1
                P.tensor_tensor(uA[:, 0:RT, :], uD[:, 0:RT, :], uD[:, 2:RT + 2, :], Alu.subtract)  # t2
                # horizontal sobel
                gx1 = uC[:, 0:RT, 0:20]
                P.tensor_tensor(gx1, uB[:, 0:RT, 0:20], uB[:, 0:RT, 2:22], Alu.subtract)
                P.tensor_tensor(uD[:, 0:RT, 0:21], uA[:, 0:RT, 0:21], uA[:, 0:RT, 1:22], Alu.add)  # w
                gy1 = uB[:, 0:RT, 0:20]
                P.tensor_tensor(gy1, uD[:, 0:RT, 0:20], uD[:, 0:RT, 1:21], Alu.add)
                # magnitude
                q1 = uA[:, 0:RT, 0:20]
                q2 = uD[:, 0:RT, 0:20]
                nc.scalar.activation(q1, gx1, Act.Square)
                nc.scalar.activation(q2, gy1, Act.Square)
                P.tensor_tensor(q1, q1, q2, Alu.add)
                m1 = uD[:, 0:RT, 0:20]
                nc.scalar.activation(m1, q1, Act.Sqrt)

                # ---- channels 0, 2 on Vector + ACT ----
                ch2_out = None
                for c in (0, 2):
                    V = nc.vector
                    img = dma_img(c)
                    tA = pool.tile([128, R, 22], f32, tag="tA")
                    tB = pool.tile([128, R, 22], f32, tag="tB")
                    tC = pool.tile([128, R, 22], f32, tag="tC")
                    tD = pool.tile([128, RV, 22], f32, tag="tD")
                    V.tensor_tensor(tA[:], img[:, :, 1:23], img[:, :, 3:25], Alu.add)
                    V.tensor_tensor(tB[:], img[:, :, 0:22], img[:, :, 4:26], Alu.add)
                    V.scalar_tensor_tensor(
                        tC[:], tA[:], gb, img[:, :, 2:24], Alu.mult, Alu.add)
                    V.scalar_tensor_tensor(
                        tC[:], tB[:], ga, tC[:], Alu.mult, Alu.add)       # bh
                    V.tensor_tensor(tA[:, 0:RV, :], tC[:, 1:RV + 1, :], tC[:, 3:RV + 3, :], Alu.add)
                    V.tensor_tensor(tB[:, 0:RV, :], tC[:, 0:RV, :], tC[:, 4:RV + 4, :], Alu.add)
                    V.scalar_tensor_tensor(
                        tD[:], tA[:, 0:RV, :], gb, tC[:, 2:RV + 2, :], Alu.mult, Alu.add)
                    V.scalar_tensor_tensor(
                        tD[:], tB[:, 0:RV, :], ga, tD[:], Alu.mult, Alu.add)  # vb
                    V.tensor_tensor(tA[:, 0:RT, :], tD[:, 0:RT, :], tD[:, 2:RT + 2, :], Alu.add)
                    V.scalar_tensor_tensor(
                        tA[:, 0:RT, :], tD[:, 1:RT + 1, :], 2.0, tA[:, 0:RT, :],
                        Alu.mult, Alu.add)                                 # t1
                    V.tensor_tensor(tB[:, 0:RT, :], tD[:, 0:RT, :], tD[:, 2:RT + 2, :], Alu.subtract)  # t2
                    gx = sgx[:] if c == 0 else tC[:, 0:RT, 0:20]
                    gy = sgy[:] if c == 0 else tD[:, 0:RT, 0:20]
                    w2 = tD[:, 0:RT, 0:20]
                    V.tensor_tensor(gx, tA[:, 0:RT, 0:20], tA[:, 0:RT, 2:22], Alu.subtract)
                    V.tensor_tensor(w2, tB[:, 0:RT, 0:20], tB[:, 0:RT, 2:22], Alu.add)
                    V.scalar_tensor_tensor(
                        gy, tB[:, 0:RT, 1:21], 2.0, w2, Alu.mult, Alu.add)
                    q1 = tA[:, 0:RT, 0:20]
                    q2 = tB[:, 0:RT, 0:20]
                    nc.scalar.activation(q1, gx, Act.Square)
                    nc.scalar.activation(q2, gy, Act.Square)
                    V.tensor_tensor(q1, q1, q2, Alu.add)
                    m = gpl[:] if c == 0 else tB[:, 0:RT, 0:20]
                    nc.scalar.activation(m, q1, Act.Sqrt)
                    if c == 2:
                        ch2_out = (gx, gy, m)

                # accumulate: channel 1 on Pool, channel 2 on Vector
                P.tensor_tensor(gpl[:], gpl[:], m1, Alu.add)
                P.tensor_tensor(sgx[:], sgx[:], gx1, Alu.add)
                P.tensor_tensor(sgy[:], sgy[:], gy1, Alu.add)
                gx2, gy2, m2 = ch2_out
                nc.vector.tensor_tensor(gpl[:], gpl[:], m2, Alu.add)
                nc.vector.tensor_tensor(sgx[:], sgx[:], gx2, Alu.add)
                nc.vector.tensor_tensor(sgy[:], sgy[:], gy2, Alu.add)

                # ---- NMS (Vector + ACT; d on Pool) ----
                rr = pool.tile([128, RN, 18], f32, tag="tA")
                ss = pool.tile([128, RN, 18], f32, tag="tB")
                m0 = pool.tile([128, RN, 18], u8, tag="mk0")
                mm2 = pool.tile([128, RN, 18], u8, tag="mk1")
                d = pool.tile([128, RN, 18], f32, tag="tC")
                dpos = pool.tile([128, RN, 18], u8, tag="mk2")
                nc.scalar.activation(rr[:], sgy[:, 1:RN + 1, 1:19], Act.Abs)
                nc.scalar.activation(ss[:], sgx[:, 1:RN + 1, 1:19], Act.Abs)
                nc.vector.scalar_tensor_tensor(m0[:], ss[:], t1c, rr[:], Alu.mult, Alu.is_ge)
                nc.vector.scalar_tensor_tensor(mm2[:], ss[:], t2c, rr[:], Alu.mult, Alu.is_le)
                P.tensor_tensor(
                    d[:], sgx[:, 1:RN + 1, 1:19], sgy[:, 1:RN + 1, 1:19], Alu.mult)
                nc.vector.tensor_scalar(dpos[:], d[:], 0.0, None, Alu.is_ge)

                cand = pool.tile([128, RN, 18], f32, tag="tD")
                cc = pool.tile([128, RN, 18], f32, tag="tA")
                cand2 = pool.tile([128, RN, 18], f32, tag="tB")
                cand3 = pool.tile([128, RN, 18], f32, tag="tC")
                nc.vector.tensor_tensor(
                    cand[:], gpl[:, 2:RN + 2, 2:20], gpl[:, 0:RN, 0:18], Alu.max)  # c1 SE/NW
                nc.vector.tensor_tensor(
                    cc[:], gpl[:, 2:RN + 2, 0:18], gpl[:, 0:RN, 2:20], Alu.max)    # c3 SW/NE
                nc.vector.copy_predicated(cc[:], dpos[:], cand[:])
                nc.vector.tensor_tensor(
                    cand2[:], gpl[:, 2:RN + 2, 1:19], gpl[:, 0:RN, 1:19], Alu.max)  # c2 S/N
                nc.vector.copy_predicated(cc[:], mm2[:], cand2[:])
                nc.vector.tensor_tensor(
                    cand3[:], gpl[:, 1:RN + 1, 2:20], gpl[:, 1:RN + 1, 0:18], Alu.max)  # c0 E/W
                nc.vector.copy_predicated(cc[:], m0[:], cand3[:])

                hp = pool.tile([128, RN, 18], f16, tag="tD")
                lm = pool.tile([128, N, 16], f32, tag="tC")
                nc.vector.scalar_tensor_tensor(
                    hp[:], cc[:], high, gpl[:, 1:RN + 1, 1:19], Alu.max, Alu.is_lt)
                nc.vector.scalar_tensor_tensor(
                    lm[:], cc[:, 1:N + 1, 1:17], lowx, gpl[:, 2:RN, 2:18],
                    Alu.max, Alu.is_lt)

                rm1 = pool.tile([128, RN, 16], f16, tag="rm1")
                rm = pool.tile([128, RN, 16], f16, tag="rm")
                cm = pool.tile([128, N, 16], f16, tag="cm")
                nc.vector.tensor_tensor(rm1[:], hp[:, :, 0:16], hp[:, :, 2:18], Alu.max)
                nc.vector.tensor_tensor(rm[:], rm1[:], hp[:, :, 1:17], Alu.max)
                nc.vector.tensor_tensor(cm[:], rm[:, 0:N, :], rm[:, 2:RN, :], Alu.max)
                nc.vector.tensor_tensor(cm[:], cm[:], rm[:, 1:N + 1, :], Alu.max)

                outt = iop.tile([128, N, 16], f32, tag="out")
                nc.vector.tensor_tensor(outt[:], lm[:], cm[:], Alu.mult)
                dst = bass.AP(out, r0 * 16, [[ROWS_PER_CORE * 16, 128], [16, N], [1, 16]])
                nc.sync.dma_start(out=dst, in_=outt[:])

    nc.finalize()
    return nc


def _get_compiled(low, high):
    key = (low, high)
    if key not in _COMPILED:
        _COMPILED[key] = _build(low, high)
    return _COMPILED[key]


def kernel(img, threshold1, threshold2, _trace=False):
    from concourse import bass_utils

    t1 = float(np.asarray(threshold1))
    t2 = float(np.asarray(threshold2))
    low, high = min(t1, t2), max(t1, t2)

    x = np.ascontiguousarray(np.asarray(img, dtype=np.float32)[0])  # [3,H,W]
    # pad rows and columns with HALO zeros on both sides
    xp = np.zeros((3, H + 2 * HALO, W + 2 * HALO), dtype=np.float32)
    xp[:, HALO:HALO + H, HALO:HALO + W] = x

    win = np.lib.stride_tricks.sliding_window_view(xp, 26, axis=2)[:, :, ::16, :]
    # win: [3, H+10, 128, 26]; per core take its 266 rows, partition-major
    in_maps = []
    for k in range(8):
        rows = win[:, k * ROWS_PER_CORE:k * ROWS_PER_CORE + SHARD_ROWS]  # [3,266,128,26]
        packed = np.ascontiguousarray(rows.transpose(0, 2, 1, 3))        # [3,128,266,26]
        in_maps.append({"x": packed})

    nc = _get_compiled(low, high)
    res = bass_utils.run_bass_kernel_spmd(nc, in_maps, core_ids=list(range(8)),
                                          trace=_trace)

    full = np.zeros((1, 1, H, W), dtype=np.float32)
    for k in range(8):
        o = res.results[k]["out"]  # [128, 256, 16]
        full[0, 0, k * ROWS_PER_CORE:(k + 1) * ROWS_PER_CORE, :] = (
            o.transpose(1, 0, 2).reshape(ROWS_PER_CORE, W))
    # reference forces image borders to zero
    full[:, :, 0, :] = 0.0
    full[:, :, -1, :] = 0.0
    full[:, :, :, 0] = 0.0
    full[:, :, :, -1] = 0.0
    if _trace:
        kernel._last_results = res
    return full


# revision 4
# speedup vs baseline: 1.4524x; 1.4524x over previous
"""Canny edge detector on 8 Trainium2 NeuronCores — v2.

Changes vs v1 baseline (437 us):
 - Host repacks the input so each SBUF partition's 26-col window is
   contiguous in HBM: input DMA drops from ~104B descriptors (17k+/tile)
   to one ~14KB descriptor per partition.  Output likewise goes to a
   per-partition-contiguous DRAM buffer and is re-assembled on host.
 - Engine redistribution: the three RGB channels are independent until
   the magnitude/orientation accumulation, so channel 1's whole conv
   chain runs on the GpSimd (Pool) engine (which only supports
   add/sub/mult tensor_tensor + tensor_scalar, so its gaussian uses
   Scalar-engine scale-folds instead of scalar_tensor_tensor) while
   channels 0/2 run on Vector (DVE).  Squares/Sqrt/Abs run on the
   Scalar (ACT) engine.  v1 had Vector 92.6% busy, Scalar 15%,
   GpSimd 2.6%.
 - In-place identical-AP outputs reduce SBUF tags so the full N=128 row
   chunk still fits with a double-buffered shared img tile.

Layout (unchanged): 2048 rows sharded over 8 cores (256 rows each,
5-row halo), columns-on-partitions: partition p owns output columns
[16p,16p+16) and holds a 26-wide input window so every stencil is a
free-dim AP offset.
"""

import numpy as np

_COMPILED = {}

H = 2048
W = 2048
HALO = 5
ROWS_PER_CORE = H // 8            # 256
SHARD_ROWS = ROWS_PER_CORE + 2 * HALO   # 266
N_CHUNK = 128                     # output rows per chunk
CHUNKS = [(r, r + N_CHUNK) for r in range(0, ROWS_PER_CORE, N_CHUNK)]


def _build(low, high):
    import concourse.bass as bass
    import concourse.bacc as bacc
    import concourse.mybir as mybir
    from concourse.tile import TileContext

    f32 = mybir.dt.float32
    f16 = mybir.dt.float16
    u8 = mybir.dt.uint8
    Alu = mybir.AluOpType
    Act = mybir.ActivationFunctionType

    g5 = np.exp(-0.5 * (np.arange(5) - 2.0) ** 2).astype(np.float32)
    ga = float(g5[0])
    gb = float(g5[1])
    gab = float(np.float32(ga) / np.float32(gb))
    t1c = float(np.float32(np.tan(np.deg2rad(np.float64(22.5)))))
    t2c = float(np.float32(np.tan(np.deg2rad(np.float64(67.5)))))
    lowx = float(np.nextafter(np.float32(low), np.float32(0.0)))

    nc = bacc.Bacc()
    # packed input: [channel][partition][row][26] — per-partition contiguous
    x = nc.dram_tensor("x", [3, 128, SHARD_ROWS, 26], f32, kind="ExternalInput")
    # packed output: [partition][row][16]
    out = nc.dram_tensor("out", [128, ROWS_PER_CORE, 16], f32, kind="ExternalOutput")

    CH_ROW = SHARD_ROWS * 26          # per-partition elems per channel

    with TileContext(nc) as tc:
        with tc.tile_pool(name="io", bufs=2) as iop, tc.tile_pool(
            name="pl", bufs=1
        ) as pool:
            for (r0, r1) in CHUNKS:
                N = r1 - r0
                R = N + 10          # img/bh rows
                RV = N + 6          # vb rows
                RT = N + 4          # t/g rows
                RN = N + 2          # nms rows

                gpl = pool.tile([128, RT, 20], f32, tag="g")
                sgx = pool.tile([128, RT, 20], f32, tag="sgx")
                sgy = pool.tile([128, RT, 20], f32, tag="sgy")

                def dma_img(c):
                    img = iop.tile([128, R, 26], f32, tag="img", name=f"img{c}")
                    src = bass.AP(
                        x, c * 128 * CH_ROW + r0 * 26,
                        [[CH_ROW, 128], [26, R], [1, 26]],
                    )
                    nc.sync.dma_start(out=img[:], in_=src)
                    return img

                # ---- channel 1 on GpSimd (add/sub/mult TT only) + ACT ----
                img = dma_img(1)
                uA = pool.tile([128, R, 22], f32, tag="uA")
                uB = pool.tile([128, R, 22], f32, tag="uB")
                uC = pool.tile([128, R, 22], f32, tag="uC")
                uD = pool.tile([128, RV, 22], f32, tag="uD")
                P = nc.vector
                # horizontal gaussian: bh = gb*(s1 + (ga/gb)*s2) + x0
                P.tensor_tensor(uA[:], img[:, :, 1:23], img[:, :, 3:25], Alu.add)
                P.tensor_tensor(uB[:], img[:, :, 0:22], img[:, :, 4:26], Alu.add)
                nc.scalar.mul(uB[:], uB[:], gab)
                P.tensor_tensor(uA[:], uA[:], uB[:], Alu.add)
                nc.scalar.mul(uA[:], uA[:], gb)
                P.tensor_tensor(uC[:], uA[:], img[:, :, 2:24], Alu.add)   # bh
                # vertical gaussian
                P.tensor_tensor(uA[:, 0:RV, :], uC[:, 1:RV + 1, :], uC[:, 3:RV + 3, :], Alu.add)
                P.tensor_tensor(uB[:, 0:RV, :], uC[:, 0:RV, :], uC[:, 4:RV + 4, :], Alu.add)
                nc.scalar.mul(uB[:, 0:RV, :], uB[:, 0:RV, :], gab)
                P.tensor_tensor(uA[:, 0:RV, :], uA[:, 0:RV, :], uB[:, 0:RV, :], Alu.add)
                nc.scalar.mul(uA[:, 0:RV, :], uA[:, 0:RV, :], gb)
                P.tensor_tensor(uD[:], uA[:, 0:RV, :], uC[:, 2:RV + 2, :], Alu.add)  # vb
                # vertical sobel: t1 = pairsums, t2 = diff
                P.tensor_tensor(uA[:, 0:RT + 1, :], uD[:, 0:RT + 1, :], uD[:, 1:RT + 2, :], Alu.add)  # p
                P.tensor_tensor(uB[:, 0:RT, :], uA[:, 0:RT, :], uA[:, 1:RT + 1, :], Alu.add)  # t1
                P.tensor_tensor(uA[:, 0:RT, :], uD[:, 0:RT, :], uD[:, 2:RT + 2, :], Alu.subtract)  # t2
                # horizontal sobel
                gx1 = uC[:, 0:RT, 0:20]
                P.tensor_tensor(gx1, uB[:, 0:RT, 0:20], uB[:, 0:RT, 2:22], Alu.subtract)
                P.tensor_tensor(uD[:, 0:RT, 0:21], uA[:, 0:RT, 0:21], uA[:, 0:RT, 1:22], Alu.add)  # w
                gy1 = uB[:, 0:RT, 0:20]
                P.tensor_tensor(gy1, uD[:, 0:RT, 0:20], uD[:, 0:RT, 1:21], Alu.add)
                # magnitude
                q1 = uA[:, 0:RT, 0:20]
                q2 = uD[:, 0:RT, 0:20]
                nc.scalar.activation(q1, gx1, Act.Square)
                nc.scalar.activation(q2, gy1, Act.Square)
                P.tensor_tensor(q1, q1, q2, Alu.add)
                m1 = uD[:, 0:RT, 0:20]
                nc.scalar.activation(m1, q1, Act.Sqrt)

                # ---- channels 0, 2 on Vector + ACT ----
                ch2_out = None
                for c in (0, 2):
                    V = nc.vector
                    img = dma_img(c)
                    tA = pool.tile([128, R, 22], f32, tag="tA")
                    tB = pool.tile([128, R, 22], f32, tag="tB")
                    tC = pool.tile([128, R, 22], f32, tag="tC")
                    tD = pool.tile([128, RV, 22], f32, tag="tD")
                    V.tensor_tensor(tA[:], img[:, :, 1:23], img[:, :, 3:25], Alu.add)
                    V.tensor_tensor(tB[:], img[:, :, 0:22], img[:, :, 4:26], Alu.add)
                    V.scalar_tensor_tensor(
                        tC[:], tA[:], gb, img[:, :, 2:24], Alu.mult, Alu.add)
                    V.scalar_tensor_tensor(
                        tC[:], tB[:], ga, tC[:], Alu.mult, Alu.add)       # bh
                    V.tensor_tensor(tA[:, 0:RV, :], tC[:, 1:RV + 1, :], tC[:, 3:RV + 3, :], Alu.add)
                    V.tensor_tensor(tB[:, 0:RV, :], tC[:, 0:RV, :], tC[:, 4:RV + 4, :], Alu.add)
                    V.scalar_tensor_tensor(
                        tD[:], tA[:, 0:RV, :], gb, tC[:, 2:RV + 2, :], Alu.mult, Alu.add)
                    V.scalar_tensor_tensor(
                        tD[:], tB[:, 0:RV, :], ga, tD[:], Alu.mult, Alu.add)  # vb
                    V.tensor_tensor(tA[:, 0:RT, :], tD[:, 0:RT, :], tD[:, 2:RT + 2, :], Alu.add)
                    V.scalar_tensor_tensor(
                        tA[:, 0:RT, :], tD[:, 1:RT + 1, :], 2.0, tA[:, 0:RT, :],
                        Alu.mult, Alu.add)                                 # t1
                    V.tensor_tensor(tB[:, 0:RT, :], tD[:, 0:RT, :], tD[:, 2:RT + 2, :], Alu.subtract)  # t2
                    gx = sgx[:] if c == 0 else tC[:, 0:RT, 0:20]
                    gy = sgy[:] if c == 0 else tD[:, 0:RT, 0:20]
                    w2 = tD[:, 0:RT, 0:20]
                    V.tensor_tensor(gx, tA[:, 0:RT, 0:20], tA[:, 0:RT, 2:22], Alu.subtract)
                    V.tensor_tensor(w2, tB[:, 0:RT, 0:20], tB[:, 0:RT, 2:22], Alu.add)
                    V.scalar_tensor_tensor(
                        gy, tB[:, 0:RT, 1:21], 2.0, w2, Alu.mult, Alu.add)
                    q1 = tA[:, 0:RT, 0:20]
                    q2 = tB[:, 0:RT, 0:20]
                    nc.scalar.activation(q1, gx, Act.Square)
                    nc.scalar.activation(q2, gy, Act.Square)
                    V.tensor_tensor(q1, q1, q2, Alu.add)
                    m = gpl[:] if c == 0 else tB[:, 0:RT, 0:20]
                    nc.scalar.activation(m, q1, Act.Sqrt)
                    if c == 2:
                        ch2_out = (gx, gy, m)

                # accumulate: channel 1 on Pool, channel 2 on Vector
                P.tensor_tensor(gpl[:], gpl[:], m1, Alu.add)
                P.tensor_tensor(sgx[:], sgx[:], gx1, Alu.add)
                P.tensor_tensor(sgy[:], sgy[:], gy1, Alu.add)
                gx2, gy2, m2 = ch2_out
                nc.vector.tensor_tensor(gpl[:], gpl[:], m2, Alu.add)
                nc.vector.tensor_tensor(sgx[:], sgx[:], gx2, Alu.add)
                nc.vector.tensor_tensor(sgy[:], sgy[:], gy2, Alu.add)

                # ---- NMS (Vector + ACT; d on Pool) ----
                rr = pool.tile([128, RN, 18], f32, tag="tA")
                ss = pool.tile([128, RN, 18], f32, tag="tB")
                m0 = pool.tile([128, RN, 18], u8, tag="mk0")
                mm2 = pool.tile([128, RN, 18], u8, tag="mk1")
                d = pool.tile([128, RN, 18], f32, tag="tC")
                dpos = pool.tile([128, RN, 18], u8, tag="mk2")
                nc.scalar.activation(rr[:], sgy[:, 1:RN + 1, 1:19], Act.Abs)
                nc.scalar.activation(ss[:], sgx[:, 1:RN + 1, 1:19], Act.Abs)
                nc.vector.scalar_tensor_tensor(m0[:], ss[:], t1c, rr[:], Alu.mult, Alu.is_ge)
                nc.vector.scalar_tensor_tensor(mm2[:], ss[:], t2c, rr[:], Alu.mult, Alu.is_le)
                P.tensor_tensor(
                    d[:], sgx[:, 1:RN + 1, 1:19], sgy[:, 1:RN + 1, 1:19], Alu.mult)
                nc.vector.tensor_scalar(dpos[:], d[:], 0.0, None, Alu.is_ge)

                cand = pool.tile([128, RN, 18], f32, tag="tD")
                cc = pool.tile([128, RN, 18], f32, tag="tA")
                cand2 = pool.tile([128, RN, 18], f32, tag="tB")
                cand3 = pool.tile([128, RN, 18], f32, tag="tC")
                nc.vector.tensor_tensor(
                    cand[:], gpl[:, 2:RN + 2, 2:20], gpl[:, 0:RN, 0:18], Alu.max)  # c1 SE/NW
                nc.vector.tensor_tensor(
                    cc[:], gpl[:, 2:RN + 2, 0:18], gpl[:, 0:RN, 2:20], Alu.max)    # c3 SW/NE
                nc.vector.copy_predicated(cc[:], dpos[:], cand[:])
                nc.vector.tensor_tensor(
                    cand2[:], gpl[:, 2:RN + 2, 1:19], gpl[:, 0:RN, 1:19], Alu.max)  # c2 S/N
                nc.vector.copy_predicated(cc[:], mm2[:], cand2[:])
                nc.vector.tensor_tensor(
                    cand3[:], gpl[:, 1:RN + 1, 2:20], gpl[:, 1:RN + 1, 0:18], Alu.max)  # c0 E/W
                nc.vector.copy_predicated(cc[:], m0[:], cand3[:])

                hp = pool.tile([128, RN, 18], f16, tag="tD")
                lm = pool.tile([128, N, 16], f32, tag="tC")
                nc.vector.scalar_tensor_tensor(
                    hp[:], cc[:], high, gpl[:, 1:RN + 1, 1:19], Alu.max, Alu.is_lt)
                nc.vector.scalar_tensor_tensor(
                    lm[:], cc[:, 1:N + 1, 1:17], lowx, gpl[:, 2:RN, 2:18],
                    Alu.max, Alu.is_lt)

                rm1 = pool.tile([128, RN, 16], f16, tag="rm1")
                rm = pool.tile([128, RN, 16], f16, tag="rm")
                cm = pool.tile([128, N, 16], f16, tag="cm")
                nc.vector.tensor_tensor(rm1[:], hp[:, :, 0:16], hp[:, :, 2:18], Alu.max)
                nc.vector.tensor_tensor(rm[:], rm1[:], hp[:, :, 1:17], Alu.max)
                nc.vector.tensor_tensor(cm[:], rm[:, 0:N, :], rm[:, 2:RN, :], Alu.max)
                nc.vector.tensor_tensor(cm[:], cm[:], rm[:, 1:N + 1, :], Alu.max)

                outt = iop.tile([128, N, 16], f32, tag="out")
                nc.vector.tensor_tensor(outt[:], lm[:], cm[:], Alu.mult)
                dst = bass.AP(out, r0 * 16, [[ROWS_PER_CORE * 16, 128], [16, N], [1, 16]])
                nc.sync.dma_start(out=dst, in_=outt[:])

    nc.finalize()
    return nc


def _get_compiled(low, high):
    key = (low, high)
    if key not in _COMPILED:
        _COMPILED[key] = _build(low, high)
    return _COMPILED[key]


def kernel(img, threshold1, threshold2, _trace=False):
    from concourse import bass_utils

    t1 = float(np.asarray(threshold1))
    t2 = float(np.asarray(threshold2))
    low, high = min(t1, t2), max(t1, t2)

    x = np.ascontiguousarray(np.asarray(img, dtype=np.float32)[0])  # [3,H,W]
    # pad rows and columns with HALO zeros on both sides
    xp = np.zeros((3, H + 2 * HALO, W + 2 * HALO), dtype=np.float32)
    xp[:, HALO:HALO + H, HALO:HALO + W] = x

    win = np.lib.stride_tricks.sliding_window_view(xp, 26, axis=2)[:, :, ::16, :]
    # win: [3, H+10, 128, 26]; per core take its 266 rows, partition-major
    in_maps = []
    for k in range(8):
        rows = win[:, k * ROWS_PER_CORE:k * ROWS_PER_CORE + SHARD_ROWS]  # [3,266,128,26]
        packed = np.ascontiguousarray(rows.transpose(0, 2, 1, 3))        # [3,128,266,26]
        in_maps.append({"x": packed})

    nc = _get_compiled(low, high)
    res = bass_utils.run_bass_kernel_spmd(nc, in_maps, core_ids=list(range(8)),
                                          trace=_trace)

    full = np.zeros((1, 1, H, W), dtype=np.float32)
    for k in range(8):
        o = res.results[k]["out"]  # [128, 256, 16]
        full[0, 0, k * ROWS_PER_CORE:(k + 1) * ROWS_PER_CORE, :] = (
            o.transpose(1, 0, 2).reshape(ROWS_PER_CORE, W))
    # reference forces image borders to zero
    full[:, :, 0, :] = 0.0
    full[:, :, -1, :] = 0.0
    full[:, :, :, 0] = 0.0
    full[:, :, :, -1] = 0.0
    if _trace:
        kernel._last_results = res
    return full


# revision 5
# speedup vs baseline: 1.6490x; 1.1353x over previous
"""Canny v3: rows-on-partitions + PE band-matmuls for vertical convs.

Per core (256 out rows, shard rows [-5,261) zero-padded):
 - two rows-mode chunks: chunk a = T/g rows [-2,126) -> out [0,124),
   chunk b = T rows [122,250) -> out [124,248).  Partition = image row,
   free dim = full 2054/2052-wide rows, so every DVE op covers 128 rows
   at ~2054 elems (vs ~2900 elems per 128 rows in the columns layout).
 - vertical (gaussian o sobel) 7-tap convs fused into PE matmuls with
   band weight matrices; halo rows accumulate from the neighboring bh
   tile via a second matmul into the same PSUM bank.
 - NMS vertical/diagonal neighbors via SBUF->SBUF DMA row-shifted
   copies of the g plane (and of rm for hysteresis).
 - last 8 rows per core done in a small v1-style columns-mode pass.
"""

import numpy as np

_COMPILED = {}

H = 2048
W = 2048
HALO = 5
RPC = H // 8                      # 256 rows per core
SHARD_ROWS = RPC + 2 * HALO       # 266
WPAD = W + 2 * HALO               # 2058
WB = W + 6                        # 2054: bh/T cols [-3, 2051)
WG = W + 4                        # 2052: g/gx/gy cols [-2, 2050)
WC = W + 2                        # 2050: cand/cc/hp cols [-1, 2049)
NT = 8                            # tail rows (columns-mode)
NR = RPC - NT                     # 248 rows via rows-mode
BLOCKS = [(0, 512), (512, 512), (1024, 512), (1536, 512), (2048, WB - 2048)]


def _weights():
    g5 = np.exp(-0.5 * (np.arange(5) - 2.0) ** 2).astype(np.float32)
    w7a = np.convolve(np.array([1, 2, 1], np.float32), g5).astype(np.float32)
    w7b = np.convolve(np.array([1, 0, -1], np.float32), g5).astype(np.float32)

    def w7i(w7, i):
        return float(w7[i]) if 0 <= i < 7 else 0.0

    wm = np.zeros((8, 128, 128), np.float32)
    for p in range(128):
        for j in range(128):
            wm[0, p, j] = w7i(w7a, p - j)          # Wa_main
            wm[3, p, j] = w7i(w7b, p - j)          # Wb_main
            if 4 <= p < 10:
                wm[1, p, j] = w7i(w7a, p + 124 - j)  # Wa_h1 (chunk a halo from bh_b)
                wm[4, p, j] = w7i(w7b, p + 124 - j)  # Wb_h1
            if p < 6:
                wm[2, p, j] = w7i(w7a, p + 128 - j)  # Wa_hx (chunk b halo from bhx)
                wm[5, p, j] = w7i(w7b, p + 128 - j)  # Wb_hx
            if p == j + 1:
                wm[6, p, j] = 1.0                    # SU: gU[j] = g[j+1]
            if p == j - 1:
                wm[7, p, j] = 1.0                    # SD: gD[j] = g[j-1]
    return wm


def _build(low, high):
    import concourse.bass as bass
    import concourse.bacc as bacc
    import concourse.mybir as mybir
    from concourse.tile import TileContext

    f32 = mybir.dt.float32
    f16 = mybir.dt.float16
    u8 = mybir.dt.uint8
    Alu = mybir.AluOpType
    Act = mybir.ActivationFunctionType

    g5 = np.exp(-0.5 * (np.arange(5) - 2.0) ** 2).astype(np.float32)
    ga = float(g5[0])
    gb = float(g5[1])
    t1c = float(np.float32(np.tan(np.deg2rad(np.float64(22.5)))))
    t2c = float(np.float32(np.tan(np.deg2rad(np.float64(67.5)))))
    lowx = float(np.nextafter(np.float32(low), np.float32(0.0)))

    nc = bacc.Bacc()
    xr = nc.dram_tensor("xr", [3, SHARD_ROWS, WPAD], f32, kind="ExternalInput")
    xt = nc.dram_tensor("xt", [3, 128, 18, 26], f32, kind="ExternalInput")
    wm = nc.dram_tensor("wm", [8, 128, 128], f32, kind="ExternalInput")
    wm16 = nc.dram_tensor("wm16", [128, 128], f16, kind="ExternalInput")
    out = nc.dram_tensor("out", [NR, W], f16, kind="ExternalOutput")
    out_t = nc.dram_tensor("out_t", [128, NT, 16], f32, kind="ExternalOutput")

    with TileContext(nc) as tc:
        with tc.tile_pool(name="io", bufs=2) as iop, \
             tc.tile_pool(name="pl", bufs=1) as pool, \
             tc.tile_pool(name="ps", bufs=2, space="PSUM") as psp:

            V = nc.vector
            S = nc.scalar

            wt_ = []
            for i in range(8):
                wti = pool.tile([128, 128], f32, tag=f"w{i}", name=f"wt{i}")
                nc.sync.dma_start(out=wti[:], in_=bass.AP(wm, i * 128 * 128,
                                                          [[128, 128], [1, 128]]))
                wt_.append(wti)
            m3t = pool.tile([128, 128], f16, tag="m3", name="m3t")
            nc.sync.dma_start(out=m3t[:], in_=bass.AP(wm16, 0, [[128, 128], [1, 128]]))

            # accumulators per chunk
            acc = {}
            for q in ("a", "b"):
                acc[q] = (
                    pool.tile([128, WG], f32, tag=f"g_{q}", name=f"g_{q}"),
                    pool.tile([128, WG], f32, tag=f"sgx_{q}", name=f"sgx_{q}"),
                    pool.tile([128, WG], f32, tag=f"sgy_{q}", name=f"sgy_{q}"),
                )

            # ---- tail: columns-mode v1-style pass for out rows [248,256) ----
            N, R, RV, RT, RN = NT, NT + 10, NT + 6, NT + 4, NT + 2
            tg = pool.tile([128, RT, 20], f32, tag="tg", name="tg")
            tsgx = pool.tile([128, RT, 20], f32, tag="tsgx", name="tsgx")
            tsgy = pool.tile([128, RT, 20], f32, tag="tsgy", name="tsgy")
            for c in range(3):
                img = pool.tile([128, R, 26], f32, tag="timg", name=f"timg{c}")
                nc.sync.dma_start(out=img[:], in_=bass.AP(
                    xt, c * 128 * 18 * 26, [[18 * 26, 128], [26, R], [1, 26]]))
                tA = pool.tile([128, R, 22], f32, tag="tlA", name=f"tlA{c}")
                tB = pool.tile([128, R, 22], f32, tag="tlB", name=f"tlB{c}")
                tC = pool.tile([128, R, 22], f32, tag="tlC", name=f"tlC{c}")
                tD = pool.tile([128, RV, 22], f32, tag="tlD", name=f"tlD{c}")
                V.tensor_tensor(tA[:], img[:, :, 1:23], img[:, :, 3:25], Alu.add)
                V.tensor_tensor(tB[:], img[:, :, 0:22], img[:, :, 4:26], Alu.add)
                V.scalar_tensor_tensor(tC[:], tA[:], gb, img[:, :, 2:24], Alu.mult, Alu.add)
                V.scalar_tensor_tensor(tC[:], tB[:], ga, tC[:], Alu.mult, Alu.add)
                V.tensor_tensor(tA[:, 0:RV, :], tC[:, 1:RV + 1, :], tC[:, 3:RV + 3, :], Alu.add)
                V.tensor_tensor(tB[:, 0:RV, :], tC[:, 0:RV, :], tC[:, 4:RV + 4, :], Alu.add)
                V.scalar_tensor_tensor(tD[:], tA[:, 0:RV, :], gb, tC[:, 2:RV + 2, :],
                                       Alu.mult, Alu.add)
                V.scalar_tensor_tensor(tD[:], tB[:, 0:RV, :], ga, tD[:], Alu.mult, Alu.add)
                V.tensor_tensor(tA[:, 0:RT, :], tD[:, 0:RT, :], tD[:, 2:RT + 2, :], Alu.add)
                V.scalar_tensor_tensor(tA[:, 0:RT, :], tD[:, 1:RT + 1, :], 2.0,
                                       tA[:, 0:RT, :], Alu.mult, Alu.add)   # t1
                V.tensor_tensor(tB[:, 0:RT, :], tD[:, 0:RT, :], tD[:, 2:RT + 2, :],
                                Alu.subtract)                                # t2
                gx = tsgx[:] if c == 0 else tC[:, 0:RT, 0:20]
                gy = tsgy[:] if c == 0 else tD[:, 0:RT, 0:20]
                w2 = tD[:, 0:RT, 0:20]
                V.tensor_tensor(gx, tA[:, 0:RT, 0:20], tA[:, 0:RT, 2:22], Alu.subtract)
                V.tensor_tensor(w2, tB[:, 0:RT, 0:20], tB[:, 0:RT, 2:22], Alu.add)
                V.scalar_tensor_tensor(gy, tB[:, 0:RT, 1:21], 2.0, w2, Alu.mult, Alu.add)
                q1 = tA[:, 0:RT, 0:20]
                q2 = tB[:, 0:RT, 0:20]
                S.activation(q1, gx, Act.Square)
                S.activation(q2, gy, Act.Square)
                V.tensor_tensor(q1, q1, q2, Alu.add)
                m = tg[:] if c == 0 else tB[:, 0:RT, 0:20]
                S.activation(m, q1, Act.Sqrt)
                if c != 0:
                    V.tensor_tensor(tg[:], tg[:], m, Alu.add)
                    V.tensor_tensor(tsgx[:], tsgx[:], gx, Alu.add)
                    V.tensor_tensor(tsgy[:], tsgy[:], gy, Alu.add)

            rr = pool.tile([128, RN, 18], f32, tag="tlA", name="trr")
            ss = pool.tile([128, RN, 18], f32, tag="tlB", name="tss")
            m0 = pool.tile([128, RN, 18], u8, tag="tmk0", name="tm0")
            m2 = pool.tile([128, RN, 18], u8, tag="tmk1", name="tm2")
            d = pool.tile([128, RN, 18], f32, tag="tlC", name="td")
            dpos = pool.tile([128, RN, 18], u8, tag="tmk2", name="tdp")
            S.activation(rr[:], tsgy[:, 1:RN + 1, 1:19], Act.Abs)
            S.activation(ss[:], tsgx[:, 1:RN + 1, 1:19], Act.Abs)
            V.scalar_tensor_tensor(m0[:], ss[:], t1c, rr[:], Alu.mult, Alu.is_ge)
            V.scalar_tensor_tensor(m2[:], ss[:], t2c, rr[:], Alu.mult, Alu.is_le)
            V.tensor_tensor(d[:], tsgx[:, 1:RN + 1, 1:19], tsgy[:, 1:RN + 1, 1:19], Alu.mult)
            V.tensor_scalar(dpos[:], d[:], 0.0, None, Alu.is_ge)
            cand = pool.tile([128, RN, 18], f32, tag="tlD", name="tc1")
            cc = pool.tile([128, RN, 18], f32, tag="tlA", name="tcc")
            cand2 = pool.tile([128, RN, 18], f32, tag="tlB", name="tc2")
            cand3 = pool.tile([128, RN, 18], f32, tag="tlC", name="tc0")
            V.tensor_tensor(cand[:], tg[:, 2:RN + 2, 2:20], tg[:, 0:RN, 0:18], Alu.max)
            V.tensor_tensor(cc[:], tg[:, 2:RN + 2, 0:18], tg[:, 0:RN, 2:20], Alu.max)
            V.copy_predicated(cc[:], dpos[:], cand[:])
            V.tensor_tensor(cand2[:], tg[:, 2:RN + 2, 1:19], tg[:, 0:RN, 1:19], Alu.max)
            V.copy_predicated(cc[:], m2[:], cand2[:])
            V.tensor_tensor(cand3[:], tg[:, 1:RN + 1, 2:20], tg[:, 1:RN + 1, 0:18], Alu.max)
            V.copy_predicated(cc[:], m0[:], cand3[:])
            hp = pool.tile([128, RN, 18], f16, tag="thp", name="thp")
            lm = pool.tile([128, N, 16], f32, tag="tlC", name="tlm")
            V.scalar_tensor_tensor(hp[:], cc[:], high, tg[:, 1:RN + 1, 1:19],
                                   Alu.max, Alu.is_lt)
            V.scalar_tensor_tensor(lm[:], cc[:, 1:N + 1, 1:17], lowx,
                                   tg[:, 2:RN, 2:18], Alu.max, Alu.is_lt)
            rm1 = pool.tile([128, RN, 16], f16, tag="trm1", name="trm1")
            rm = pool.tile([128, RN, 16], f16, tag="trm", name="trm")
            cm = pool.tile([128, N, 16], f16, tag="tcm", name="tcm")
            V.tensor_tensor(rm1[:], hp[:, :, 0:16], hp[:, :, 2:18], Alu.max)
            V.tensor_tensor(rm[:], rm1[:], hp[:, :, 1:17], Alu.max)
            V.tensor_tensor(cm[:], rm[:, 0:N, :], rm[:, 2:RN, :], Alu.max)
            V.tensor_tensor(cm[:], cm[:], rm[:, 1:N + 1, :], Alu.max)
            outt = pool.tile([128, N, 16], f32, tag="tlD", name="touts")
            V.tensor_tensor(outt[:], lm[:], cm[:], Alu.mult)
            nc.sync.dma_start(
                out=bass.AP(out_t, 0, [[NT * 16, 128], [16, N], [1, 16]]),
                in_=outt[:])


            shifts = {}
            for c in range(3):
                # ---- hgauss for the three bh tiles of this channel ----
                bhs = {}
                for (nm, row0, nrows, tag) in (
                        ("a", 0, 128, "bhA"), ("b", 124, 128, "bhB"),
                        ("x", 252, 6, "bhX")):
                    img = iop.tile([128, WPAD], f32, tag="img", name=f"img{c}{nm}")
                    off0 = c * SHARD_ROWS * WPAD + row0 * WPAD
                    nc.sync.dma_start(
                        out=img[0:nrows, :],
                        in_=bass.AP(xr, off0, [[WPAD, nrows], [343, 6], [1, 343]]))
                    s1 = pool.tile([128, WB], f32, tag="S1", name=f"s1_{c}{nm}")
                    s2 = pool.tile([128, WB], f32, tag="S2", name=f"s2_{c}{nm}")
                    bh = pool.tile([128, WB], f32, tag=tag, name=f"bh{nm}{c}")
                    r = slice(0, nrows)
                    V.tensor_tensor(s1[r, :], img[r, 1:WB + 1], img[r, 3:WB + 3], Alu.add)
                    V.tensor_tensor(s2[r, :], img[r, 0:WB], img[r, 4:WB + 4], Alu.add)
                    V.scalar_tensor_tensor(bh[r, :], s1[r, :], gb, img[r, 2:WB + 2],
                                           Alu.mult, Alu.add)
                    V.scalar_tensor_tensor(bh[r, :], s2[r, :], ga, bh[r, :],
                                           Alu.mult, Alu.add)
                    bhs[nm] = bh

                # ---- per chunk: PE T1/T2, then hsobel + mag on DVE ----
                for q in ("a", "b"):
                    if q == "a":
                        main_src, halo_src, halo_k, wia, wib = bhs["a"], bhs["b"], 128, 1, 4
                    else:
                        main_src, halo_src, halo_k, wia, wib = bhs["b"], bhs["x"], 6, 2, 5
                    S1 = pool.tile([128, WB], f32, tag="S1", name=f"S1_{c}{q}")
                    S2 = pool.tile([128, WB], f32, tag="S2", name=f"S2_{c}{q}")
                    gq, sgxq, sgyq = acc[q]
                    gxt = sgxq if c == 0 else pool.tile(
                        [128, WG], f32, tag="gxT", name=f"gx{c}{q}")
                    gyt = sgyq if c == 0 else pool.tile(
                        [128, WG], f32, tag="gyT", name=f"gy{c}{q}")
                    gx = gxt[:]
                    gy = gyt[:]
                    wt2 = pool.tile([128, WB - 1], f32, tag="wt2", name=f"w2_{c}{q}")
                    for (b0, bw) in BLOCKS:
                        pt1 = psp.tile([128, bw], f32, tag="pT1", name=f"pt1_{c}{q}{b0}")
                        nc.tensor.matmul(pt1[:], wt_[0][:], main_src[:, b0:b0 + bw],
                                         start=True, stop=False)
                        nc.tensor.matmul(pt1[:], wt_[wia][0:halo_k, :],
                                         halo_src[0:halo_k, b0:b0 + bw],
                                         start=False, stop=True)
                        S.copy(S1[:, b0:b0 + bw], pt1[:])
                        pt2 = psp.tile([128, bw], f32, tag="pT2", name=f"pt2_{c}{q}{b0}")
                        nc.tensor.matmul(pt2[:], wt_[3][:], main_src[:, b0:b0 + bw],
                                         start=True, stop=False)
                        nc.tensor.matmul(pt2[:], wt_[wib][0:halo_k, :],
                                         halo_src[0:halo_k, b0:b0 + bw],
                                         start=False, stop=True)
                        S.copy(S2[:, b0:b0 + bw], pt2[:])
                        # hsobel for the column range this block completes
                        g0 = max(0, b0 - 2)
                        g1 = min(WG, b0 + bw - 2)
                        u0 = max(0, b0 - 1)
                        u1 = min(WB - 1, b0 + bw - 1)
                        V.tensor_tensor(gxt[:, g0:g1], S1[:, g0:g1],
                                        S1[:, g0 + 2:g1 + 2], Alu.subtract)
                        V.tensor_tensor(wt2[:, u0:u1], S2[:, u0:u1],
                                        S2[:, u0 + 1:u1 + 1], Alu.add)
                        V.tensor_tensor(gyt[:, g0:g1], wt2[:, g0:g1],
                                        wt2[:, g0 + 1:g1 + 1], Alu.add)
                    # final sliver: gx/gy cols [WB-8..WG) done, but blocks end at
                    # b0+bw-2 = WB-2 = WG+2 > WG, so full range is covered.
                    S.activation(S1[:, 0:WG], gx, Act.Square)
                    S.activation(S2[:, 0:WG], gy, Act.Square)
                    V.tensor_tensor(S1[:, 0:WG], S1[:, 0:WG], S2[:, 0:WG], Alu.add)
                    m = gq[:] if c == 0 else S2[:, 0:WG]
                    S.activation(m, S1[:, 0:WG], Act.Sqrt)
                    if c != 0:
                        V.tensor_tensor(gq[:], gq[:], m, Alu.add)
                        V.tensor_tensor(sgxq[:], sgxq[:], gx, Alu.add)
                        V.tensor_tensor(sgyq[:], sgyq[:], gy, Alu.add)
                    if c == 2:
                        gU = pool.tile([128, WG], f32, tag="gU", name=f"gU{q}")
                        gD = pool.tile([128, WG], f32, tag="gD", name=f"gD{q}")
                        for sb in range(0, WG, 512):
                            sw = min(512, WG - sb)
                            pu = psp.tile([128, sw], f32, tag="pT1", name=f"pu{q}{sb}")
                            nc.tensor.matmul(pu[:], wt_[6][:], gq[:, sb:sb + sw],
                                             start=True, stop=True)
                            S.copy(gU[:, sb:sb + sw], pu[:])
                            pd = psp.tile([128, sw], f32, tag="pT2", name=f"pd{q}{sb}")
                            nc.tensor.matmul(pd[:], wt_[7][:], gq[:, sb:sb + sw],
                                             start=True, stop=True)
                            S.copy(gD[:, sb:sb + sw], pd[:])
                        shifts[q] = (gU, gD)

            # ---- NMS per chunk ----
            for qi, q in enumerate(("a", "b")):
                gq, sgxq, sgyq = acc[q]
                rr = pool.tile([128, WG], f32, tag="bhA", name=f"rr{q}")
                ss = pool.tile([128, WG], f32, tag="bhB", name=f"ss{q}")
                m0 = pool.tile([128, WG], u8, tag="mk0", name=f"m0{q}")
                m2 = pool.tile([128, WG], u8, tag="mk1", name=f"m2{q}")
                d = pool.tile([128, WG], f32, tag="bhX", name=f"d{q}")
                dpos = pool.tile([128, WG], u8, tag="mk2", name=f"dp{q}")
                S.activation(rr[:], sgyq[:], Act.Abs)
                S.activation(ss[:], sgxq[:], Act.Abs)
                V.scalar_tensor_tensor(m0[:], ss[:], t1c, rr[:], Alu.mult, Alu.is_ge)
                V.scalar_tensor_tensor(m2[:], ss[:], t2c, rr[:], Alu.mult, Alu.is_le)
                V.tensor_tensor(d[:], sgxq[:], sgyq[:], Alu.mult)
                V.tensor_scalar(dpos[:], d[:], 0.0, None, Alu.is_ge)

                gU, gD = shifts[q]

                cand1 = pool.tile([128, WC], f32, tag="wt2", name=f"c1{q}")
                cc = pool.tile([128, WC], f32, tag="gxT", name=f"cc{q}")
                cand2 = pool.tile([128, WC], f32, tag="gyT", name=f"c2{q}")
                V.tensor_tensor(cand1[:], gU[:, 2:WG], gD[:, 0:WC], Alu.max)   # SE/NW
                V.tensor_tensor(cc[:], gU[:, 0:WC], gD[:, 2:WG], Alu.max)      # SW/NE
                V.copy_predicated(cc[:], dpos[:, 1:WC + 1], cand1[:])
                V.tensor_tensor(cand2[:], gU[:, 1:WC + 1], gD[:, 1:WC + 1], Alu.max)  # S/N
                V.copy_predicated(cc[:], m2[:, 1:WC + 1], cand2[:])
                cand0 = pool.tile([128, WC], f32, tag="wt2", name=f"c0{q}")
                V.tensor_tensor(cand0[:], gq[:, 2:WG], gq[:, 0:WC], Alu.max)   # E/W
                V.copy_predicated(cc[:], m0[:, 1:WC + 1], cand0[:])

                hp = pool.tile([128, WC], f16, tag="hp", name=f"hp{q}")
                lm = pool.tile([128, W], f16, tag="lm", name=f"lm{q}")
                V.scalar_tensor_tensor(hp[:], cc[:], high, gq[:, 1:WC + 1],
                                       Alu.max, Alu.is_lt)
                V.scalar_tensor_tensor(lm[:], cc[:, 1:W + 1], lowx, gq[:, 2:W + 2],
                                       Alu.max, Alu.is_lt)

                rm1 = pool.tile([128, W], f16, tag="rm1", name=f"rm1{q}")
                rm = pool.tile([128, W], f16, tag="rm", name=f"rm{q}")
                V.tensor_tensor(rm1[:], hp[:, 0:W], hp[:, 2:W + 2], Alu.max)
                V.tensor_tensor(rm[:], rm1[:], hp[:, 1:W + 1], Alu.max)
                # vertical 3-row OR via tridiagonal-ones matmul on PE
                zp = pool.tile([128, W], f16, tag="rm1", name=f"zp{q}")
                for zb in range(0, W, 512):
                    pz = psp.tile([128, 512], f32, tag="pT1", name=f"pz{q}{zb}")
                    nc.tensor.matmul(pz[:], m3t[:], rm[:, zb:zb + 512],
                                     start=True, stop=True)
                    S.copy(zp[:, zb:zb + 512], pz[:])
                outt = iop.tile([128, W], f16, tag="out", name=f"out{q}")
                V.scalar_tensor_tensor(outt[:], zp[:], 0.5, lm[:],
                                       Alu.is_ge, Alu.mult)
                r0 = 0 if q == "a" else 124
                nc.sync.dma_start(
                    out=bass.AP(out, r0 * W, [[W, 124], [1, W]]),
                    in_=outt[2:126, :])

    nc.finalize()
    return nc


def _get_compiled(low, high):
    key = (low, high)
    if key not in _COMPILED:
        _COMPILED[key] = _build(low, high)
    return _COMPILED[key]


def kernel(img, threshold1, threshold2, _trace=False):
    from concourse import bass_utils

    t1 = float(np.asarray(threshold1))
    t2 = float(np.asarray(threshold2))
    low, high = min(t1, t2), max(t1, t2)

    x = np.ascontiguousarray(np.asarray(img, dtype=np.float32)[0])  # [3,H,W]
    xp = np.zeros((3, H + 2 * HALO, W + 2 * HALO), dtype=np.float32)
    xp[:, HALO:HALO + H, HALO:HALO + W] = x

    wmv = _weights()
    m3v = np.zeros((128, 128), np.float16)
    for p in range(128):
        for j in range(max(0, p - 1), min(128, p + 2)):
            m3v[p, j] = 1.0
    win = np.lib.stride_tricks.sliding_window_view(xp, 26, axis=2)[:, :, ::16, :]
    in_maps = []
    for k in range(8):
        rows = np.ascontiguousarray(xp[:, k * RPC:k * RPC + SHARD_ROWS, :])
        tw = win[:, k * RPC + NR:k * RPC + NR + 18]          # [3,18,128,26]
        packed = np.ascontiguousarray(tw.transpose(0, 2, 1, 3))
        in_maps.append({"xr": rows, "xt": packed, "wm": wmv, "wm16": m3v})

    nc = _get_compiled(low, high)
    res = bass_utils.run_bass_kernel_spmd(nc, in_maps, core_ids=list(range(8)),
                                          trace=_trace)

    full = np.zeros((1, 1, H, W), dtype=np.float32)
    for k in range(8):
        o = res.results[k]["out"]          # [248, 2048] f16
        ot = res.results[k]["out_t"]       # [128, 8, 16] f32
        full[0, 0, k * RPC:k * RPC + NR, :] = o.astype(np.float32)
        full[0, 0, k * RPC + NR:(k + 1) * RPC, :] = (
            ot.transpose(1, 0, 2).reshape(NT, W))
    full[:, :, 0, :] = 0.0
    full[:, :, -1, :] = 0.0
    full[:, :, :, 0] = 0.0
    full[:, :, :, -1] = 0.0
    if _trace:
        kernel._last_results = res
    return full


# revision 6
# speedup vs baseline: 1.6582x; 1.0056x over previous
"""Canny v3: rows-on-partitions + PE band-matmuls for vertical convs.

Per core (256 out rows, shard rows [-5,261) zero-padded):
 - two rows-mode chunks: chunk a = T/g rows [-2,126) -> out [0,124),
   chunk b = T rows [122,250) -> out [124,248).  Partition = image row,
   free dim = full 2054/2052-wide rows, so every DVE op covers 128 rows
   at ~2054 elems (vs ~2900 elems per 128 rows in the columns layout).
 - vertical (gaussian o sobel) 7-tap convs fused into PE matmuls with
   band weight matrices; halo rows accumulate from the neighboring bh
   tile via a second matmul into the same PSUM bank.
 - NMS vertical/diagonal neighbors via SBUF->SBUF DMA row-shifted
   copies of the g plane (and of rm for hysteresis).
 - last 8 rows per core done in a small v1-style columns-mode pass.
"""

import numpy as np

_COMPILED = {}

H = 2048
W = 2048
HALO = 5
RPC = H // 8                      # 256 rows per core
SHARD_ROWS = RPC + 2 * HALO       # 266
WPAD = W + 2 * HALO               # 2058
WB = W + 6                        # 2054: bh/T cols [-3, 2051)
WG = W + 4                        # 2052: g/gx/gy cols [-2, 2050)
WC = W + 2                        # 2050: cand/cc/hp cols [-1, 2049)
NT = 14                           # tail rows (columns-mode)
NR = RPC - NT                     # 248 rows via rows-mode
BLOCKS = [(0, 512), (512, 512), (1024, 512), (1536, 512), (2048, WB - 2048)]


def _weights():
    g5 = np.exp(-0.5 * (np.arange(5) - 2.0) ** 2).astype(np.float32)
    w7a = np.convolve(np.array([1, 2, 1], np.float32), g5).astype(np.float32)
    w7b = np.convolve(np.array([1, 0, -1], np.float32), g5).astype(np.float32)

    def w7i(w7, i):
        return float(w7[i]) if 0 <= i < 7 else 0.0

    wm = np.zeros((8, 128, 128), np.float32)
    for p in range(128):
        for j in range(128):
            wm[0, p, j] = w7i(w7a, p - j)          # Wa_main
            wm[3, p, j] = w7i(w7b, p - j)          # Wb_main
            if 4 <= p < 10:
                wm[1, p, j] = w7i(w7a, p + 124 - j)  # Wa_h1 (chunk a halo from bh_b)
                wm[4, p, j] = w7i(w7b, p + 124 - j)  # Wb_h1
            if p < 6:
                wm[2, p, j] = w7i(w7a, p + 128 - j)  # Wa_hx (chunk b halo from bhx)
                wm[5, p, j] = w7i(w7b, p + 128 - j)  # Wb_hx
            if p == j + 1:
                wm[6, p, j] = 1.0                    # SU: gU[j] = g[j+1]
            if p == j - 1:
                wm[7, p, j] = 1.0                    # SD: gD[j] = g[j-1]
    return wm


def _build(low, high):
    import concourse.bass as bass
    import concourse.bacc as bacc
    import concourse.mybir as mybir
    from concourse.tile import TileContext

    f32 = mybir.dt.float32
    f16 = mybir.dt.float16
    u8 = mybir.dt.uint8
    Alu = mybir.AluOpType
    Act = mybir.ActivationFunctionType

    g5 = np.exp(-0.5 * (np.arange(5) - 2.0) ** 2).astype(np.float32)
    ga = float(g5[0])
    gb = float(g5[1])
    t1c = float(np.float32(np.tan(np.deg2rad(np.float64(22.5)))))
    t2c = float(np.float32(np.tan(np.deg2rad(np.float64(67.5)))))
    lowx = float(np.nextafter(np.float32(low), np.float32(0.0)))

    nc = bacc.Bacc()
    xr = nc.dram_tensor("xr", [3, SHARD_ROWS, WPAD], f32, kind="ExternalInput")
    xt = nc.dram_tensor("xt", [3, 128, 24, 26], f32, kind="ExternalInput")
    wm = nc.dram_tensor("wm", [8, 128, 128], f32, kind="ExternalInput")
    wm16 = nc.dram_tensor("wm16", [128, 128], f16, kind="ExternalInput")
    out = nc.dram_tensor("out", [NR, W], f16, kind="ExternalOutput")
    out_t = nc.dram_tensor("out_t", [128, NT, 16], f32, kind="ExternalOutput")

    with TileContext(nc) as tc:
        with tc.tile_pool(name="io", bufs=2) as iop, \
             tc.tile_pool(name="pl", bufs=1) as pool, \
             tc.tile_pool(name="ps", bufs=2, space="PSUM") as psp:

            V = nc.vector
            S = nc.scalar

            wt_ = []
            for i in range(8):
                wti = pool.tile([128, 128], f32, tag=f"w{i}", name=f"wt{i}")
                nc.sync.dma_start(out=wti[:], in_=bass.AP(wm, i * 128 * 128,
                                                          [[128, 128], [1, 128]]))
                wt_.append(wti)
            m3t = pool.tile([128, 128], f16, tag="m3", name="m3t")
            nc.sync.dma_start(out=m3t[:], in_=bass.AP(wm16, 0, [[128, 128], [1, 128]]))

            # accumulators per chunk
            acc = {}
            for q in ("a", "b"):
                acc[q] = (
                    pool.tile([128, WG], f32, tag=f"g_{q}", name=f"g_{q}"),
                    pool.tile([128, WG], f32, tag=f"sgx_{q}", name=f"sgx_{q}"),
                    pool.tile([128, WG], f32, tag=f"sgy_{q}", name=f"sgy_{q}"),
                )

            # ---- tail: columns-mode v1-style pass for out rows [248,256) ----
            N, R, RV, RT, RN = NT, NT + 10, NT + 6, NT + 4, NT + 2
            tg = pool.tile([128, RT, 20], f32, tag="tg", name="tg")
            tsgx = pool.tile([128, RT, 20], f32, tag="tsgx", name="tsgx")
            tsgy = pool.tile([128, RT, 20], f32, tag="tsgy", name="tsgy")
            for c in range(3):
                img = pool.tile([128, R, 26], f32, tag="timg", name=f"timg{c}")
                nc.sync.dma_start(out=img[:], in_=bass.AP(
                    xt, c * 128 * 24 * 26, [[24 * 26, 128], [26, R], [1, 26]]))
                tA = pool.tile([128, R, 22], f32, tag="tlA", name=f"tlA{c}")
                tB = pool.tile([128, R, 22], f32, tag="tlB", name=f"tlB{c}")
                tC = pool.tile([128, R, 22], f32, tag="tlC", name=f"tlC{c}")
                tD = pool.tile([128, RV, 22], f32, tag="tlD", name=f"tlD{c}")
                V.tensor_tensor(tA[:], img[:, :, 1:23], img[:, :, 3:25], Alu.add)
                V.tensor_tensor(tB[:], img[:, :, 0:22], img[:, :, 4:26], Alu.add)
                V.scalar_tensor_tensor(tC[:], tA[:], gb, img[:, :, 2:24], Alu.mult, Alu.add)
                V.scalar_tensor_tensor(tC[:], tB[:], ga, tC[:], Alu.mult, Alu.add)
                V.tensor_tensor(tA[:, 0:RV, :], tC[:, 1:RV + 1, :], tC[:, 3:RV + 3, :], Alu.add)
                V.tensor_tensor(tB[:, 0:RV, :], tC[:, 0:RV, :], tC[:, 4:RV + 4, :], Alu.add)
                V.scalar_tensor_tensor(tD[:], tA[:, 0:RV, :], gb, tC[:, 2:RV + 2, :],
                                       Alu.mult, Alu.add)
                V.scalar_tensor_tensor(tD[:], tB[:, 0:RV, :], ga, tD[:], Alu.mult, Alu.add)
                V.tensor_tensor(tA[:, 0:RT, :], tD[:, 0:RT, :], tD[:, 2:RT + 2, :], Alu.add)
                V.scalar_tensor_tensor(tA[:, 0:RT, :], tD[:, 1:RT + 1, :], 2.0,
                                       tA[:, 0:RT, :], Alu.mult, Alu.add)   # t1
                V.tensor_tensor(tB[:, 0:RT, :], tD[:, 0:RT, :], tD[:, 2:RT + 2, :],
                                Alu.subtract)                                # t2
                gx = tsgx[:] if c == 0 else tC[:, 0:RT, 0:20]
                gy = tsgy[:] if c == 0 else tD[:, 0:RT, 0:20]
                w2 = tD[:, 0:RT, 0:20]
                V.tensor_tensor(gx, tA[:, 0:RT, 0:20], tA[:, 0:RT, 2:22], Alu.subtract)
                V.tensor_tensor(w2, tB[:, 0:RT, 0:20], tB[:, 0:RT, 2:22], Alu.add)
                V.scalar_tensor_tensor(gy, tB[:, 0:RT, 1:21], 2.0, w2, Alu.mult, Alu.add)
                q1 = tA[:, 0:RT, 0:20]
                q2 = tB[:, 0:RT, 0:20]
                S.activation(q1, gx, Act.Square)
                S.activation(q2, gy, Act.Square)
                V.tensor_tensor(q1, q1, q2, Alu.add)
                m = tg[:] if c == 0 else tB[:, 0:RT, 0:20]
                S.activation(m, q1, Act.Sqrt)
                if c != 0:
                    V.tensor_tensor(tg[:], tg[:], m, Alu.add)
                    V.tensor_tensor(tsgx[:], tsgx[:], gx, Alu.add)
                    V.tensor_tensor(tsgy[:], tsgy[:], gy, Alu.add)

            rr = pool.tile([128, RN, 18], f32, tag="tlA", name="trr")
            ss = pool.tile([128, RN, 18], f32, tag="tlB", name="tss")
            m0 = pool.tile([128, RN, 18], u8, tag="tmk0", name="tm0")
            m2 = pool.tile([128, RN, 18], u8, tag="tmk1", name="tm2")
            d = pool.tile([128, RN, 18], f32, tag="tlC", name="td")
            dpos = pool.tile([128, RN, 18], u8, tag="tmk2", name="tdp")
            S.activation(rr[:], tsgy[:, 1:RN + 1, 1:19], Act.Abs)
            S.activation(ss[:], tsgx[:, 1:RN + 1, 1:19], Act.Abs)
            V.scalar_tensor_tensor(m0[:], ss[:], t1c, rr[:], Alu.mult, Alu.is_ge)
            V.scalar_tensor_tensor(m2[:], ss[:], t2c, rr[:], Alu.mult, Alu.is_le)
            V.tensor_tensor(d[:], tsgx[:, 1:RN + 1, 1:19], tsgy[:, 1:RN + 1, 1:19], Alu.mult)
            V.tensor_scalar(dpos[:], d[:], 0.0, None, Alu.is_ge)
            cand = pool.tile([128, RN, 18], f32, tag="tlD", name="tc1")
            cc = pool.tile([128, RN, 18], f32, tag="tlA", name="tcc")
            cand2 = pool.tile([128, RN, 18], f32, tag="tlB", name="tc2")
            cand3 = pool.tile([128, RN, 18], f32, tag="tlC", name="tc0")
            V.tensor_tensor(cand[:], tg[:, 2:RN + 2, 2:20], tg[:, 0:RN, 0:18], Alu.max)
            V.tensor_tensor(cc[:], tg[:, 2:RN + 2, 0:18], tg[:, 0:RN, 2:20], Alu.max)
            V.copy_predicated(cc[:], dpos[:], cand[:])
            V.tensor_tensor(cand2[:], tg[:, 2:RN + 2, 1:19], tg[:, 0:RN, 1:19], Alu.max)
            V.copy_predicated(cc[:], m2[:], cand2[:])
            V.tensor_tensor(cand3[:], tg[:, 1:RN + 1, 2:20], tg[:, 1:RN + 1, 0:18], Alu.max)
            V.copy_predicated(cc[:], m0[:], cand3[:])
            hp = pool.tile([128, RN, 18], f16, tag="thp", name="thp")
            lm = pool.tile([128, N, 16], f32, tag="tlC", name="tlm")
            V.scalar_tensor_tensor(hp[:], cc[:], high, tg[:, 1:RN + 1, 1:19],
                                   Alu.max, Alu.is_lt)
            V.scalar_tensor_tensor(lm[:], cc[:, 1:N + 1, 1:17], lowx,
                                   tg[:, 2:RN, 2:18], Alu.max, Alu.is_lt)
            rm1 = pool.tile([128, RN, 16], f16, tag="trm1", name="trm1")
            rm = pool.tile([128, RN, 16], f16, tag="trm", name="trm")
            cm = pool.tile([128, N, 16], f16, tag="tcm", name="tcm")
            V.tensor_tensor(rm1[:], hp[:, :, 0:16], hp[:, :, 2:18], Alu.max)
            V.tensor_tensor(rm[:], rm1[:], hp[:, :, 1:17], Alu.max)
            V.tensor_tensor(cm[:], rm[:, 0:N, :], rm[:, 2:RN, :], Alu.max)
            V.tensor_tensor(cm[:], cm[:], rm[:, 1:N + 1, :], Alu.max)
            outt = pool.tile([128, N, 16], f32, tag="tlD", name="touts")
            V.tensor_tensor(outt[:], lm[:], cm[:], Alu.mult)
            nc.sync.dma_start(
                out=bass.AP(out_t, 0, [[NT * 16, 128], [16, N], [1, 16]]),
                in_=outt[:])


            shifts = {}
            for c in range(3):
                # ---- hgauss for the three bh tiles of this channel ----
                bhs = {}
                for (nm, row0, nrows, tag) in (
                        ("a", 0, 128, "bhA"), ("b", 124, 128, "bhB")):
                    img = iop.tile([128, WPAD], f32, tag="img", name=f"img{c}{nm}")
                    off0 = c * SHARD_ROWS * WPAD + row0 * WPAD
                    nc.sync.dma_start(
                        out=img[0:nrows, :],
                        in_=bass.AP(xr, off0, [[WPAD, nrows], [343, 6], [1, 343]]))
                    s1 = pool.tile([128, WB], f32, tag="S1", name=f"s1_{c}{nm}")
                    s2 = pool.tile([128, WB], f32, tag="S2", name=f"s2_{c}{nm}")
                    bh = pool.tile([128, WB], f32, tag=tag, name=f"bh{nm}{c}")
                    r = slice(0, nrows)
                    V.tensor_tensor(s1[r, :], img[r, 1:WB + 1], img[r, 3:WB + 3], Alu.add)
                    V.tensor_tensor(s2[r, :], img[r, 0:WB], img[r, 4:WB + 4], Alu.add)
                    V.scalar_tensor_tensor(bh[r, :], s1[r, :], gb, img[r, 2:WB + 2],
                                           Alu.mult, Alu.add)
                    V.scalar_tensor_tensor(bh[r, :], s2[r, :], ga, bh[r, :],
                                           Alu.mult, Alu.add)
                    bhs[nm] = bh

                # ---- per chunk: PE T1/T2, then hsobel + mag on DVE ----
                for q in ("a", "b"):
                    if q == "a":
                        main_src, halo_src, halo_k, wia, wib = bhs["a"], bhs["b"], 128, 1, 4
                    else:
                        main_src, halo_src, halo_k, wia, wib = bhs["b"], None, 0, 2, 5
                    S1 = pool.tile([128, WB], f32, tag="S1", name=f"S1_{c}{q}")
                    S2 = pool.tile([128, WB], f32, tag="S2", name=f"S2_{c}{q}")
                    gq, sgxq, sgyq = acc[q]
                    gxt = sgxq if c == 0 else pool.tile(
                        [128, WG], f32, tag="gxT", name=f"gx{c}{q}")
                    gyt = sgyq if c == 0 else pool.tile(
                        [128, WG], f32, tag="gyT", name=f"gy{c}{q}")
                    gx = gxt[:]
                    gy = gyt[:]
                    wt2 = pool.tile([128, WB - 1], f32, tag="wt2", name=f"w2_{c}{q}")
                    for (b0, bw) in BLOCKS:
                        has_halo = halo_src is not None
                        pt1 = psp.tile([128, bw], f32, tag="pT1", name=f"pt1_{c}{q}{b0}")
                        nc.tensor.matmul(pt1[:], wt_[0][:], main_src[:, b0:b0 + bw],
                                         start=True, stop=not has_halo)
                        if has_halo:
                            nc.tensor.matmul(pt1[:], wt_[wia][0:halo_k, :],
                                             halo_src[0:halo_k, b0:b0 + bw],
                                             start=False, stop=True)
                        S.copy(S1[:, b0:b0 + bw], pt1[:])
                        pt2 = psp.tile([128, bw], f32, tag="pT2", name=f"pt2_{c}{q}{b0}")
                        nc.tensor.matmul(pt2[:], wt_[3][:], main_src[:, b0:b0 + bw],
                                         start=True, stop=not has_halo)
                        if has_halo:
                            nc.tensor.matmul(pt2[:], wt_[wib][0:halo_k, :],
                                             halo_src[0:halo_k, b0:b0 + bw],
                                             start=False, stop=True)
                        S.copy(S2[:, b0:b0 + bw], pt2[:])
                        # hsobel for the column range this block completes
                        g0 = max(0, b0 - 2)
                        g1 = min(WG, b0 + bw - 2)
                        u0 = max(0, b0 - 1)
                        u1 = min(WB - 1, b0 + bw - 1)
                        V.tensor_tensor(gxt[:, g0:g1], S1[:, g0:g1],
                                        S1[:, g0 + 2:g1 + 2], Alu.subtract)
                        V.tensor_tensor(wt2[:, u0:u1], S2[:, u0:u1],
                                        S2[:, u0 + 1:u1 + 1], Alu.add)
                        V.tensor_tensor(gyt[:, g0:g1], wt2[:, g0:g1],
                                        wt2[:, g0 + 1:g1 + 1], Alu.add)
                    # final sliver: gx/gy cols [WB-8..WG) done, but blocks end at
                    # b0+bw-2 = WB-2 = WG+2 > WG, so full range is covered.
                    S.activation(S1[:, 0:WG], gx, Act.Square)
                    S.activation(S2[:, 0:WG], gy, Act.Square)
                    V.tensor_tensor(S1[:, 0:WG], S1[:, 0:WG], S2[:, 0:WG], Alu.add)
                    m = gq[:] if c == 0 else S2[:, 0:WG]
                    S.activation(m, S1[:, 0:WG], Act.Sqrt)
                    if c != 0:
                        V.tensor_tensor(gq[:], gq[:], m, Alu.add)
                        V.tensor_tensor(sgxq[:], sgxq[:], gx, Alu.add)
                        V.tensor_tensor(sgyq[:], sgyq[:], gy, Alu.add)
                    if c == 2:
                        gU = pool.tile([128, WG], f32, tag="gU", name=f"gU{q}")
                        gD = pool.tile([128, WG], f32, tag="gD", name=f"gD{q}")
                        for sb in range(0, WG, 512):
                            sw = min(512, WG - sb)
                            pu = psp.tile([128, sw], f32, tag="pT1", name=f"pu{q}{sb}")
                            nc.tensor.matmul(pu[:], wt_[6][:], gq[:, sb:sb + sw],
                                             start=True, stop=True)
                            S.copy(gU[:, sb:sb + sw], pu[:])
                            pd = psp.tile([128, sw], f32, tag="pT2", name=f"pd{q}{sb}")
                            nc.tensor.matmul(pd[:], wt_[7][:], gq[:, sb:sb + sw],
                                             start=True, stop=True)
                            S.copy(gD[:, sb:sb + sw], pd[:])
                        shifts[q] = (gU, gD)

            # ---- NMS per chunk ----
            for qi, q in enumerate(("a", "b")):
                gq, sgxq, sgyq = acc[q]
                rr = pool.tile([128, WG], f32, tag="bhA", name=f"rr{q}")
                ss = pool.tile([128, WG], f32, tag="bhB", name=f"ss{q}")
                m0 = pool.tile([128, WG], u8, tag="mk0", name=f"m0{q}")
                m2 = pool.tile([128, WG], u8, tag="mk1", name=f"m2{q}")
                d = pool.tile([128, WG], f32, tag="bhX", name=f"d{q}")
                dpos = pool.tile([128, WG], u8, tag="mk2", name=f"dp{q}")
                S.activation(rr[:], sgyq[:], Act.Abs)
                S.activation(ss[:], sgxq[:], Act.Abs)
                V.scalar_tensor_tensor(m0[:], ss[:], t1c, rr[:], Alu.mult, Alu.is_ge)
                V.scalar_tensor_tensor(m2[:], ss[:], t2c, rr[:], Alu.mult, Alu.is_le)
                V.tensor_tensor(d[:], sgxq[:], sgyq[:], Alu.mult)
                V.tensor_scalar(dpos[:], d[:], 0.0, None, Alu.is_ge)

                gU, gD = shifts[q]

                cand1 = pool.tile([128, WC], f32, tag="wt2", name=f"c1{q}")
                cc = pool.tile([128, WC], f32, tag="gxT", name=f"cc{q}")
                cand2 = pool.tile([128, WC], f32, tag="gyT", name=f"c2{q}")
                V.tensor_tensor(cand1[:], gU[:, 2:WG], gD[:, 0:WC], Alu.max)   # SE/NW
                V.tensor_tensor(cc[:], gU[:, 0:WC], gD[:, 2:WG], Alu.max)      # SW/NE
                V.copy_predicated(cc[:], dpos[:, 1:WC + 1], cand1[:])
                V.tensor_tensor(cand2[:], gU[:, 1:WC + 1], gD[:, 1:WC + 1], Alu.max)  # S/N
                V.copy_predicated(cc[:], m2[:, 1:WC + 1], cand2[:])
                cand0 = pool.tile([128, WC], f32, tag="wt2", name=f"c0{q}")
                V.tensor_tensor(cand0[:], gq[:, 2:WG], gq[:, 0:WC], Alu.max)   # E/W
                V.copy_predicated(cc[:], m0[:, 1:WC + 1], cand0[:])

                hp = pool.tile([128, WC], f16, tag="hp", name=f"hp{q}")
                lm = pool.tile([128, W], f16, tag="lm", name=f"lm{q}")
                V.scalar_tensor_tensor(hp[:], cc[:], high, gq[:, 1:WC + 1],
                                       Alu.max, Alu.is_lt)
                V.scalar_tensor_tensor(lm[:], cc[:, 1:W + 1], lowx, gq[:, 2:W + 2],
                                       Alu.max, Alu.is_lt)

                rm1 = pool.tile([128, W], f16, tag="rm1", name=f"rm1{q}")
                rm = pool.tile([128, W], f16, tag="rm", name=f"rm{q}")
                V.tensor_tensor(rm1[:], hp[:, 0:W], hp[:, 2:W + 2], Alu.max)
                V.tensor_tensor(rm[:], rm1[:], hp[:, 1:W + 1], Alu.max)
                # vertical 3-row OR via tridiagonal-ones matmul on PE
                zp = pool.tile([128, W], f16, tag="rm1", name=f"zp{q}")
                for zb in range(0, W, 512):
                    pz = psp.tile([128, 512], f32, tag="pT1", name=f"pz{q}{zb}")
                    nc.tensor.matmul(pz[:], m3t[:], rm[:, zb:zb + 512],
                                     start=True, stop=True)
                    S.copy(zp[:, zb:zb + 512], pz[:])
                outt = iop.tile([128, W], f16, tag="out", name=f"out{q}")
                V.scalar_tensor_tensor(outt[:], zp[:], 0.5, lm[:],
                                       Alu.is_ge, Alu.mult)
                r0, nrows_out = (0, 124) if q == "a" else (124, NR - 124)
                nc.sync.dma_start(
                    out=bass.AP(out, r0 * W, [[W, nrows_out], [1, W]]),
                    in_=outt[2:2 + nrows_out, :])

    nc.finalize()
    return nc


def _get_compiled(low, high):
    key = (low, high)
    if key not in _COMPILED:
        _COMPILED[key] = _build(low, high)
    return _COMPILED[key]


def kernel(img, threshold1, threshold2, _trace=False):
    from concourse import bass_utils

    t1 = float(np.asarray(threshold1))
    t2 = float(np.asarray(threshold2))
    low, high = min(t1, t2), max(t1, t2)

    x = np.ascontiguousarray(np.asarray(img, dtype=np.float32)[0])  # [3,H,W]
    xp = np.zeros((3, H + 2 * HALO, W + 2 * HALO), dtype=np.float32)
    xp[:, HALO:HALO + H, HALO:HALO + W] = x

    wmv = _weights()
    m3v = np.zeros((128, 128), np.float16)
    for p in range(128):
        for j in range(max(0, p - 1), min(128, p + 2)):
            m3v[p, j] = 1.0
    win = np.lib.stride_tricks.sliding_window_view(xp, 26, axis=2)[:, :, ::16, :]
    in_maps = []
    for k in range(8):
        rows = np.ascontiguousarray(xp[:, k * RPC:k * RPC + SHARD_ROWS, :])
        tw = win[:, k * RPC + NR:k * RPC + NR + NT + 10]     # [3,24,128,26]
        packed = np.ascontiguousarray(tw.transpose(0, 2, 1, 3))
        in_maps.append({"xr": rows, "xt": packed, "wm": wmv, "wm16": m3v})

    nc = _get_compiled(low, high)
    res = bass_utils.run_bass_kernel_spmd(nc, in_maps, core_ids=list(range(8)),
                                          trace=_trace)

    full = np.zeros((1, 1, H, W), dtype=np.float32)
    for k in range(8):
        o = res.results[k]["out"]          # [248, 2048] f16
        ot = res.results[k]["out_t"]       # [128, 8, 16] f32
        full[0, 0, k * RPC:k * RPC + NR, :] = o.astype(np.float32)
        full[0, 0, k * RPC + NR:(k + 1) * RPC, :] = (
            ot.transpose(1, 0, 2).reshape(NT, W))
    full[:, :, 0, :] = 0.0
    full[:, :, -1, :] = 0.0
    full[:, :, :, 0] = 0.0
    full[:, :, :, -1] = 0.0
    if _trace:
        kernel._last_results = res
    return full


# revision 7
# speedup vs baseline: 1.7058x; 1.0287x over previous
"""Canny v3: rows-on-partitions + PE band-matmuls for vertical convs.

Per core (256 out rows, shard rows [-5,261) zero-padded):
 - two rows-mode chunks: chunk a = T/g rows [-2,126) -> out [0,124),
   chunk b = T rows [122,250) -> out [124,248).  Partition = image row,
   free dim = full 2054/2052-wide rows, so every DVE op covers 128 rows
   at ~2054 elems (vs ~2900 elems per 128 rows in the columns layout).
 - vertical (gaussian o sobel) 7-tap convs fused into PE matmuls with
   band weight matrices; halo rows accumulate from the neighboring bh
   tile via a second matmul into the same PSUM bank.
 - NMS vertical/diagonal neighbors via SBUF->SBUF DMA row-shifted
   copies of the g plane (and of rm for hysteresis).
 - last 8 rows per core done in a small v1-style columns-mode pass.
"""

import numpy as np

_COMPILED = {}

H = 2048
W = 2048
HALO = 5
RPC = H // 8                      # 256 rows per core
SHARD_ROWS = RPC + 2 * HALO       # 266
WPAD = W + 2 * HALO               # 2058
WB = W + 6                        # 2054: bh/T cols [-3, 2051)
WG = W + 4                        # 2052: g/gx/gy cols [-2, 2050)
WC = W + 2                        # 2050: cand/cc/hp cols [-1, 2049)
NT = 14                           # tail rows (columns-mode)
NR = RPC - NT                     # 248 rows via rows-mode
BLOCKS = [(0, 512), (512, 512), (1024, 512), (1536, 512), (2048, WB - 2048)]


def _weights():
    g5 = np.exp(-0.5 * (np.arange(5) - 2.0) ** 2).astype(np.float32)
    w7a = np.convolve(np.array([1, 2, 1], np.float32), g5).astype(np.float32)
    w7b = np.convolve(np.array([1, 0, -1], np.float32), g5).astype(np.float32)

    def w7i(w7, i):
        return float(w7[i]) if 0 <= i < 7 else 0.0

    wm = np.zeros((8, 128, 128), np.float32)
    for p in range(128):
        for j in range(128):
            wm[0, p, j] = w7i(w7a, p - j)          # Wa_main
            wm[3, p, j] = w7i(w7b, p - j)          # Wb_main
            if 4 <= p < 10:
                wm[1, p, j] = w7i(w7a, p + 124 - j)  # Wa_h1 (chunk a halo from bh_b)
                wm[4, p, j] = w7i(w7b, p + 124 - j)  # Wb_h1
            if p < 6:
                wm[2, p, j] = w7i(w7a, p + 128 - j)  # Wa_hx (chunk b halo from bhx)
                wm[5, p, j] = w7i(w7b, p + 128 - j)  # Wb_hx
            if p == j + 1:
                wm[6, p, j] = 1.0                    # SU: gU[j] = g[j+1]
            if p == j - 1:
                wm[7, p, j] = 1.0                    # SD: gD[j] = g[j-1]
    return wm


def _build(low, high):
    import concourse.bass as bass
    import concourse.bacc as bacc
    import concourse.mybir as mybir
    from concourse.tile import TileContext

    f32 = mybir.dt.float32
    f16 = mybir.dt.float16
    u8 = mybir.dt.uint8
    Alu = mybir.AluOpType
    Act = mybir.ActivationFunctionType

    g5 = np.exp(-0.5 * (np.arange(5) - 2.0) ** 2).astype(np.float32)
    ga = float(g5[0])
    gb = float(g5[1])
    t1c = float(np.float32(np.tan(np.deg2rad(np.float64(22.5)))))
    t2c = float(np.float32(np.tan(np.deg2rad(np.float64(67.5)))))
    lowx = float(np.nextafter(np.float32(low), np.float32(0.0)))

    nc = bacc.Bacc()
    xr = nc.dram_tensor("xr", [3, SHARD_ROWS, WPAD], f32, kind="ExternalInput")
    xt = nc.dram_tensor("xt", [3, 128, 24, 26], f32, kind="ExternalInput")
    wm = nc.dram_tensor("wm", [8, 128, 128], f32, kind="ExternalInput")
    wm16 = nc.dram_tensor("wm16", [128, 128], f16, kind="ExternalInput")
    out = nc.dram_tensor("out", [NR, W], f16, kind="ExternalOutput")
    out_t = nc.dram_tensor("out_t", [128, NT, 16], f32, kind="ExternalOutput")

    with TileContext(nc) as tc:
        with tc.tile_pool(name="io", bufs=2) as iop, \
             tc.tile_pool(name="pl", bufs=1) as pool, \
             tc.tile_pool(name="ps", bufs=2, space="PSUM") as psp:

            V = nc.vector
            S = nc.scalar

            wt_ = []
            for i in range(8):
                wti = pool.tile([128, 128], f32, tag=f"w{i}", name=f"wt{i}")
                nc.sync.dma_start(out=wti[:], in_=bass.AP(wm, i * 128 * 128,
                                                          [[128, 128], [1, 128]]))
                wt_.append(wti)
            m3t = pool.tile([128, 128], f16, tag="m3", name="m3t")
            nc.sync.dma_start(out=m3t[:], in_=bass.AP(wm16, 0, [[128, 128], [1, 128]]))

            # accumulators per chunk
            acc = {}
            for q in ("a", "b"):
                acc[q] = (
                    pool.tile([128, WG], f32, tag=f"g_{q}", name=f"g_{q}"),
                    pool.tile([128, WG], f32, tag=f"sgx_{q}", name=f"sgx_{q}"),
                    pool.tile([128, WG], f32, tag=f"sgy_{q}", name=f"sgy_{q}"),
                )

            # ---- tail: columns-mode v1-style pass for out rows [248,256) ----
            N, R, RV, RT, RN = NT, NT + 10, NT + 6, NT + 4, NT + 2
            tg = pool.tile([128, RT, 20], f32, tag="tg", name="tg")
            tsgx = pool.tile([128, RT, 20], f32, tag="tsgx", name="tsgx")
            tsgy = pool.tile([128, RT, 20], f32, tag="tsgy", name="tsgy")
            for c in range(3):
                img = pool.tile([128, R, 26], f32, tag="timg", name=f"timg{c}")
                nc.sync.dma_start(out=img[:], in_=bass.AP(
                    xt, c * 128 * 24 * 26, [[24 * 26, 128], [26, R], [1, 26]]))
                tA = pool.tile([128, R, 22], f32, tag="tlA", name=f"tlA{c}")
                tB = pool.tile([128, R, 22], f32, tag="tlB", name=f"tlB{c}")
                tC = pool.tile([128, R, 22], f32, tag="tlC", name=f"tlC{c}")
                tD = pool.tile([128, RV, 22], f32, tag="tlD", name=f"tlD{c}")
                V.tensor_tensor(tA[:], img[:, :, 1:23], img[:, :, 3:25], Alu.add)
                V.tensor_tensor(tB[:], img[:, :, 0:22], img[:, :, 4:26], Alu.add)
                V.scalar_tensor_tensor(tC[:], tA[:], gb, img[:, :, 2:24], Alu.mult, Alu.add)
                V.scalar_tensor_tensor(tC[:], tB[:], ga, tC[:], Alu.mult, Alu.add)
                V.tensor_tensor(tA[:, 0:RV, :], tC[:, 1:RV + 1, :], tC[:, 3:RV + 3, :], Alu.add)
                V.tensor_tensor(tB[:, 0:RV, :], tC[:, 0:RV, :], tC[:, 4:RV + 4, :], Alu.add)
                V.scalar_tensor_tensor(tD[:], tA[:, 0:RV, :], gb, tC[:, 2:RV + 2, :],
                                       Alu.mult, Alu.add)
                V.scalar_tensor_tensor(tD[:], tB[:, 0:RV, :], ga, tD[:], Alu.mult, Alu.add)
                V.tensor_tensor(tA[:, 0:RT, :], tD[:, 0:RT, :], tD[:, 2:RT + 2, :], Alu.add)
                V.scalar_tensor_tensor(tA[:, 0:RT, :], tD[:, 1:RT + 1, :], 2.0,
                                       tA[:, 0:RT, :], Alu.mult, Alu.add)   # t1
                V.tensor_tensor(tB[:, 0:RT, :], tD[:, 0:RT, :], tD[:, 2:RT + 2, :],
                                Alu.subtract)                                # t2
                gx = tsgx[:] if c == 0 else tC[:, 0:RT, 0:20]
                gy = tsgy[:] if c == 0 else tD[:, 0:RT, 0:20]
                w2 = tD[:, 0:RT, 0:20]
                V.tensor_tensor(gx, tA[:, 0:RT, 0:20], tA[:, 0:RT, 2:22], Alu.subtract)
                V.tensor_tensor(w2, tB[:, 0:RT, 0:20], tB[:, 0:RT, 2:22], Alu.add)
                V.scalar_tensor_tensor(gy, tB[:, 0:RT, 1:21], 2.0, w2, Alu.mult, Alu.add)
                q1 = tA[:, 0:RT, 0:20]
                q2 = tB[:, 0:RT, 0:20]
                S.activation(q1, gx, Act.Square)
                S.activation(q2, gy, Act.Square)
                V.tensor_tensor(q1, q1, q2, Alu.add)
                m = tg[:] if c == 0 else tB[:, 0:RT, 0:20]
                S.activation(m, q1, Act.Sqrt)
                if c != 0:
                    V.tensor_tensor(tg[:], tg[:], m, Alu.add)
                    V.tensor_tensor(tsgx[:], tsgx[:], gx, Alu.add)
                    V.tensor_tensor(tsgy[:], tsgy[:], gy, Alu.add)




            shifts = {}
            for c in range(3):
                # ---- hgauss for the three bh tiles of this channel ----
                bhs = {}
                for (nm, row0, nrows, tag) in (
                        ("a", 0, 128, "bhA"), ("b", 124, 128, "bhB")):
                    img = iop.tile([128, WPAD], f32, tag="img", name=f"img{c}{nm}")
                    off0 = c * SHARD_ROWS * WPAD + row0 * WPAD
                    nc.sync.dma_start(
                        out=img[0:nrows, :],
                        in_=bass.AP(xr, off0, [[WPAD, nrows], [343, 6], [1, 343]]))
                    s1 = pool.tile([128, WB], f32, tag="S1", name=f"s1_{c}{nm}")
                    s2 = pool.tile([128, WB], f32, tag="S2", name=f"s2_{c}{nm}")
                    bh = pool.tile([128, WB], f32, tag=tag, name=f"bh{nm}{c}")
                    r = slice(0, nrows)
                    V.tensor_tensor(s1[r, :], img[r, 1:WB + 1], img[r, 3:WB + 3], Alu.add)
                    V.tensor_tensor(s2[r, :], img[r, 0:WB], img[r, 4:WB + 4], Alu.add)
                    V.scalar_tensor_tensor(bh[r, :], s1[r, :], gb, img[r, 2:WB + 2],
                                           Alu.mult, Alu.add)
                    V.scalar_tensor_tensor(bh[r, :], s2[r, :], ga, bh[r, :],
                                           Alu.mult, Alu.add)
                    bhs[nm] = bh

                # ---- per chunk: PE T1/T2, then hsobel + mag on DVE ----
                for q in ("a", "b"):
                    if q == "a":
                        main_src, halo_src, halo_k, wia, wib = bhs["a"], bhs["b"], 128, 1, 4
                    else:
                        main_src, halo_src, halo_k, wia, wib = bhs["b"], None, 0, 2, 5
                    S1 = pool.tile([128, WB], f32, tag="S1", name=f"S1_{c}{q}")
                    S2 = pool.tile([128, WB], f32, tag="S2", name=f"S2_{c}{q}")
                    gq, sgxq, sgyq = acc[q]
                    gxt = sgxq if c == 0 else pool.tile(
                        [128, WG], f32, tag="gxT", name=f"gx{c}{q}")
                    gyt = sgyq if c == 0 else pool.tile(
                        [128, WG], f32, tag="gyT", name=f"gy{c}{q}")
                    gx = gxt[:]
                    gy = gyt[:]
                    wt2 = pool.tile([128, WB - 1], f32, tag="wt2", name=f"w2_{c}{q}")
                    for (b0, bw) in BLOCKS:
                        has_halo = halo_src is not None
                        pt1 = psp.tile([128, bw], f32, tag="pT1", name=f"pt1_{c}{q}{b0}")
                        nc.tensor.matmul(pt1[:], wt_[0][:], main_src[:, b0:b0 + bw],
                                         start=True, stop=not has_halo)
                        if has_halo:
                            nc.tensor.matmul(pt1[:], wt_[wia][0:halo_k, :],
                                             halo_src[0:halo_k, b0:b0 + bw],
                                             start=False, stop=True)
                        S.copy(S1[:, b0:b0 + bw], pt1[:])
                        pt2 = psp.tile([128, bw], f32, tag="pT2", name=f"pt2_{c}{q}{b0}")
                        nc.tensor.matmul(pt2[:], wt_[3][:], main_src[:, b0:b0 + bw],
                                         start=True, stop=not has_halo)
                        if has_halo:
                            nc.tensor.matmul(pt2[:], wt_[wib][0:halo_k, :],
                                             halo_src[0:halo_k, b0:b0 + bw],
                                             start=False, stop=True)
                        S.copy(S2[:, b0:b0 + bw], pt2[:])
                        # hsobel for the column range this block completes
                        g0 = max(0, b0 - 2)
                        g1 = min(WG, b0 + bw - 2)
                        u0 = max(0, b0 - 1)
                        u1 = min(WB - 1, b0 + bw - 1)
                        V.tensor_tensor(gxt[:, g0:g1], S1[:, g0:g1],
                                        S1[:, g0 + 2:g1 + 2], Alu.subtract)
                        V.tensor_tensor(wt2[:, u0:u1], S2[:, u0:u1],
                                        S2[:, u0 + 1:u1 + 1], Alu.add)
                        V.tensor_tensor(gyt[:, g0:g1], wt2[:, g0:g1],
                                        wt2[:, g0 + 1:g1 + 1], Alu.add)
                    # final sliver: gx/gy cols [WB-8..WG) done, but blocks end at
                    # b0+bw-2 = WB-2 = WG+2 > WG, so full range is covered.
                    S.activation(S1[:, 0:WG], gx, Act.Square)
                    S.activation(S2[:, 0:WG], gy, Act.Square)
                    V.tensor_tensor(S1[:, 0:WG], S1[:, 0:WG], S2[:, 0:WG], Alu.add)
                    m = gq[:] if c == 0 else S2[:, 0:WG]
                    S.activation(m, S1[:, 0:WG], Act.Sqrt)
                    if c != 0:
                        V.tensor_tensor(gq[:], gq[:], m, Alu.add)
                        V.tensor_tensor(sgxq[:], sgxq[:], gx, Alu.add)
                        V.tensor_tensor(sgyq[:], sgyq[:], gy, Alu.add)
                    if c == 2:
                        gU = pool.tile([128, WG], f32, tag="gU", name=f"gU{q}")
                        gD = pool.tile([128, WG], f32, tag="gD", name=f"gD{q}")
                        for sb in range(0, WG, 512):
                            sw = min(512, WG - sb)
                            pu = psp.tile([128, sw], f32, tag="pT1", name=f"pu{q}{sb}")
                            nc.tensor.matmul(pu[:], wt_[6][:], gq[:, sb:sb + sw],
                                             start=True, stop=True)
                            S.copy(gU[:, sb:sb + sw], pu[:])
                            pd = psp.tile([128, sw], f32, tag="pT2", name=f"pd{q}{sb}")
                            nc.tensor.matmul(pd[:], wt_[7][:], gq[:, sb:sb + sw],
                                             start=True, stop=True)
                            S.copy(gD[:, sb:sb + sw], pd[:])
                        shifts[q] = (gU, gD)

            # ---- NMS per chunk ----
            for qi, q in enumerate(("a", "b")):
                gq, sgxq, sgyq = acc[q]
                rr = pool.tile([128, WG], f32, tag="bhA", name=f"rr{q}")
                ss = pool.tile([128, WG], f32, tag="bhB", name=f"ss{q}")
                m0 = pool.tile([128, WG], u8, tag="mk0", name=f"m0{q}")
                m2 = pool.tile([128, WG], u8, tag="mk1", name=f"m2{q}")
                d = pool.tile([128, WG], f32, tag="bhX", name=f"d{q}")
                dpos = pool.tile([128, WG], u8, tag="mk2", name=f"dp{q}")
                S.activation(rr[:], sgyq[:], Act.Abs)
                S.activation(ss[:], sgxq[:], Act.Abs)
                V.scalar_tensor_tensor(m0[:], ss[:], t1c, rr[:], Alu.mult, Alu.is_ge)
                V.scalar_tensor_tensor(m2[:], ss[:], t2c, rr[:], Alu.mult, Alu.is_le)
                V.tensor_tensor(d[:], sgxq[:], sgyq[:], Alu.mult)
                V.tensor_scalar(dpos[:], d[:], 0.0, None, Alu.is_ge)

                gU, gD = shifts[q]

                cand1 = pool.tile([128, WC], f32, tag="wt2", name=f"c1{q}")
                cc = pool.tile([128, WC], f32, tag="gxT", name=f"cc{q}")
                cand2 = pool.tile([128, WC], f32, tag="gyT", name=f"c2{q}")
                V.tensor_tensor(cand1[:], gU[:, 2:WG], gD[:, 0:WC], Alu.max)   # SE/NW
                V.tensor_tensor(cc[:], gU[:, 0:WC], gD[:, 2:WG], Alu.max)      # SW/NE
                V.copy_predicated(cc[:], dpos[:, 1:WC + 1], cand1[:])
                V.tensor_tensor(cand2[:], gU[:, 1:WC + 1], gD[:, 1:WC + 1], Alu.max)  # S/N
                V.copy_predicated(cc[:], m2[:, 1:WC + 1], cand2[:])
                cand0 = pool.tile([128, WC], f32, tag="wt2", name=f"c0{q}")
                V.tensor_tensor(cand0[:], gq[:, 2:WG], gq[:, 0:WC], Alu.max)   # E/W
                V.copy_predicated(cc[:], m0[:, 1:WC + 1], cand0[:])

                hp = pool.tile([128, WC], f16, tag="hp", name=f"hp{q}")
                lm = pool.tile([128, W], f16, tag="lm", name=f"lm{q}")
                V.scalar_tensor_tensor(hp[:], cc[:], high, gq[:, 1:WC + 1],
                                       Alu.max, Alu.is_lt)
                V.scalar_tensor_tensor(lm[:], cc[:, 1:W + 1], lowx, gq[:, 2:W + 2],
                                       Alu.max, Alu.is_lt)

                rm1 = pool.tile([128, W], f16, tag="rm1", name=f"rm1{q}")
                rm = pool.tile([128, W], f16, tag="rm", name=f"rm{q}")
                V.tensor_tensor(rm1[:], hp[:, 0:W], hp[:, 2:W + 2], Alu.max)
                V.tensor_tensor(rm[:], rm1[:], hp[:, 1:W + 1], Alu.max)
                # vertical 3-row OR via tridiagonal-ones matmul on PE
                zp = pool.tile([128, W], f16, tag="rm1", name=f"zp{q}")
                for zb in range(0, W, 512):
                    pz = psp.tile([128, 512], f32, tag="pT1", name=f"pz{q}{zb}")
                    nc.tensor.matmul(pz[:], m3t[:], rm[:, zb:zb + 512],
                                     start=True, stop=True)
                    S.copy(zp[:, zb:zb + 512], pz[:])
                outt = iop.tile([128, W], f16, tag="out", name=f"out{q}")
                V.scalar_tensor_tensor(outt[:], zp[:], 0.5, lm[:],
                                       Alu.is_ge, Alu.mult)
                r0, nrows_out = (0, 124) if q == "a" else (124, NR - 124)
                nc.sync.dma_start(
                    out=bass.AP(out, r0 * W, [[W, nrows_out], [1, W]]),
                    in_=outt[2:2 + nrows_out, :])

            rr = pool.tile([128, RN, 18], f32, tag="tlA", name="trr")
            ss = pool.tile([128, RN, 18], f32, tag="tlB", name="tss")
            m0 = pool.tile([128, RN, 18], u8, tag="tmk0", name="tm0")
            m2 = pool.tile([128, RN, 18], u8, tag="tmk1", name="tm2")
            d = pool.tile([128, RN, 18], f32, tag="tlC", name="td")
            dpos = pool.tile([128, RN, 18], u8, tag="tmk2", name="tdp")
            S.activation(rr[:], tsgy[:, 1:RN + 1, 1:19], Act.Abs)
            S.activation(ss[:], tsgx[:, 1:RN + 1, 1:19], Act.Abs)
            V.scalar_tensor_tensor(m0[:], ss[:], t1c, rr[:], Alu.mult, Alu.is_ge)
            V.scalar_tensor_tensor(m2[:], ss[:], t2c, rr[:], Alu.mult, Alu.is_le)
            V.tensor_tensor(d[:], tsgx[:, 1:RN + 1, 1:19], tsgy[:, 1:RN + 1, 1:19], Alu.mult)
            V.tensor_scalar(dpos[:], d[:], 0.0, None, Alu.is_ge)
            cand = pool.tile([128, RN, 18], f32, tag="tlD", name="tc1")
            cc = pool.tile([128, RN, 18], f32, tag="tlA", name="tcc")
            cand2 = pool.tile([128, RN, 18], f32, tag="tlB", name="tc2")
            cand3 = pool.tile([128, RN, 18], f32, tag="tlC", name="tc0")
            V.tensor_tensor(cand[:], tg[:, 2:RN + 2, 2:20], tg[:, 0:RN, 0:18], Alu.max)
            V.tensor_tensor(cc[:], tg[:, 2:RN + 2, 0:18], tg[:, 0:RN, 2:20], Alu.max)
            V.copy_predicated(cc[:], dpos[:], cand[:])
            V.tensor_tensor(cand2[:], tg[:, 2:RN + 2, 1:19], tg[:, 0:RN, 1:19], Alu.max)
            V.copy_predicated(cc[:], m2[:], cand2[:])
            V.tensor_tensor(cand3[:], tg[:, 1:RN + 1, 2:20], tg[:, 1:RN + 1, 0:18], Alu.max)
            V.copy_predicated(cc[:], m0[:], cand3[:])
            hp = pool.tile([128, RN, 18], f16, tag="thp", name="thp")
            lm = pool.tile([128, N, 16], f32, tag="tlC", name="tlm")
            V.scalar_tensor_tensor(hp[:], cc[:], high, tg[:, 1:RN + 1, 1:19],
                                   Alu.max, Alu.is_lt)
            V.scalar_tensor_tensor(lm[:], cc[:, 1:N + 1, 1:17], lowx,
                                   tg[:, 2:RN, 2:18], Alu.max, Alu.is_lt)
            rm1 = pool.tile([128, RN, 16], f16, tag="trm1", name="trm1")
            rm = pool.tile([128, RN, 16], f16, tag="trm", name="trm")
            cm = pool.tile([128, N, 16], f16, tag="tcm", name="tcm")
            V.tensor_tensor(rm1[:], hp[:, :, 0:16], hp[:, :, 2:18], Alu.max)
            V.tensor_tensor(rm[:], rm1[:], hp[:, :, 1:17], Alu.max)
            V.tensor_tensor(cm[:], rm[:, 0:N, :], rm[:, 2:RN, :], Alu.max)
            V.tensor_tensor(cm[:], cm[:], rm[:, 1:N + 1, :], Alu.max)
            outt = pool.tile([128, N, 16], f32, tag="tlD", name="touts")
            V.tensor_tensor(outt[:], lm[:], cm[:], Alu.mult)
            nc.sync.dma_start(
                out=bass.AP(out_t, 0, [[NT * 16, 128], [16, N], [1, 16]]),
                in_=outt[:])

    nc.finalize()
    return nc


def _get_compiled(low, high):
    key = (low, high)
    if key not in _COMPILED:
        _COMPILED[key] = _build(low, high)
    return _COMPILED[key]


def kernel(img, threshold1, threshold2, _trace=False):
    from concourse import bass_utils

    t1 = float(np.asarray(threshold1))
    t2 = float(np.asarray(threshold2))
    low, high = min(t1, t2), max(t1, t2)

    x = np.ascontiguousarray(np.asarray(img, dtype=np.float32)[0])  # [3,H,W]
    xp = np.zeros((3, H + 2 * HALO, W + 2 * HALO), dtype=np.float32)
    xp[:, HALO:HALO + H, HALO:HALO + W] = x

    wmv = _weights()
    m3v = np.zeros((128, 128), np.float16)
    for p in range(128):
        for j in range(max(0, p - 1), min(128, p + 2)):
            m3v[p, j] = 1.0
    win = np.lib.stride_tricks.sliding_window_view(xp, 26, axis=2)[:, :, ::16, :]
    in_maps = []
    for k in range(8):
        rows = np.ascontiguousarray(xp[:, k * RPC:k * RPC + SHARD_ROWS, :])
        tw = win[:, k * RPC + NR:k * RPC + NR + NT + 10]     # [3,24,128,26]
        packed = np.ascontiguousarray(tw.transpose(0, 2, 1, 3))
        in_maps.append({"xr": rows, "xt": packed, "wm": wmv, "wm16": m3v})

    nc = _get_compiled(low, high)
    res = bass_utils.run_bass_kernel_spmd(nc, in_maps, core_ids=list(range(8)),
                                          trace=_trace)

    full = np.zeros((1, 1, H, W), dtype=np.float32)
    for k in range(8):
        o = res.results[k]["out"]          # [248, 2048] f16
        ot = res.results[k]["out_t"]       # [128, 8, 16] f32
        full[0, 0, k * RPC:k * RPC + NR, :] = o.astype(np.float32)
        full[0, 0, k * RPC + NR:(k + 1) * RPC, :] = (
            ot.transpose(1, 0, 2).reshape(NT, W))
    full[:, :, 0, :] = 0.0
    full[:, :, -1, :] = 0.0
    full[:, :, :, 0] = 0.0
    full[:, :, :, -1] = 0.0
    if _trace:
        kernel._last_results = res
    return full


# revision 8
# speedup vs baseline: 1.7986x; 1.0544x over previous
"""Canny v3: rows-on-partitions + PE band-matmuls for vertical convs.

Per core (256 out rows, shard rows [-5,261) zero-padded):
 - two rows-mode chunks: chunk a = T/g rows [-2,126) -> out [0,124),
   chunk b = T rows [122,250) -> out [124,248).  Partition = image row,
   free dim = full 2054/2052-wide rows, so every DVE op covers 128 rows
   at ~2054 elems (vs ~2900 elems per 128 rows in the columns layout).
 - vertical (gaussian o sobel) 7-tap convs fused into PE matmuls with
   band weight matrices; halo rows accumulate from the neighboring bh
   tile via a second matmul into the same PSUM bank.
 - NMS vertical/diagonal neighbors via SBUF->SBUF DMA row-shifted
   copies of the g plane (and of rm for hysteresis).
 - last 8 rows per core done in a small v1-style columns-mode pass.
"""

import numpy as np

_COMPILED = {}

H = 2048
W = 2048
HALO = 5
RPC = H // 8                      # 256 rows per core
SHARD_ROWS = RPC + 2 * HALO       # 266
WPAD = W + 2 * HALO               # 2058
WB = W + 6                        # 2054: bh/T cols [-3, 2051)
WG = W + 4                        # 2052: g/gx/gy cols [-2, 2050)
WC = W + 2                        # 2050: cand/cc/hp cols [-1, 2049)
NT = 14                           # tail rows (columns-mode)
NR = RPC - NT                     # 248 rows via rows-mode
BLOCKS = [(0, 512), (512, 512), (1024, 512), (1536, 512), (2048, WB - 2048)]


def _weights():
    g5 = np.exp(-0.5 * (np.arange(5) - 2.0) ** 2).astype(np.float32)
    w7a = np.convolve(np.array([1, 2, 1], np.float32), g5).astype(np.float32)
    w7b = np.convolve(np.array([1, 0, -1], np.float32), g5).astype(np.float32)

    def w7i(w7, i):
        return float(w7[i]) if 0 <= i < 7 else 0.0

    wm = np.zeros((8, 128, 128), np.float32)
    for p in range(128):
        for j in range(128):
            wm[0, p, j] = w7i(w7a, p - j)          # Wa_main
            wm[3, p, j] = w7i(w7b, p - j)          # Wb_main
            if 4 <= p < 10:
                wm[1, p, j] = w7i(w7a, p + 124 - j)  # Wa_h1 (chunk a halo from bh_b)
                wm[4, p, j] = w7i(w7b, p + 124 - j)  # Wb_h1
            if p < 6:
                wm[2, p, j] = w7i(w7a, p + 128 - j)  # Wa_hx (chunk b halo from bhx)
                wm[5, p, j] = w7i(w7b, p + 128 - j)  # Wb_hx
            if p == j + 1:
                wm[6, p, j] = 1.0                    # SU: gU[j] = g[j+1]
            if p == j - 1:
                wm[7, p, j] = 1.0                    # SD: gD[j] = g[j-1]
    return wm


def _build(low, high):
    import concourse.bass as bass
    import concourse.bacc as bacc
    import concourse.mybir as mybir
    from concourse.tile import TileContext

    f32 = mybir.dt.float32
    f16 = mybir.dt.float16
    u8 = mybir.dt.uint8
    Alu = mybir.AluOpType
    Act = mybir.ActivationFunctionType

    g5 = np.exp(-0.5 * (np.arange(5) - 2.0) ** 2).astype(np.float32)
    ga = float(g5[0])
    gb = float(g5[1])
    t1c = float(np.float32(np.tan(np.deg2rad(np.float64(22.5)))))
    t2c = float(np.float32(np.tan(np.deg2rad(np.float64(67.5)))))
    lowx = float(np.nextafter(np.float32(low), np.float32(0.0)))

    nc = bacc.Bacc()
    xr = nc.dram_tensor("xr", [3, SHARD_ROWS, WPAD], f32, kind="ExternalInput")
    xt = nc.dram_tensor("xt", [3, 128, 24, 26], f32, kind="ExternalInput")
    wm = nc.dram_tensor("wm", [8, 128, 128], f32, kind="ExternalInput")
    wm16 = nc.dram_tensor("wm16", [128, 128], f16, kind="ExternalInput")
    out = nc.dram_tensor("out", [NR, W], f16, kind="ExternalOutput")
    out_t = nc.dram_tensor("out_t", [128, NT, 16], f32, kind="ExternalOutput")

    with TileContext(nc) as tc:
        with tc.tile_pool(name="io", bufs=2) as iop, \
             tc.tile_pool(name="pl", bufs=1) as pool, \
             tc.tile_pool(name="ps", bufs=2, space="PSUM") as psp:

            V = nc.vector
            S = nc.scalar

            imgs = {}

            def dma_img(c, nm, row0, nrows):
                img = iop.tile([128, WPAD], f32, tag="img", name=f"img{c}{nm}",
                               bufs=3)
                off0 = c * SHARD_ROWS * WPAD + row0 * WPAD
                nc.sync.dma_start(
                    out=img[0:nrows, :],
                    in_=bass.AP(xr, off0, [[WPAD, nrows], [343, 6], [1, 343]]))
                imgs[(c, nm)] = img

            dma_img(0, "a", 0, 128)
            dma_img(0, "b", 124, 128)

            wt_ = []
            for i in range(8):
                wti = pool.tile([128, 128], f32, tag=f"w{i}", name=f"wt{i}")
                nc.sync.dma_start(out=wti[:], in_=bass.AP(wm, i * 128 * 128,
                                                          [[128, 128], [1, 128]]))
                wt_.append(wti)
            m3t = pool.tile([128, 128], f16, tag="m3", name="m3t")
            nc.sync.dma_start(out=m3t[:], in_=bass.AP(wm16, 0, [[128, 128], [1, 128]]))

            # accumulators per chunk
            acc = {}
            for q in ("a", "b"):
                acc[q] = (
                    pool.tile([128, WG], f32, tag=f"g_{q}", name=f"g_{q}"),
                    pool.tile([128, WG], f32, tag=f"sgx_{q}", name=f"sgx_{q}"),
                    pool.tile([128, WG], f32, tag=f"sgy_{q}", name=f"sgy_{q}"),
                )

            # ---- tail: columns-mode v1-style pass for out rows [248,256) ----
            N, R, RV, RT, RN = NT, NT + 10, NT + 6, NT + 4, NT + 2
            tg = pool.tile([128, RT, 20], f32, tag="tg", name="tg")
            tsgx = pool.tile([128, RT, 20], f32, tag="tsgx", name="tsgx")
            tsgy = pool.tile([128, RT, 20], f32, tag="tsgy", name="tsgy")
            for c in range(3):
                img = pool.tile([128, R, 26], f32, tag="timg", name=f"timg{c}")
                nc.sync.dma_start(out=img[:], in_=bass.AP(
                    xt, c * 128 * 24 * 26, [[24 * 26, 128], [26, R], [1, 26]]))
                tA = pool.tile([128, R, 22], f32, tag="tlA", name=f"tlA{c}")
                tB = pool.tile([128, R, 22], f32, tag="tlB", name=f"tlB{c}")
                tC = pool.tile([128, R, 22], f32, tag="tlC", name=f"tlC{c}")
                tD = pool.tile([128, RV, 22], f32, tag="tlD", name=f"tlD{c}")
                V.tensor_tensor(tA[:], img[:, :, 1:23], img[:, :, 3:25], Alu.add)
                V.tensor_tensor(tB[:], img[:, :, 0:22], img[:, :, 4:26], Alu.add)
                V.scalar_tensor_tensor(tC[:], tA[:], gb, img[:, :, 2:24], Alu.mult, Alu.add)
                V.scalar_tensor_tensor(tC[:], tB[:], ga, tC[:], Alu.mult, Alu.add)
                V.tensor_tensor(tA[:, 0:RV, :], tC[:, 1:RV + 1, :], tC[:, 3:RV + 3, :], Alu.add)
                V.tensor_tensor(tB[:, 0:RV, :], tC[:, 0:RV, :], tC[:, 4:RV + 4, :], Alu.add)
                V.scalar_tensor_tensor(tD[:], tA[:, 0:RV, :], gb, tC[:, 2:RV + 2, :],
                                       Alu.mult, Alu.add)
                V.scalar_tensor_tensor(tD[:], tB[:, 0:RV, :], ga, tD[:], Alu.mult, Alu.add)
                V.tensor_tensor(tA[:, 0:RT, :], tD[:, 0:RT, :], tD[:, 2:RT + 2, :], Alu.add)
                V.scalar_tensor_tensor(tA[:, 0:RT, :], tD[:, 1:RT + 1, :], 2.0,
                                       tA[:, 0:RT, :], Alu.mult, Alu.add)   # t1
                V.tensor_tensor(tB[:, 0:RT, :], tD[:, 0:RT, :], tD[:, 2:RT + 2, :],
                                Alu.subtract)                                # t2
                gx = tsgx[:] if c == 0 else tC[:, 0:RT, 0:20]
                gy = tsgy[:] if c == 0 else tD[:, 0:RT, 0:20]
                w2 = tD[:, 0:RT, 0:20]
                V.tensor_tensor(gx, tA[:, 0:RT, 0:20], tA[:, 0:RT, 2:22], Alu.subtract)
                V.tensor_tensor(w2, tB[:, 0:RT, 0:20], tB[:, 0:RT, 2:22], Alu.add)
                V.scalar_tensor_tensor(gy, tB[:, 0:RT, 1:21], 2.0, w2, Alu.mult, Alu.add)
                q1 = tA[:, 0:RT, 0:20]
                q2 = tB[:, 0:RT, 0:20]
                S.activation(q1, gx, Act.Square)
                S.activation(q2, gy, Act.Square)
                V.tensor_tensor(q1, q1, q2, Alu.add)
                m = tg[:] if c == 0 else tB[:, 0:RT, 0:20]
                S.activation(m, q1, Act.Sqrt)
                if c != 0:
                    V.tensor_tensor(tg[:], tg[:], m, Alu.add)
                    V.tensor_tensor(tsgx[:], tsgx[:], gx, Alu.add)
                    V.tensor_tensor(tsgy[:], tsgy[:], gy, Alu.add)




            for c_ in (1, 2):
                dma_img(c_, "a", 0, 128)
                dma_img(c_, "b", 124, 128)

            shifts = {}
            for c in range(3):
                # ---- hgauss for the three bh tiles of this channel ----
                bhs = {}
                for (nm, row0, nrows, tag) in (
                        ("a", 0, 128, "bhA"), ("b", 124, 128, "bhB")):
                    img = imgs[(c, nm)]
                    s1 = pool.tile([128, WB], f32, tag="S1", name=f"s1_{c}{nm}")
                    s2 = pool.tile([128, WB], f32, tag="S2", name=f"s2_{c}{nm}")
                    bh = pool.tile([128, WB], f32, tag=tag, name=f"bh{nm}{c}")
                    r = slice(0, nrows)
                    V.tensor_tensor(s1[r, :], img[r, 1:WB + 1], img[r, 3:WB + 3], Alu.add)
                    V.tensor_tensor(s2[r, :], img[r, 0:WB], img[r, 4:WB + 4], Alu.add)
                    V.scalar_tensor_tensor(bh[r, :], s1[r, :], gb, img[r, 2:WB + 2],
                                           Alu.mult, Alu.add)
                    V.scalar_tensor_tensor(bh[r, :], s2[r, :], ga, bh[r, :],
                                           Alu.mult, Alu.add)
                    bhs[nm] = bh

                # ---- per chunk: PE T1/T2, then hsobel + mag on DVE ----
                for q in ("a", "b"):
                    if q == "a":
                        main_src, halo_src, halo_k, wia, wib = bhs["a"], bhs["b"], 128, 1, 4
                    else:
                        main_src, halo_src, halo_k, wia, wib = bhs["b"], None, 0, 2, 5
                    S1 = pool.tile([128, WB], f32, tag="S1", name=f"S1_{c}{q}")
                    S2 = pool.tile([128, WB], f32, tag="S2", name=f"S2_{c}{q}")
                    gq, sgxq, sgyq = acc[q]
                    gxt = sgxq if c == 0 else pool.tile(
                        [128, WG], f32, tag="gxT", name=f"gx{c}{q}")
                    gyt = sgyq if c == 0 else pool.tile(
                        [128, WG], f32, tag="gyT", name=f"gy{c}{q}")
                    gx = gxt[:]
                    gy = gyt[:]
                    wt2 = pool.tile([128, WB - 1], f32, tag="wt2", name=f"w2_{c}{q}")
                    for (b0, bw) in BLOCKS:
                        has_halo = halo_src is not None
                        pt1 = psp.tile([128, bw], f32, tag="pT1", name=f"pt1_{c}{q}{b0}")
                        nc.tensor.matmul(pt1[:], wt_[0][:], main_src[:, b0:b0 + bw],
                                         start=True, stop=not has_halo)
                        if has_halo:
                            nc.tensor.matmul(pt1[:], wt_[wia][0:halo_k, :],
                                             halo_src[0:halo_k, b0:b0 + bw],
                                             start=False, stop=True)
                        S.copy(S1[:, b0:b0 + bw], pt1[:])
                        pt2 = psp.tile([128, bw], f32, tag="pT2", name=f"pt2_{c}{q}{b0}")
                        nc.tensor.matmul(pt2[:], wt_[3][:], main_src[:, b0:b0 + bw],
                                         start=True, stop=not has_halo)
                        if has_halo:
                            nc.tensor.matmul(pt2[:], wt_[wib][0:halo_k, :],
                                             halo_src[0:halo_k, b0:b0 + bw],
                                             start=False, stop=True)
                        S.copy(S2[:, b0:b0 + bw], pt2[:])
                        # hsobel for the column range this block completes
                        g0 = max(0, b0 - 2)
                        g1 = min(WG, b0 + bw - 2)
                        u0 = max(0, b0 - 1)
                        u1 = min(WB - 1, b0 + bw - 1)
                        V.tensor_tensor(gxt[:, g0:g1], S1[:, g0:g1],
                                        S1[:, g0 + 2:g1 + 2], Alu.subtract)
                        V.tensor_tensor(wt2[:, u0:u1], S2[:, u0:u1],
                                        S2[:, u0 + 1:u1 + 1], Alu.add)
                        V.tensor_tensor(gyt[:, g0:g1], wt2[:, g0:g1],
                                        wt2[:, g0 + 1:g1 + 1], Alu.add)
                    # final sliver: gx/gy cols [WB-8..WG) done, but blocks end at
                    # b0+bw-2 = WB-2 = WG+2 > WG, so full range is covered.
                    S.activation(S1[:, 0:WG], gx, Act.Square)
                    S.activation(S2[:, 0:WG], gy, Act.Square)
                    V.tensor_tensor(S1[:, 0:WG], S1[:, 0:WG], S2[:, 0:WG], Alu.add)
                    m = gq[:] if c == 0 else S2[:, 0:WG]
                    S.activation(m, S1[:, 0:WG], Act.Sqrt)
                    if c != 0:
                        V.tensor_tensor(gq[:], gq[:], m, Alu.add)
                        V.tensor_tensor(sgxq[:], sgxq[:], gx, Alu.add)
                        V.tensor_tensor(sgyq[:], sgyq[:], gy, Alu.add)
                    if c == 2:
                        gU = pool.tile([128, WG], f32, tag="gU", name=f"gU{q}")
                        gD = pool.tile([128, WG], f32, tag="gD", name=f"gD{q}")
                        for sb in range(0, WG, 512):
                            sw = min(512, WG - sb)
                            pu = psp.tile([128, sw], f32, tag="pT1", name=f"pu{q}{sb}")
                            nc.tensor.matmul(pu[:], wt_[6][:], gq[:, sb:sb + sw],
                                             start=True, stop=True)
                            S.copy(gU[:, sb:sb + sw], pu[:])
                            pd = psp.tile([128, sw], f32, tag="pT2", name=f"pd{q}{sb}")
                            nc.tensor.matmul(pd[:], wt_[7][:], gq[:, sb:sb + sw],
                                             start=True, stop=True)
                            S.copy(gD[:, sb:sb + sw], pd[:])
                        shifts[q] = (gU, gD)

            # ---- NMS per chunk ----
            def nms_chunk(q):
                gq, sgxq, sgyq = acc[q]
                rr = pool.tile([128, WG], f32, tag="bhA", name=f"rr{q}")
                ss = pool.tile([128, WG], f32, tag="bhB", name=f"ss{q}")
                m0 = pool.tile([128, WG], u8, tag="mk0", name=f"m0{q}")
                m2 = pool.tile([128, WG], u8, tag="mk1", name=f"m2{q}")
                d = pool.tile([128, WG], f32, tag="bhX", name=f"d{q}")
                dpos = pool.tile([128, WG], u8, tag="mk2", name=f"dp{q}")
                S.activation(rr[:], sgyq[:], Act.Abs)
                S.activation(ss[:], sgxq[:], Act.Abs)
                V.scalar_tensor_tensor(m0[:], ss[:], t1c, rr[:], Alu.mult, Alu.is_ge)
                V.scalar_tensor_tensor(m2[:], ss[:], t2c, rr[:], Alu.mult, Alu.is_le)
                V.tensor_tensor(d[:], sgxq[:], sgyq[:], Alu.mult)
                V.tensor_scalar(dpos[:], d[:], 0.0, None, Alu.is_ge)

                gU, gD = shifts[q]

                cand1 = pool.tile([128, WC], f32, tag="wt2", name=f"c1{q}")
                cc = pool.tile([128, WC], f32, tag="gxT", name=f"cc{q}")
                cand2 = pool.tile([128, WC], f32, tag="gyT", name=f"c2{q}")
                V.tensor_tensor(cand1[:], gU[:, 2:WG], gD[:, 0:WC], Alu.max)   # SE/NW
                V.tensor_tensor(cc[:], gU[:, 0:WC], gD[:, 2:WG], Alu.max)      # SW/NE
                V.copy_predicated(cc[:], dpos[:, 1:WC + 1], cand1[:])
                V.tensor_tensor(cand2[:], gU[:, 1:WC + 1], gD[:, 1:WC + 1], Alu.max)  # S/N
                V.copy_predicated(cc[:], m2[:, 1:WC + 1], cand2[:])
                cand0 = pool.tile([128, WC], f32, tag="wt2", name=f"c0{q}")
                V.tensor_tensor(cand0[:], gq[:, 2:WG], gq[:, 0:WC], Alu.max)   # E/W
                V.copy_predicated(cc[:], m0[:, 1:WC + 1], cand0[:])

                hp = pool.tile([128, WC], f16, tag="hp", name=f"hp{q}")
                lm = pool.tile([128, W], f16, tag="lm", name=f"lm{q}")
                V.scalar_tensor_tensor(hp[:], cc[:], high, gq[:, 1:WC + 1],
                                       Alu.max, Alu.is_lt)
                V.scalar_tensor_tensor(lm[:], cc[:, 1:W + 1], lowx, gq[:, 2:W + 2],
                                       Alu.max, Alu.is_lt)

                rm1 = pool.tile([128, W], f16, tag="rm1", name=f"rm1{q}")
                rm = pool.tile([128, W], f16, tag="rm", name=f"rm{q}")
                V.tensor_tensor(rm1[:], hp[:, 0:W], hp[:, 2:W + 2], Alu.max)
                V.tensor_tensor(rm[:], rm1[:], hp[:, 1:W + 1], Alu.max)
                # vertical 3-row OR via tridiagonal-ones matmul on PE
                zp = pool.tile([128, W], f16, tag="rm1", name=f"zp{q}")
                for zb in range(0, W, 512):
                    pz = psp.tile([128, 512], f32, tag="pT1", name=f"pz{q}{zb}")
                    nc.tensor.matmul(pz[:], m3t[:], rm[:, zb:zb + 512],
                                     start=True, stop=True)
                    S.copy(zp[:, zb:zb + 512], pz[:])
                outt = iop.tile([128, W], f16, tag="out", name=f"out{q}")
                V.scalar_tensor_tensor(outt[:], zp[:], 0.5, lm[:],
                                       Alu.is_ge, Alu.mult)
                r0, nrows_out = (0, 124) if q == "a" else (124, NR - 124)
                nc.sync.dma_start(
                    out=bass.AP(out, r0 * W, [[W, nrows_out], [1, W]]),
                    in_=outt[2:2 + nrows_out, :])



            nms_chunk("a")

            rr = pool.tile([128, RN, 18], f32, tag="tlA", name="trr")
            ss = pool.tile([128, RN, 18], f32, tag="tlB", name="tss")
            m0 = pool.tile([128, RN, 18], u8, tag="tmk0", name="tm0")
            m2 = pool.tile([128, RN, 18], u8, tag="tmk1", name="tm2")
            d = pool.tile([128, RN, 18], f32, tag="tlC", name="td")
            dpos = pool.tile([128, RN, 18], u8, tag="tmk2", name="tdp")
            S.activation(rr[:], tsgy[:, 1:RN + 1, 1:19], Act.Abs)
            S.activation(ss[:], tsgx[:, 1:RN + 1, 1:19], Act.Abs)
            V.scalar_tensor_tensor(m0[:], ss[:], t1c, rr[:], Alu.mult, Alu.is_ge)
            V.scalar_tensor_tensor(m2[:], ss[:], t2c, rr[:], Alu.mult, Alu.is_le)
            V.tensor_tensor(d[:], tsgx[:, 1:RN + 1, 1:19], tsgy[:, 1:RN + 1, 1:19], Alu.mult)
            V.tensor_scalar(dpos[:], d[:], 0.0, None, Alu.is_ge)
            cand = pool.tile([128, RN, 18], f32, tag="tlD", name="tc1")
            cc = pool.tile([128, RN, 18], f32, tag="tlA", name="tcc")
            cand2 = pool.tile([128, RN, 18], f32, tag="tlB", name="tc2")
            cand3 = pool.tile([128, RN, 18], f32, tag="tlC", name="tc0")
            V.tensor_tensor(cand[:], tg[:, 2:RN + 2, 2:20], tg[:, 0:RN, 0:18], Alu.max)
            V.tensor_tensor(cc[:], tg[:, 2:RN + 2, 0:18], tg[:, 0:RN, 2:20], Alu.max)
            V.copy_predicated(cc[:], dpos[:], cand[:])
            V.tensor_tensor(cand2[:], tg[:, 2:RN + 2, 1:19], tg[:, 0:RN, 1:19], Alu.max)
            V.copy_predicated(cc[:], m2[:], cand2[:])
            V.tensor_tensor(cand3[:], tg[:, 1:RN + 1, 2:20], tg[:, 1:RN + 1, 0:18], Alu.max)
            V.copy_predicated(cc[:], m0[:], cand3[:])
            hp = pool.tile([128, RN, 18], f16, tag="thp", name="thp")
            lm = pool.tile([128, N, 16], f32, tag="tlC", name="tlm")
            V.scalar_tensor_tensor(hp[:], cc[:], high, tg[:, 1:RN + 1, 1:19],
                                   Alu.max, Alu.is_lt)
            V.scalar_tensor_tensor(lm[:], cc[:, 1:N + 1, 1:17], lowx,
                                   tg[:, 2:RN, 2:18], Alu.max, Alu.is_lt)
            rm1 = pool.tile([128, RN, 16], f16, tag="trm1", name="trm1")
            rm = pool.tile([128, RN, 16], f16, tag="trm", name="trm")
            cm = pool.tile([128, N, 16], f16, tag="tcm", name="tcm")
            V.tensor_tensor(rm1[:], hp[:, :, 0:16], hp[:, :, 2:18], Alu.max)
            V.tensor_tensor(rm[:], rm1[:], hp[:, :, 1:17], Alu.max)
            V.tensor_tensor(cm[:], rm[:, 0:N, :], rm[:, 2:RN, :], Alu.max)
            V.tensor_tensor(cm[:], cm[:], rm[:, 1:N + 1, :], Alu.max)
            outt = pool.tile([128, N, 16], f32, tag="tlD", name="touts")
            V.tensor_tensor(outt[:], lm[:], cm[:], Alu.mult)
            nc.sync.dma_start(
                out=bass.AP(out_t, 0, [[NT * 16, 128], [16, N], [1, 16]]),
                in_=outt[:])

            nms_chunk("b")

    nc.finalize()
    return nc


def _get_compiled(low, high):
    key = (low, high)
    if key not in _COMPILED:
        _COMPILED[key] = _build(low, high)
    return _COMPILED[key]


def kernel(img, threshold1, threshold2, _trace=False):
    from concourse import bass_utils

    t1 = float(np.asarray(threshold1))
    t2 = float(np.asarray(threshold2))
    low, high = min(t1, t2), max(t1, t2)

    x = np.ascontiguousarray(np.asarray(img, dtype=np.float32)[0])  # [3,H,W]
    xp = np.zeros((3, H + 2 * HALO, W + 2 * HALO), dtype=np.float32)
    xp[:, HALO:HALO + H, HALO:HALO + W] = x

    wmv = _weights()
    m3v = np.zeros((128, 128), np.float16)
    for p in range(128):
        for j in range(max(0, p - 1), min(128, p + 2)):
            m3v[p, j] = 1.0
    win = np.lib.stride_tricks.sliding_window_view(xp, 26, axis=2)[:, :, ::16, :]
    in_maps = []
    for k in range(8):
        rows = np.ascontiguousarray(xp[:, k * RPC:k * RPC + SHARD_ROWS, :])
        tw = win[:, k * RPC + NR:k * RPC + NR + NT + 10]     # [3,24,128,26]
        packed = np.ascontiguousarray(tw.transpose(0, 2, 1, 3))
        in_maps.append({"xr": rows, "xt": packed, "wm": wmv, "wm16": m3v})

    nc = _get_compiled(low, high)
    res = bass_utils.run_bass_kernel_spmd(nc, in_maps, core_ids=list(range(8)),
                                          trace=_trace)

    full = np.zeros((1, 1, H, W), dtype=np.float32)
    for k in range(8):
        o = res.results[k]["out"]          # [248, 2048] f16
        ot = res.results[k]["out_t"]       # [128, 8, 16] f32
        full[0, 0, k * RPC:k * RPC + NR, :] = o.astype(np.float32)
        full[0, 0, k * RPC + NR:(k + 1) * RPC, :] = (
            ot.transpose(1, 0, 2).reshape(NT, W))
    full[:, :, 0, :] = 0.0
    full[:, :, -1, :] = 0.0
    full[:, :, :, 0] = 0.0
    full[:, :, :, -1] = 0.0
    if _trace:
        kernel._last_results = res
    return full


# revision 9
# speedup vs baseline: 1.8102x; 1.0064x over previous
"""Canny v3: rows-on-partitions + PE band-matmuls for vertical convs.

Per core (256 out rows, shard rows [-5,261) zero-padded):
 - two rows-mode chunks: chunk a = T/g rows [-2,126) -> out [0,124),
   chunk b = T rows [122,250) -> out [124,248).  Partition = image row,
   free dim = full 2054/2052-wide rows, so every DVE op covers 128 rows
   at ~2054 elems (vs ~2900 elems per 128 rows in the columns layout).
 - vertical (gaussian o sobel) 7-tap convs fused into PE matmuls with
   band weight matrices; halo rows accumulate from the neighboring bh
   tile via a second matmul into the same PSUM bank.
 - NMS vertical/diagonal neighbors via SBUF->SBUF DMA row-shifted
   copies of the g plane (and of rm for hysteresis).
 - last 8 rows per core done in a small v1-style columns-mode pass.
"""

import numpy as np

_COMPILED = {}

H = 2048
W = 2048
HALO = 5
RPC = H // 8                      # 256 rows per core
SHARD_ROWS = RPC + 2 * HALO       # 266
WPAD = W + 2 * HALO               # 2058
WB = W + 6                        # 2054: bh/T cols [-3, 2051)
WG = W + 4                        # 2052: g/gx/gy cols [-2, 2050)
WC = W + 2                        # 2050: cand/cc/hp cols [-1, 2049)
NT = 14                           # tail rows (columns-mode)
NR = RPC - NT                     # 248 rows via rows-mode
BLOCKS = [(0, 512), (512, 512), (1024, 512), (1536, 512), (2048, WB - 2048)]


def _weights():
    g5 = np.exp(-0.5 * (np.arange(5) - 2.0) ** 2).astype(np.float32)
    w7a = np.convolve(np.array([1, 2, 1], np.float32), g5).astype(np.float32)
    w7b = np.convolve(np.array([1, 0, -1], np.float32), g5).astype(np.float32)

    def w7i(w7, i):
        return float(w7[i]) if 0 <= i < 7 else 0.0

    wm = np.zeros((8, 128, 128), np.float32)
    for p in range(128):
        for j in range(128):
            wm[0, p, j] = w7i(w7a, p - j)          # Wa_main
            wm[3, p, j] = w7i(w7b, p - j)          # Wb_main
            if 4 <= p < 10:
                wm[1, p, j] = w7i(w7a, p + 124 - j)  # Wa_h1 (chunk a halo from bh_b)
                wm[4, p, j] = w7i(w7b, p + 124 - j)  # Wb_h1
            if p < 6:
                wm[2, p, j] = w7i(w7a, p + 128 - j)  # Wa_hx (chunk b halo from bhx)
                wm[5, p, j] = w7i(w7b, p + 128 - j)  # Wb_hx
            if p == j + 1:
                wm[6, p, j] = 1.0                    # SU: gU[j] = g[j+1]
            if p == j - 1:
                wm[7, p, j] = 1.0                    # SD: gD[j] = g[j-1]
    return wm


def _build(low, high):
    import concourse.bass as bass
    import concourse.bacc as bacc
    import concourse.mybir as mybir
    from concourse.tile import TileContext

    f32 = mybir.dt.float32
    f16 = mybir.dt.float16
    u8 = mybir.dt.uint8
    Alu = mybir.AluOpType
    Act = mybir.ActivationFunctionType

    g5 = np.exp(-0.5 * (np.arange(5) - 2.0) ** 2).astype(np.float32)
    ga = float(g5[0])
    gb = float(g5[1])
    t1c = float(np.float32(np.tan(np.deg2rad(np.float64(22.5)))))
    t2c = float(np.float32(np.tan(np.deg2rad(np.float64(67.5)))))
    lowx = float(np.nextafter(np.float32(low), np.float32(0.0)))

    nc = bacc.Bacc()
    xr = nc.dram_tensor("xr", [3, SHARD_ROWS, WPAD], f32, kind="ExternalInput")
    xt = nc.dram_tensor("xt", [3, 128, 24, 26], f32, kind="ExternalInput")
    wm = nc.dram_tensor("wm", [8, 128, 128], f32, kind="ExternalInput")
    wm16 = nc.dram_tensor("wm16", [128, 128], f16, kind="ExternalInput")
    out = nc.dram_tensor("out", [NR, W], f16, kind="ExternalOutput")
    out_t = nc.dram_tensor("out_t", [128, NT, 16], f32, kind="ExternalOutput")

    with TileContext(nc) as tc:
        with tc.tile_pool(name="io", bufs=2) as iop, \
             tc.tile_pool(name="pl", bufs=1) as pool, \
             tc.tile_pool(name="ps", bufs=3, space="PSUM") as psp:

            V = nc.vector
            S = nc.scalar

            imgs = {}

            def dma_img(c, nm, row0, nrows):
                img = iop.tile([128, WPAD], f32, tag="img", name=f"img{c}{nm}",
                               bufs=3)
                off0 = c * SHARD_ROWS * WPAD + row0 * WPAD
                nc.sync.dma_start(
                    out=img[0:nrows, :],
                    in_=bass.AP(xr, off0, [[WPAD, nrows], [343, 6], [1, 343]]))
                imgs[(c, nm)] = img

            dma_img(0, "a", 0, 128)
            dma_img(0, "b", 124, 128)

            wt_ = []
            for i in range(8):
                wti = pool.tile([128, 128], f32, tag=f"w{i}", name=f"wt{i}")
                nc.sync.dma_start(out=wti[:], in_=bass.AP(wm, i * 128 * 128,
                                                          [[128, 128], [1, 128]]))
                wt_.append(wti)
            m3t = pool.tile([128, 128], f16, tag="m3", name="m3t")
            nc.sync.dma_start(out=m3t[:], in_=bass.AP(wm16, 0, [[128, 128], [1, 128]]))

            # accumulators per chunk
            acc = {}
            for q in ("a", "b"):
                acc[q] = (
                    pool.tile([128, WG], f32, tag=f"g_{q}", name=f"g_{q}"),
                    pool.tile([128, WG], f32, tag=f"sgx_{q}", name=f"sgx_{q}"),
                    pool.tile([128, WG], f32, tag=f"sgy_{q}", name=f"sgy_{q}"),
                )

            # ---- tail: columns-mode v1-style pass for out rows [248,256) ----
            N, R, RV, RT, RN = NT, NT + 10, NT + 6, NT + 4, NT + 2
            tg = pool.tile([128, RT, 20], f32, tag="tg", name="tg")
            tsgx = pool.tile([128, RT, 20], f32, tag="tsgx", name="tsgx")
            tsgy = pool.tile([128, RT, 20], f32, tag="tsgy", name="tsgy")
            for c in range(3):
                img = pool.tile([128, R, 26], f32, tag="timg", name=f"timg{c}")
                nc.sync.dma_start(out=img[:], in_=bass.AP(
                    xt, c * 128 * 24 * 26, [[24 * 26, 128], [26, R], [1, 26]]))
                tA = pool.tile([128, R, 22], f32, tag="tlA", name=f"tlA{c}")
                tB = pool.tile([128, R, 22], f32, tag="tlB", name=f"tlB{c}")
                tC = pool.tile([128, R, 22], f32, tag="tlC", name=f"tlC{c}")
                tD = pool.tile([128, RV, 22], f32, tag="tlD", name=f"tlD{c}")
                V.tensor_tensor(tA[:], img[:, :, 1:23], img[:, :, 3:25], Alu.add)
                V.tensor_tensor(tB[:], img[:, :, 0:22], img[:, :, 4:26], Alu.add)
                V.scalar_tensor_tensor(tC[:], tA[:], gb, img[:, :, 2:24], Alu.mult, Alu.add)
                V.scalar_tensor_tensor(tC[:], tB[:], ga, tC[:], Alu.mult, Alu.add)
                V.tensor_tensor(tA[:, 0:RV, :], tC[:, 1:RV + 1, :], tC[:, 3:RV + 3, :], Alu.add)
                V.tensor_tensor(tB[:, 0:RV, :], tC[:, 0:RV, :], tC[:, 4:RV + 4, :], Alu.add)
                V.scalar_tensor_tensor(tD[:], tA[:, 0:RV, :], gb, tC[:, 2:RV + 2, :],
                                       Alu.mult, Alu.add)
                V.scalar_tensor_tensor(tD[:], tB[:, 0:RV, :], ga, tD[:], Alu.mult, Alu.add)
                V.tensor_tensor(tA[:, 0:RT, :], tD[:, 0:RT, :], tD[:, 2:RT + 2, :], Alu.add)
                V.scalar_tensor_tensor(tA[:, 0:RT, :], tD[:, 1:RT + 1, :], 2.0,
                                       tA[:, 0:RT, :], Alu.mult, Alu.add)   # t1
                V.tensor_tensor(tB[:, 0:RT, :], tD[:, 0:RT, :], tD[:, 2:RT + 2, :],
                                Alu.subtract)                                # t2
                gx = tsgx[:] if c == 0 else tC[:, 0:RT, 0:20]
                gy = tsgy[:] if c == 0 else tD[:, 0:RT, 0:20]
                w2 = tD[:, 0:RT, 0:20]
                V.tensor_tensor(gx, tA[:, 0:RT, 0:20], tA[:, 0:RT, 2:22], Alu.subtract)
                V.tensor_tensor(w2, tB[:, 0:RT, 0:20], tB[:, 0:RT, 2:22], Alu.add)
                V.scalar_tensor_tensor(gy, tB[:, 0:RT, 1:21], 2.0, w2, Alu.mult, Alu.add)
                q1 = tA[:, 0:RT, 0:20]
                q2 = tB[:, 0:RT, 0:20]
                S.activation(q1, gx, Act.Square)
                S.activation(q2, gy, Act.Square)
                V.tensor_tensor(q1, q1, q2, Alu.add)
                m = tg[:] if c == 0 else tB[:, 0:RT, 0:20]
                S.activation(m, q1, Act.Sqrt)
                if c != 0:
                    V.tensor_tensor(tg[:], tg[:], m, Alu.add)
                    V.tensor_tensor(tsgx[:], tsgx[:], gx, Alu.add)
                    V.tensor_tensor(tsgy[:], tsgy[:], gy, Alu.add)




            for c_ in (1, 2):
                dma_img(c_, "a", 0, 128)
                dma_img(c_, "b", 124, 128)

            shifts = {}
            for c in range(3):
                # ---- hgauss for the three bh tiles of this channel ----
                bhs = {}
                for (nm, row0, nrows, tag) in (
                        ("a", 0, 128, "bhA"), ("b", 124, 128, "bhB")):
                    img = imgs[(c, nm)]
                    s1 = pool.tile([128, WB], f32, tag="S1", name=f"s1_{c}{nm}")
                    s2 = pool.tile([128, WB], f32, tag="S2", name=f"s2_{c}{nm}")
                    bh = pool.tile([128, WB], f32, tag=tag, name=f"bh{nm}{c}")
                    r = slice(0, nrows)
                    V.tensor_tensor(s1[r, :], img[r, 1:WB + 1], img[r, 3:WB + 3], Alu.add)
                    V.tensor_tensor(s2[r, :], img[r, 0:WB], img[r, 4:WB + 4], Alu.add)
                    V.scalar_tensor_tensor(bh[r, :], s1[r, :], gb, img[r, 2:WB + 2],
                                           Alu.mult, Alu.add)
                    V.scalar_tensor_tensor(bh[r, :], s2[r, :], ga, bh[r, :],
                                           Alu.mult, Alu.add)
                    bhs[nm] = bh

                # ---- per chunk: PE T1/T2, then hsobel + mag on DVE ----
                for q in ("a", "b"):
                    if q == "a":
                        main_src, halo_src, halo_k, wia, wib = bhs["a"], bhs["b"], 128, 1, 4
                    else:
                        main_src, halo_src, halo_k, wia, wib = bhs["b"], None, 0, 2, 5
                    S1 = pool.tile([128, WB], f32, tag="S1", name=f"S1_{c}{q}")
                    S2 = pool.tile([128, WB], f32, tag="S2", name=f"S2_{c}{q}")
                    gq, sgxq, sgyq = acc[q]
                    gxt = sgxq if c == 0 else pool.tile(
                        [128, WG], f32, tag="gxT", name=f"gx{c}{q}")
                    gyt = sgyq if c == 0 else pool.tile(
                        [128, WG], f32, tag="gyT", name=f"gy{c}{q}")
                    gx = gxt[:]
                    gy = gyt[:]
                    wt2 = pool.tile([128, WB - 1], f32, tag="wt2", name=f"w2_{c}{q}")
                    for (b0, bw) in BLOCKS:
                        has_halo = halo_src is not None
                        pt1 = psp.tile([128, bw], f32, tag="pT1", name=f"pt1_{c}{q}{b0}")
                        nc.tensor.matmul(pt1[:], wt_[0][:], main_src[:, b0:b0 + bw],
                                         start=True, stop=not has_halo)
                        if has_halo:
                            nc.tensor.matmul(pt1[:], wt_[wia][0:halo_k, :],
                                             halo_src[0:halo_k, b0:b0 + bw],
                                             start=False, stop=True)
                        S.copy(S1[:, b0:b0 + bw], pt1[:])
                        pt2 = psp.tile([128, bw], f32, tag="pT2", name=f"pt2_{c}{q}{b0}")
                        nc.tensor.matmul(pt2[:], wt_[3][:], main_src[:, b0:b0 + bw],
                                         start=True, stop=not has_halo)
                        if has_halo:
                            nc.tensor.matmul(pt2[:], wt_[wib][0:halo_k, :],
                                             halo_src[0:halo_k, b0:b0 + bw],
                                             start=False, stop=True)
                        S.copy(S2[:, b0:b0 + bw], pt2[:])
                        # hsobel for the column range this block completes
                        g0 = max(0, b0 - 2)
                        g1 = min(WG, b0 + bw - 2)
                        u0 = max(0, b0 - 1)
                        u1 = min(WB - 1, b0 + bw - 1)
                        V.tensor_tensor(gxt[:, g0:g1], S1[:, g0:g1],
                                        S1[:, g0 + 2:g1 + 2], Alu.subtract)
                        V.tensor_tensor(wt2[:, u0:u1], S2[:, u0:u1],
                                        S2[:, u0 + 1:u1 + 1], Alu.add)
                        V.tensor_tensor(gyt[:, g0:g1], wt2[:, g0:g1],
                                        wt2[:, g0 + 1:g1 + 1], Alu.add)
                    # final sliver: gx/gy cols [WB-8..WG) done, but blocks end at
                    # b0+bw-2 = WB-2 = WG+2 > WG, so full range is covered.
                    S.activation(S1[:, 0:WG], gx, Act.Square)
                    S.activation(S2[:, 0:WG], gy, Act.Square)
                    V.tensor_tensor(S1[:, 0:WG], S1[:, 0:WG], S2[:, 0:WG], Alu.add)
                    m = gq[:] if c == 0 else S2[:, 0:WG]
                    S.activation(m, S1[:, 0:WG], Act.Sqrt)
                    if c != 0:
                        V.tensor_tensor(gq[:], gq[:], m, Alu.add)
                        V.tensor_tensor(sgxq[:], sgxq[:], gx, Alu.add)
                        V.tensor_tensor(sgyq[:], sgyq[:], gy, Alu.add)


            # ---- NMS per chunk ----
            def nms_chunk(q):
                gq, sgxq, sgyq = acc[q]
                rr = pool.tile([128, WG], f32, tag="bhA", name=f"rr{q}")
                ss = pool.tile([128, WG], f32, tag="bhB", name=f"ss{q}")
                m0 = pool.tile([128, WG], u8, tag="mk0", name=f"m0{q}")
                m2 = pool.tile([128, WG], u8, tag="mk1", name=f"m2{q}")
                d = pool.tile([128, WG], f32, tag="bhX", name=f"d{q}")
                dpos = pool.tile([128, WG], u8, tag="mk2", name=f"dp{q}")
                S.activation(rr[:], sgyq[:], Act.Abs)
                S.activation(ss[:], sgxq[:], Act.Abs)
                V.scalar_tensor_tensor(m0[:], ss[:], t1c, rr[:], Alu.mult, Alu.is_ge)
                V.scalar_tensor_tensor(m2[:], ss[:], t2c, rr[:], Alu.mult, Alu.is_le)
                V.tensor_tensor(d[:], sgxq[:], sgyq[:], Alu.mult)
                V.tensor_scalar(dpos[:], d[:], 0.0, None, Alu.is_ge)

                gU, gD = shifts[q]

                cand1 = pool.tile([128, WC], f32, tag="wt2", name=f"c1{q}")
                cc = pool.tile([128, WC], f32, tag="gxT", name=f"cc{q}")
                cand2 = pool.tile([128, WC], f32, tag="gyT", name=f"c2{q}")
                V.tensor_tensor(cand1[:], gU[:, 2:WG], gD[:, 0:WC], Alu.max)   # SE/NW
                V.tensor_tensor(cc[:], gU[:, 0:WC], gD[:, 2:WG], Alu.max)      # SW/NE
                V.copy_predicated(cc[:], dpos[:, 1:WC + 1], cand1[:])
                V.tensor_tensor(cand2[:], gU[:, 1:WC + 1], gD[:, 1:WC + 1], Alu.max)  # S/N
                V.copy_predicated(cc[:], m2[:, 1:WC + 1], cand2[:])
                cand0 = pool.tile([128, WC], f32, tag="wt2", name=f"c0{q}")
                V.tensor_tensor(cand0[:], gq[:, 2:WG], gq[:, 0:WC], Alu.max)   # E/W
                V.copy_predicated(cc[:], m0[:, 1:WC + 1], cand0[:])

                hp = pool.tile([128, WC], f16, tag="hp", name=f"hp{q}")
                lm = pool.tile([128, W], f16, tag="lm", name=f"lm{q}")
                V.scalar_tensor_tensor(hp[:], cc[:], high, gq[:, 1:WC + 1],
                                       Alu.max, Alu.is_lt)
                V.scalar_tensor_tensor(lm[:], cc[:, 1:W + 1], lowx, gq[:, 2:W + 2],
                                       Alu.max, Alu.is_lt)

                rm1 = pool.tile([128, W], f16, tag="rm1", name=f"rm1{q}")
                rm = pool.tile([128, W], f16, tag="rm", name=f"rm{q}")
                V.tensor_tensor(rm1[:], hp[:, 0:W], hp[:, 2:W + 2], Alu.max)
                V.tensor_tensor(rm[:], rm1[:], hp[:, 1:W + 1], Alu.max)
                # vertical 3-row OR via tridiagonal-ones matmul on PE
                zp = pool.tile([128, W], f16, tag="rm1", name=f"zp{q}")
                for zb in range(0, W, 512):
                    pz = psp.tile([128, 512], f32, tag="pT1", name=f"pz{q}{zb}")
                    nc.tensor.matmul(pz[:], m3t[:], rm[:, zb:zb + 512],
                                     start=True, stop=True)
                    S.copy(zp[:, zb:zb + 512], pz[:])
                outt = iop.tile([128, W], f16, tag="out", name=f"out{q}")
                V.scalar_tensor_tensor(outt[:], zp[:], 0.5, lm[:],
                                       Alu.is_ge, Alu.mult)
                r0, nrows_out = (0, 124) if q == "a" else (124, NR - 124)
                nc.sync.dma_start(
                    out=bass.AP(out, r0 * W, [[W, nrows_out], [1, W]]),
                    in_=outt[2:2 + nrows_out, :])



            for q in ("a", "b"):
                gq = acc[q][0]
                gU = pool.tile([128, WG], f32, tag="gU", name=f"gU{q}")
                gD = pool.tile([128, WG], f32, tag="gD", name=f"gD{q}")
                for sb in range(0, WG, 512):
                    sw = min(512, WG - sb)
                    pu = psp.tile([128, sw], f32, tag="pT1", name=f"pu{q}{sb}")
                    nc.tensor.matmul(pu[:], wt_[6][:], gq[:, sb:sb + sw],
                                     start=True, stop=True)
                    S.copy(gU[:, sb:sb + sw], pu[:])
                    pd = psp.tile([128, sw], f32, tag="pT2", name=f"pd{q}{sb}")
                    nc.tensor.matmul(pd[:], wt_[7][:], gq[:, sb:sb + sw],
                                     start=True, stop=True)
                    S.copy(gD[:, sb:sb + sw], pd[:])
                shifts[q] = (gU, gD)

            nms_chunk("a")

            rr = pool.tile([128, RN, 18], f32, tag="tlA", name="trr")
            ss = pool.tile([128, RN, 18], f32, tag="tlB", name="tss")
            m0 = pool.tile([128, RN, 18], u8, tag="tmk0", name="tm0")
            m2 = pool.tile([128, RN, 18], u8, tag="tmk1", name="tm2")
            d = pool.tile([128, RN, 18], f32, tag="tlC", name="td")
            dpos = pool.tile([128, RN, 18], u8, tag="tmk2", name="tdp")
            S.activation(rr[:], tsgy[:, 1:RN + 1, 1:19], Act.Abs)
            S.activation(ss[:], tsgx[:, 1:RN + 1, 1:19], Act.Abs)
            V.scalar_tensor_tensor(m0[:], ss[:], t1c, rr[:], Alu.mult, Alu.is_ge)
            V.scalar_tensor_tensor(m2[:], ss[:], t2c, rr[:], Alu.mult, Alu.is_le)
            V.tensor_tensor(d[:], tsgx[:, 1:RN + 1, 1:19], tsgy[:, 1:RN + 1, 1:19], Alu.mult)
            V.tensor_scalar(dpos[:], d[:], 0.0, None, Alu.is_ge)
            cand = pool.tile([128, RN, 18], f32, tag="tlD", name="tc1")
            cc = pool.tile([128, RN, 18], f32, tag="tlA", name="tcc")
            cand2 = pool.tile([128, RN, 18], f32, tag="tlB", name="tc2")
            cand3 = pool.tile([128, RN, 18], f32, tag="tlC", name="tc0")
            V.tensor_tensor(cand[:], tg[:, 2:RN + 2, 2:20], tg[:, 0:RN, 0:18], Alu.max)
            V.tensor_tensor(cc[:], tg[:, 2:RN + 2, 0:18], tg[:, 0:RN, 2:20], Alu.max)
            V.copy_predicated(cc[:], dpos[:], cand[:])
            V.tensor_tensor(cand2[:], tg[:, 2:RN + 2, 1:19], tg[:, 0:RN, 1:19], Alu.max)
            V.copy_predicated(cc[:], m2[:], cand2[:])
            V.tensor_tensor(cand3[:], tg[:, 1:RN + 1, 2:20], tg[:, 1:RN + 1, 0:18], Alu.max)
            V.copy_predicated(cc[:], m0[:], cand3[:])
            hp = pool.tile([128, RN, 18], f16, tag="thp", name="thp")
            lm = pool.tile([128, N, 16], f32, tag="tlC", name="tlm")
            V.scalar_tensor_tensor(hp[:], cc[:], high, tg[:, 1:RN + 1, 1:19],
                                   Alu.max, Alu.is_lt)
            V.scalar_tensor_tensor(lm[:], cc[:, 1:N + 1, 1:17], lowx,
                                   tg[:, 2:RN, 2:18], Alu.max, Alu.is_lt)
            rm1 = pool.tile([128, RN, 16], f16, tag="trm1", name="trm1")
            rm = pool.tile([128, RN, 16], f16, tag="trm", name="trm")
            cm = pool.tile([128, N, 16], f16, tag="tcm", name="tcm")
            V.tensor_tensor(rm1[:], hp[:, :, 0:16], hp[:, :, 2:18], Alu.max)
            V.tensor_tensor(rm[:], rm1[:], hp[:, :, 1:17], Alu.max)
            V.tensor_tensor(cm[:], rm[:, 0:N, :], rm[:, 2:RN, :], Alu.max)
            V.tensor_tensor(cm[:], cm[:], rm[:, 1:N + 1, :], Alu.max)
            outt = pool.tile([128, N, 16], f32, tag="tlD", name="touts")
            V.tensor_tensor(outt[:], lm[:], cm[:], Alu.mult)
            nc.sync.dma_start(
                out=bass.AP(out_t, 0, [[NT * 16, 128], [16, N], [1, 16]]),
                in_=outt[:])

            nms_chunk("b")

    nc.finalize()
    return nc


def _get_compiled(low, high):
    key = (low, high)
    if key not in _COMPILED:
        _COMPILED[key] = _build(low, high)
    return _COMPILED[key]


def kernel(img, threshold1, threshold2, _trace=False):
    from concourse import bass_utils

    t1 = float(np.asarray(threshold1))
    t2 = float(np.asarray(threshold2))
    low, high = min(t1, t2), max(t1, t2)

    x = np.ascontiguousarray(np.asarray(img, dtype=np.float32)[0])  # [3,H,W]
    xp = np.zeros((3, H + 2 * HALO, W + 2 * HALO), dtype=np.float32)
    xp[:, HALO:HALO + H, HALO:HALO + W] = x

    wmv = _weights()
    m3v = np.zeros((128, 128), np.float16)
    for p in range(128):
        for j in range(max(0, p - 1), min(128, p + 2)):
            m3v[p, j] = 1.0
    win = np.lib.stride_tricks.sliding_window_view(xp, 26, axis=2)[:, :, ::16, :]
    in_maps = []
    for k in range(8):
        rows = np.ascontiguousarray(xp[:, k * RPC:k * RPC + SHARD_ROWS, :])
        tw = win[:, k * RPC + NR:k * RPC + NR + NT + 10]     # [3,24,128,26]
        packed = np.ascontiguousarray(tw.transpose(0, 2, 1, 3))
        in_maps.append({"xr": rows, "xt": packed, "wm": wmv, "wm16": m3v})

    nc = _get_compiled(low, high)
    res = bass_utils.run_bass_kernel_spmd(nc, in_maps, core_ids=list(range(8)),
                                          trace=_trace)

    full = np.zeros((1, 1, H, W), dtype=np.float32)
    for k in range(8):
        o = res.results[k]["out"]          # [248, 2048] f16
        ot = res.results[k]["out_t"]       # [128, 8, 16] f32
        full[0, 0, k * RPC:k * RPC + NR, :] = o.astype(np.float32)
        full[0, 0, k * RPC + NR:(k + 1) * RPC, :] = (
            ot.transpose(1, 0, 2).reshape(NT, W))
    full[:, :, 0, :] = 0.0
    full[:, :, -1, :] = 0.0
    full[:, :, :, 0] = 0.0
    full[:, :, :, -1] = 0.0
    if _trace:
        kernel._last_results = res
    return full


# revision 10
# speedup vs baseline: 1.9204x; 1.0609x over previous
"""Canny v3: rows-on-partitions + PE band-matmuls for vertical convs.

Per core (256 out rows, shard rows [-5,261) zero-padded):
 - two rows-mode chunks: chunk a = T/g rows [-2,126) -> out [0,124),
   chunk b = T rows [122,250) -> out [124,248).  Partition = image row,
   free dim = full 2054/2052-wide rows, so every DVE op covers 128 rows
   at ~2054 elems (vs ~2900 elems per 128 rows in the columns layout).
 - vertical (gaussian o sobel) 7-tap convs fused into PE matmuls with
   band weight matrices; halo rows accumulate from the neighboring bh
   tile via a second matmul into the same PSUM bank.
 - NMS vertical/diagonal neighbors via SBUF->SBUF DMA row-shifted
   copies of the g plane (and of rm for hysteresis).
 - last 8 rows per core done in a small v1-style columns-mode pass.
"""

import numpy as np

_COMPILED = {}

H = 2048
W = 2048
HALO = 5
RPC = H // 8                      # 256 rows per core
SHARD_ROWS = RPC + 2 * HALO       # 266
WPAD = W + 2 * HALO               # 2058
WB = W + 6                        # 2054: bh/T cols [-3, 2051)
WG = W + 4                        # 2052: g/gx/gy cols [-2, 2050)
WC = W + 2                        # 2050: cand/cc/hp cols [-1, 2049)
NT = 14                           # tail rows (columns-mode)
NR = RPC - NT                     # 248 rows via rows-mode
BLOCKS = [(0, 512), (512, 512), (1024, 512), (1536, 512), (2048, WB - 2048)]


def _weights():
    g5 = np.exp(-0.5 * (np.arange(5) - 2.0) ** 2).astype(np.float32)
    w7a = np.convolve(np.array([1, 2, 1], np.float32), g5).astype(np.float32)
    w7b = np.convolve(np.array([1, 0, -1], np.float32), g5).astype(np.float32)

    def w7i(w7, i):
        return float(w7[i]) if 0 <= i < 7 else 0.0

    wm = np.zeros((8, 128, 128), np.float32)
    for p in range(128):
        for j in range(128):
            wm[0, p, j] = w7i(w7a, p - j)          # Wa_main
            wm[3, p, j] = w7i(w7b, p - j)          # Wb_main
            if 4 <= p < 10:
                wm[1, p, j] = w7i(w7a, p + 124 - j)  # Wa_h1 (chunk a halo from bh_b)
                wm[4, p, j] = w7i(w7b, p + 124 - j)  # Wb_h1
            if p < 6:
                wm[2, p, j] = w7i(w7a, p + 128 - j)  # Wa_hx (chunk b halo from bhx)
                wm[5, p, j] = w7i(w7b, p + 128 - j)  # Wb_hx
            if p == j + 1:
                wm[6, p, j] = 1.0                    # SU: gU[j] = g[j+1]
            if p == j - 1:
                wm[7, p, j] = 1.0                    # SD: gD[j] = g[j-1]
    return wm


def _build(low, high):
    import concourse.bass as bass
    import concourse.bacc as bacc
    import concourse.mybir as mybir
    from concourse.tile import TileContext

    f32 = mybir.dt.float32
    f16 = mybir.dt.float16
    u8 = mybir.dt.uint8
    Alu = mybir.AluOpType
    Act = mybir.ActivationFunctionType

    g5 = np.exp(-0.5 * (np.arange(5) - 2.0) ** 2).astype(np.float32)
    ga = float(g5[0])
    gb = float(g5[1])
    t1c = float(np.float32(np.tan(np.deg2rad(np.float64(22.5)))))
    t2c = float(np.float32(np.tan(np.deg2rad(np.float64(67.5)))))
    lowx = float(np.nextafter(np.float32(low), np.float32(0.0)))

    nc = bacc.Bacc()
    xr = nc.dram_tensor("xr", [3, SHARD_ROWS, WPAD], f32, kind="ExternalInput")
    xt = nc.dram_tensor("xt", [3, 128, 24, 26], f32, kind="ExternalInput")
    wm = nc.dram_tensor("wm", [8, 128, 128], f32, kind="ExternalInput")
    wm16 = nc.dram_tensor("wm16", [128, 128], f16, kind="ExternalInput")
    out = nc.dram_tensor("out", [NR, W], f16, kind="ExternalOutput")
    out_t = nc.dram_tensor("out_t", [128, NT, 16], f32, kind="ExternalOutput")

    with TileContext(nc) as tc:
        with tc.tile_pool(name="io", bufs=2) as iop, \
             tc.tile_pool(name="pl", bufs=1) as pool, \
             tc.tile_pool(name="ps", bufs=3, space="PSUM") as psp:

            V = nc.vector
            S = nc.scalar

            imgs = {}

            def dma_img(c, nm, row0, nrows):
                img = iop.tile([128, WPAD], f32, tag="img", name=f"img{c}{nm}",
                               bufs=3)
                off0 = c * SHARD_ROWS * WPAD + row0 * WPAD
                nc.sync.dma_start(
                    out=img[0:nrows, :],
                    in_=bass.AP(xr, off0, [[WPAD, nrows], [343, 6], [1, 343]]))
                imgs[(c, nm)] = img

            timgs = []
            for c_ in range(3):
                ti = pool.tile([128, NT + 10, 26], f32, tag="timg",
                               name=f"timg{c_}", bufs=2)
                nc.sync.dma_start(out=ti[:], in_=bass.AP(
                    xt, c_ * 128 * 24 * 26, [[24 * 26, 128], [26, NT + 10], [1, 26]]))
                timgs.append(ti)

            dma_img(0, "a", 0, 128)
            dma_img(0, "b", 124, 128)

            wt_ = []
            for i in range(8):
                if i in (2, 5):
                    wt_.append(None)
                    continue
                wti = pool.tile([128, 128], f32, tag=f"w{i}", name=f"wt{i}")
                nc.sync.dma_start(out=wti[:], in_=bass.AP(wm, i * 128 * 128,
                                                          [[128, 128], [1, 128]]))
                wt_.append(wti)
            m3t = pool.tile([128, 128], f16, tag="m3", name="m3t")
            nc.sync.dma_start(out=m3t[:], in_=bass.AP(wm16, 0, [[128, 128], [1, 128]]))

            # accumulators per chunk
            acc = {}
            for q in ("a", "b"):
                acc[q] = (
                    pool.tile([128, WG], f32, tag=f"g_{q}", name=f"g_{q}"),
                    pool.tile([128, WG], f32, tag=f"sgx_{q}", name=f"sgx_{q}"),
                    pool.tile([128, WG], f32, tag=f"sgy_{q}", name=f"sgy_{q}"),
                )

            # ---- tail: columns-mode v1-style pass for out rows [248,256) ----
            N, R, RV, RT, RN = NT, NT + 10, NT + 6, NT + 4, NT + 2
            tg = pool.tile([128, RT, 20], f32, tag="tg", name="tg")
            tsgx = pool.tile([128, RT, 20], f32, tag="tsgx", name="tsgx")
            tsgy = pool.tile([128, RT, 20], f32, tag="tsgy", name="tsgy")
            for c in range(3):
                img = timgs[c]
                tA = pool.tile([128, R, 22], f32, tag="tlA", name=f"tlA{c}")
                tB = pool.tile([128, R, 22], f32, tag="tlB", name=f"tlB{c}")
                tC = pool.tile([128, R, 22], f32, tag="tlC", name=f"tlC{c}")
                tD = pool.tile([128, RV, 22], f32, tag="tlD", name=f"tlD{c}")
                V.tensor_tensor(tA[:], img[:, :, 1:23], img[:, :, 3:25], Alu.add)
                V.tensor_tensor(tB[:], img[:, :, 0:22], img[:, :, 4:26], Alu.add)
                V.scalar_tensor_tensor(tC[:], tA[:], gb, img[:, :, 2:24], Alu.mult, Alu.add)
                V.scalar_tensor_tensor(tC[:], tB[:], ga, tC[:], Alu.mult, Alu.add)
                V.tensor_tensor(tA[:, 0:RV, :], tC[:, 1:RV + 1, :], tC[:, 3:RV + 3, :], Alu.add)
                V.tensor_tensor(tB[:, 0:RV, :], tC[:, 0:RV, :], tC[:, 4:RV + 4, :], Alu.add)
                V.scalar_tensor_tensor(tD[:], tA[:, 0:RV, :], gb, tC[:, 2:RV + 2, :],
                                       Alu.mult, Alu.add)
                V.scalar_tensor_tensor(tD[:], tB[:, 0:RV, :], ga, tD[:], Alu.mult, Alu.add)
                V.tensor_tensor(tA[:, 0:RT, :], tD[:, 0:RT, :], tD[:, 2:RT + 2, :], Alu.add)
                V.scalar_tensor_tensor(tA[:, 0:RT, :], tD[:, 1:RT + 1, :], 2.0,
                                       tA[:, 0:RT, :], Alu.mult, Alu.add)   # t1
                V.tensor_tensor(tB[:, 0:RT, :], tD[:, 0:RT, :], tD[:, 2:RT + 2, :],
                                Alu.subtract)                                # t2
                gx = tsgx[:] if c == 0 else tC[:, 0:RT, 0:20]
                gy = tsgy[:] if c == 0 else tD[:, 0:RT, 0:20]
                w2 = tD[:, 0:RT, 0:20]
                V.tensor_tensor(gx, tA[:, 0:RT, 0:20], tA[:, 0:RT, 2:22], Alu.subtract)
                V.tensor_tensor(w2, tB[:, 0:RT, 0:20], tB[:, 0:RT, 2:22], Alu.add)
                V.scalar_tensor_tensor(gy, tB[:, 0:RT, 1:21], 2.0, w2, Alu.mult, Alu.add)
                q1 = tA[:, 0:RT, 0:20]
                q2 = tB[:, 0:RT, 0:20]
                S.activation(q1, gx, Act.Square)
                S.activation(q2, gy, Act.Square)
                V.tensor_tensor(q1, q1, q2, Alu.add)
                m = tg[:] if c == 0 else tB[:, 0:RT, 0:20]
                S.activation(m, q1, Act.Sqrt)
                if c != 0:
                    V.tensor_tensor(tg[:], tg[:], m, Alu.add)
                    V.tensor_tensor(tsgx[:], tsgx[:], gx, Alu.add)
                    V.tensor_tensor(tsgy[:], tsgy[:], gy, Alu.add)




            for c_ in (1, 2):
                dma_img(c_, "a", 0, 128)
                dma_img(c_, "b", 124, 128)

            shifts = {}
            for c in range(3):
                # ---- hgauss for the three bh tiles of this channel ----
                bhs = {}
                for (nm, row0, nrows, tag) in (
                        ("a", 0, 128, "bhA"), ("b", 124, 128, "bhB")):
                    img = imgs[(c, nm)]
                    s1 = pool.tile([128, WB], f32, tag="S1", name=f"s1_{c}{nm}")
                    s2 = pool.tile([128, WB], f32, tag="S2", name=f"s2_{c}{nm}")
                    bh = pool.tile([128, WB], f32, tag=tag, name=f"bh{nm}{c}")
                    r = slice(0, nrows)
                    V.tensor_tensor(s1[r, :], img[r, 1:WB + 1], img[r, 3:WB + 3], Alu.add)
                    V.tensor_tensor(s2[r, :], img[r, 0:WB], img[r, 4:WB + 4], Alu.add)
                    V.scalar_tensor_tensor(bh[r, :], s1[r, :], gb, img[r, 2:WB + 2],
                                           Alu.mult, Alu.add)
                    V.scalar_tensor_tensor(bh[r, :], s2[r, :], ga, bh[r, :],
                                           Alu.mult, Alu.add)
                    bhs[nm] = bh

                # ---- per chunk: PE T1/T2, then hsobel + mag on DVE ----
                for q in ("a", "b"):
                    if q == "a":
                        main_src, halo_src, halo_k, wia, wib = bhs["a"], bhs["b"], 128, 1, 4
                    else:
                        main_src, halo_src, halo_k, wia, wib = bhs["b"], None, 0, 2, 5
                    S1 = pool.tile([128, WB], f32, tag="S1", name=f"S1_{c}{q}")
                    S2 = pool.tile([128, WB], f32, tag="S2", name=f"S2_{c}{q}")
                    gq, sgxq, sgyq = acc[q]
                    gxt = sgxq if c == 0 else pool.tile(
                        [128, WG], f32, tag="gxT", name=f"gx{c}{q}")
                    gyt = sgyq if c == 0 else pool.tile(
                        [128, WG], f32, tag="gyT", name=f"gy{c}{q}")
                    gx = gxt[:]
                    gy = gyt[:]
                    wt2 = pool.tile([128, WB - 1], f32, tag="wt2", name=f"w2_{c}{q}")
                    for (b0, bw) in BLOCKS:
                        has_halo = halo_src is not None
                        pt1 = psp.tile([128, bw], f32, tag="pT1", name=f"pt1_{c}{q}{b0}")
                        nc.tensor.matmul(pt1[:], wt_[0][:], main_src[:, b0:b0 + bw],
                                         start=True, stop=not has_halo)
                        if has_halo:
                            nc.tensor.matmul(pt1[:], wt_[wia][0:halo_k, :],
                                             halo_src[0:halo_k, b0:b0 + bw],
                                             start=False, stop=True)
                        S.copy(S1[:, b0:b0 + bw], pt1[:])
                        pt2 = psp.tile([128, bw], f32, tag="pT2", name=f"pt2_{c}{q}{b0}")
                        nc.tensor.matmul(pt2[:], wt_[3][:], main_src[:, b0:b0 + bw],
                                         start=True, stop=not has_halo)
                        if has_halo:
                            nc.tensor.matmul(pt2[:], wt_[wib][0:halo_k, :],
                                             halo_src[0:halo_k, b0:b0 + bw],
                                             start=False, stop=True)
                        S.copy(S2[:, b0:b0 + bw], pt2[:])
                        # hsobel for the column range this block completes
                        g0 = max(0, b0 - 2)
                        g1 = min(WG, b0 + bw - 2)
                        u0 = max(0, b0 - 1)
                        u1 = min(WB - 1, b0 + bw - 1)
                        V.tensor_tensor(gxt[:, g0:g1], S1[:, g0:g1],
                                        S1[:, g0 + 2:g1 + 2], Alu.subtract)
                        V.tensor_tensor(wt2[:, u0:u1], S2[:, u0:u1],
                                        S2[:, u0 + 1:u1 + 1], Alu.add)
                        V.tensor_tensor(gyt[:, g0:g1], wt2[:, g0:g1],
                                        wt2[:, g0 + 1:g1 + 1], Alu.add)
                    # final sliver: gx/gy cols [WB-8..WG) done, but blocks end at
                    # b0+bw-2 = WB-2 = WG+2 > WG, so full range is covered.
                    S.activation(S1[:, 0:WG], gx, Act.Square)
                    S.activation(S2[:, 0:WG], gy, Act.Square)
                    V.tensor_tensor(S1[:, 0:WG], S1[:, 0:WG], S2[:, 0:WG], Alu.add)
                    m = gq[:] if c == 0 else S2[:, 0:WG]
                    S.activation(m, S1[:, 0:WG], Act.Sqrt)
                    if c != 0:
                        V.tensor_tensor(gq[:], gq[:], m, Alu.add)
                        V.tensor_tensor(sgxq[:], sgxq[:], gx, Alu.add)
                        V.tensor_tensor(sgyq[:], sgyq[:], gy, Alu.add)


            # ---- NMS per chunk ----
            def nms_chunk(q):
                gq, sgxq, sgyq = acc[q]
                rr = pool.tile([128, WG], f32, tag="bhA", name=f"rr{q}")
                ss = pool.tile([128, WG], f32, tag="bhB", name=f"ss{q}")
                m0 = pool.tile([128, WG], u8, tag="mk0", name=f"m0{q}")
                m2 = pool.tile([128, WG], u8, tag="mk1", name=f"m2{q}")
                d = pool.tile([128, WG], f32, tag="bhX", name=f"d{q}")
                dpos = pool.tile([128, WG], u8, tag="mk2", name=f"dp{q}")
                S.activation(rr[:], sgyq[:], Act.Abs)
                S.activation(ss[:], sgxq[:], Act.Abs)
                V.scalar_tensor_tensor(m0[:], ss[:], t1c, rr[:], Alu.mult, Alu.is_ge)
                V.scalar_tensor_tensor(m2[:], ss[:], t2c, rr[:], Alu.mult, Alu.is_le)
                V.tensor_tensor(d[:], sgxq[:], sgyq[:], Alu.mult)
                V.tensor_scalar(dpos[:], d[:], 0.0, None, Alu.is_ge)

                gU, gD = shifts[q]

                cand1 = pool.tile([128, WC], f32, tag="wt2", name=f"c1{q}")
                cc = pool.tile([128, WC], f32, tag="gxT", name=f"cc{q}")
                cand2 = pool.tile([128, WC], f32, tag="gyT", name=f"c2{q}")
                V.tensor_tensor(cand1[:], gU[:, 2:WG], gD[:, 0:WC], Alu.max)   # SE/NW
                V.tensor_tensor(cc[:], gU[:, 0:WC], gD[:, 2:WG], Alu.max)      # SW/NE
                V.copy_predicated(cc[:], dpos[:, 1:WC + 1], cand1[:])
                V.tensor_tensor(cand2[:], gU[:, 1:WC + 1], gD[:, 1:WC + 1], Alu.max)  # S/N
                V.copy_predicated(cc[:], m2[:, 1:WC + 1], cand2[:])
                cand0 = pool.tile([128, WC], f32, tag="wt2", name=f"c0{q}")
                V.tensor_tensor(cand0[:], gq[:, 2:WG], gq[:, 0:WC], Alu.max)   # E/W
                V.copy_predicated(cc[:], m0[:, 1:WC + 1], cand0[:])

                hp = pool.tile([128, WC], f16, tag="hp", name=f"hp{q}")
                lm = pool.tile([128, W], f16, tag="lm", name=f"lm{q}")
                V.scalar_tensor_tensor(hp[:], cc[:], high, gq[:, 1:WC + 1],
                                       Alu.max, Alu.is_lt)
                V.scalar_tensor_tensor(lm[:], cc[:, 1:W + 1], lowx, gq[:, 2:W + 2],
                                       Alu.max, Alu.is_lt)

                rm1 = pool.tile([128, W], f16, tag="rm1", name=f"rm1{q}")
                rm = pool.tile([128, W], f16, tag="rm", name=f"rm{q}")
                V.tensor_tensor(rm1[:], hp[:, 0:W], hp[:, 2:W + 2], Alu.max)
                V.tensor_tensor(rm[:], rm1[:], hp[:, 1:W + 1], Alu.max)
                # vertical 3-row OR via tridiagonal-ones matmul on PE
                zp = pool.tile([128, W], f16, tag="rm1", name=f"zp{q}")
                for zb in range(0, W, 512):
                    pz = psp.tile([128, 512], f32, tag="pT1", name=f"pz{q}{zb}")
                    nc.tensor.matmul(pz[:], m3t[:], rm[:, zb:zb + 512],
                                     start=True, stop=True)
                    S.copy(zp[:, zb:zb + 512], pz[:])
                outt = iop.tile([128, W], f16, tag="out", name=f"out{q}")
                r0, nrows_out = (0, 124) if q == "a" else (124, NR - 124)
                hw3 = W // 2
                for h0 in (0, hw3):
                    V.scalar_tensor_tensor(outt[:, h0:h0 + hw3], zp[:, h0:h0 + hw3],
                                           0.5, lm[:, h0:h0 + hw3],
                                           Alu.is_ge, Alu.mult)
                    nc.sync.dma_start(
                        out=bass.AP(out, r0 * W + h0, [[W, nrows_out], [1, hw3]]),
                        in_=outt[2:2 + nrows_out, h0:h0 + hw3])



            for q in ("a", "b"):
                gq = acc[q][0]
                gU = pool.tile([128, WG], f32, tag="gU", name=f"gU{q}")
                gD = pool.tile([128, WG], f32, tag="gD", name=f"gD{q}")
                for sb in range(0, WG, 512):
                    sw = min(512, WG - sb)
                    pu = psp.tile([128, sw], f32, tag="pT1", name=f"pu{q}{sb}")
                    nc.tensor.matmul(pu[:], wt_[6][:], gq[:, sb:sb + sw],
                                     start=True, stop=True)
                    S.copy(gU[:, sb:sb + sw], pu[:])
                    pd = psp.tile([128, sw], f32, tag="pT2", name=f"pd{q}{sb}")
                    nc.tensor.matmul(pd[:], wt_[7][:], gq[:, sb:sb + sw],
                                     start=True, stop=True)
                    S.copy(gD[:, sb:sb + sw], pd[:])
                shifts[q] = (gU, gD)

            nms_chunk("a")

            rr = pool.tile([128, RN, 18], f32, tag="tlA", name="trr")
            ss = pool.tile([128, RN, 18], f32, tag="tlB", name="tss")
            m0 = pool.tile([128, RN, 18], u8, tag="tmk0", name="tm0")
            m2 = pool.tile([128, RN, 18], u8, tag="tmk1", name="tm2")
            d = pool.tile([128, RN, 18], f32, tag="tlC", name="td")
            dpos = pool.tile([128, RN, 18], u8, tag="tmk2", name="tdp")
            S.activation(rr[:], tsgy[:, 1:RN + 1, 1:19], Act.Abs)
            S.activation(ss[:], tsgx[:, 1:RN + 1, 1:19], Act.Abs)
            V.scalar_tensor_tensor(m0[:], ss[:], t1c, rr[:], Alu.mult, Alu.is_ge)
            V.scalar_tensor_tensor(m2[:], ss[:], t2c, rr[:], Alu.mult, Alu.is_le)
            V.tensor_tensor(d[:], tsgx[:, 1:RN + 1, 1:19], tsgy[:, 1:RN + 1, 1:19], Alu.mult)
            V.tensor_scalar(dpos[:], d[:], 0.0, None, Alu.is_ge)
            cand = pool.tile([128, RN, 18], f32, tag="tlD", name="tc1")
            cc = pool.tile([128, RN, 18], f32, tag="tlA", name="tcc")
            cand2 = pool.tile([128, RN, 18], f32, tag="tlB", name="tc2")
            cand3 = pool.tile([128, RN, 18], f32, tag="tlC", name="tc0")
            V.tensor_tensor(cand[:], tg[:, 2:RN + 2, 2:20], tg[:, 0:RN, 0:18], Alu.max)
            V.tensor_tensor(cc[:], tg[:, 2:RN + 2, 0:18], tg[:, 0:RN, 2:20], Alu.max)
            V.copy_predicated(cc[:], dpos[:], cand[:])
            V.tensor_tensor(cand2[:], tg[:, 2:RN + 2, 1:19], tg[:, 0:RN, 1:19], Alu.max)
            V.copy_predicated(cc[:], m2[:], cand2[:])
            V.tensor_tensor(cand3[:], tg[:, 1:RN + 1, 2:20], tg[:, 1:RN + 1, 0:18], Alu.max)
            V.copy_predicated(cc[:], m0[:], cand3[:])
            hp = pool.tile([128, RN, 18], f16, tag="thp", name="thp")
            lm = pool.tile([128, N, 16], f32, tag="tlC", name="tlm")
            V.scalar_tensor_tensor(hp[:], cc[:], high, tg[:, 1:RN + 1, 1:19],
                                   Alu.max, Alu.is_lt)
            V.scalar_tensor_tensor(lm[:], cc[:, 1:N + 1, 1:17], lowx,
                                   tg[:, 2:RN, 2:18], Alu.max, Alu.is_lt)
            rm1 = pool.tile([128, RN, 16], f16, tag="trm1", name="trm1")
            rm = pool.tile([128, RN, 16], f16, tag="trm", name="trm")
            cm = pool.tile([128, N, 16], f16, tag="tcm", name="tcm")
            V.tensor_tensor(rm1[:], hp[:, :, 0:16], hp[:, :, 2:18], Alu.max)
            V.tensor_tensor(rm[:], rm1[:], hp[:, :, 1:17], Alu.max)
            V.tensor_tensor(cm[:], rm[:, 0:N, :], rm[:, 2:RN, :], Alu.max)
            V.tensor_tensor(cm[:], cm[:], rm[:, 1:N + 1, :], Alu.max)
            outt = pool.tile([128, N, 16], f32, tag="tlD", name="touts")
            V.tensor_tensor(outt[:], lm[:], cm[:], Alu.mult)
            nc.sync.dma_start(
                out=bass.AP(out_t, 0, [[NT * 16, 128], [16, N], [1, 16]]),
                in_=outt[:])

            nms_chunk("b")

    nc.finalize()
    return nc


def _get_compiled(low, high):
    key = (low, high)
    if key not in _COMPILED:
        _COMPILED[key] = _build(low, high)
    return _COMPILED[key]


def kernel(img, threshold1, threshold2, _trace=False):
    from concourse import bass_utils

    t1 = float(np.asarray(threshold1))
    t2 = float(np.asarray(threshold2))
    low, high = min(t1, t2), max(t1, t2)

    x = np.ascontiguousarray(np.asarray(img, dtype=np.float32)[0])  # [3,H,W]
    xp = np.zeros((3, H + 2 * HALO, W + 2 * HALO), dtype=np.float32)
    xp[:, HALO:HALO + H, HALO:HALO + W] = x

    wmv = _weights()
    m3v = np.zeros((128, 128), np.float16)
    for p in range(128):
        for j in range(max(0, p - 1), min(128, p + 2)):
            m3v[p, j] = 1.0
    win = np.lib.stride_tricks.sliding_window_view(xp, 26, axis=2)[:, :, ::16, :]
    in_maps = []
    for k in range(8):
        rows = np.ascontiguousarray(xp[:, k * RPC:k * RPC + SHARD_ROWS, :])
        tw = win[:, k * RPC + NR:k * RPC + NR + NT + 10]     # [3,24,128,26]
        packed = np.ascontiguousarray(tw.transpose(0, 2, 1, 3))
        in_maps.append({"xr": rows, "xt": packed, "wm": wmv, "wm16": m3v})

    nc = _get_compiled(low, high)
    res = bass_utils.run_bass_kernel_spmd(nc, in_maps, core_ids=list(range(8)),
                                          trace=_trace)

    full = np.zeros((1, 1, H, W), dtype=np.float32)
    for k in range(8):
        o = res.results[k]["out"]          # [248, 2048] f16
        ot = res.results[k]["out_t"]       # [128, 8, 16] f32
        full[0, 0, k * RPC:k * RPC + NR, :] = o.astype(np.float32)
        full[0, 0, k * RPC + NR:(k + 1) * RPC, :] = (
            ot.transpose(1, 0, 2).reshape(NT, W))
    full[:, :, 0, :] = 0.0
    full[:, :, -1, :] = 0.0
    full[:, :, :, 0] = 0.0
    full[:, :, :, -1] = 0.0
    if _trace:
        kernel._last_results = res
    return full


# revision 11
# speedup vs baseline: 1.9399x; 1.0101x over previous
"""Canny v3: rows-on-partitions + PE band-matmuls for vertical convs.

Per core (256 out rows, shard rows [-5,261) zero-padded):
 - two rows-mode chunks: chunk a = T/g rows [-2,126) -> out [0,124),
   chunk b = T rows [122,250) -> out [124,248).  Partition = image row,
   free dim = full 2054/2052-wide rows, so every DVE op covers 128 rows
   at ~2054 elems (vs ~2900 elems per 128 rows in the columns layout).
 - vertical (gaussian o sobel) 7-tap convs fused into PE matmuls with
   band weight matrices; halo rows accumulate from the neighboring bh
   tile via a second matmul into the same PSUM bank.
 - NMS vertical/diagonal neighbors via SBUF->SBUF DMA row-shifted
   copies of the g plane (and of rm for hysteresis).
 - last 8 rows per core done in a small v1-style columns-mode pass.
"""

import numpy as np

_COMPILED = {}

H = 2048
W = 2048
HALO = 5
RPC = H // 8                      # 256 rows per core
SHARD_ROWS = RPC + 2 * HALO       # 266
WPAD = W + 2 * HALO               # 2058
WB = W + 6                        # 2054: bh/T cols [-3, 2051)
WG = W + 4                        # 2052: g/gx/gy cols [-2, 2050)
WC = W + 2                        # 2050: cand/cc/hp cols [-1, 2049)
NT = 14                           # tail rows (columns-mode)
NR = RPC - NT                     # 248 rows via rows-mode
BLOCKS = [(0, 512), (512, 512), (1024, 512), (1536, 512), (2048, WB - 2048)]


def _weights():
    g5 = np.exp(-0.5 * (np.arange(5) - 2.0) ** 2).astype(np.float32)
    w7a = np.convolve(np.array([1, 2, 1], np.float32), g5).astype(np.float32)
    w7b = np.convolve(np.array([1, 0, -1], np.float32), g5).astype(np.float32)

    def w7i(w7, i):
        return float(w7[i]) if 0 <= i < 7 else 0.0

    wm = np.zeros((8, 128, 128), np.float32)
    for p in range(128):
        for j in range(128):
            wm[0, p, j] = w7i(w7a, p - j)          # Wa_main
            wm[3, p, j] = w7i(w7b, p - j)          # Wb_main
            if 4 <= p < 10:
                wm[1, p, j] = w7i(w7a, p + 124 - j)  # Wa_h1 (chunk a halo from bh_b)
                wm[4, p, j] = w7i(w7b, p + 124 - j)  # Wb_h1
            if p < 6:
                wm[2, p, j] = w7i(w7a, p + 128 - j)  # Wa_hx (chunk b halo from bhx)
                wm[5, p, j] = w7i(w7b, p + 128 - j)  # Wb_hx
            if p == j + 1:
                wm[6, p, j] = 1.0                    # SU: gU[j] = g[j+1]
            if p == j - 1:
                wm[7, p, j] = 1.0                    # SD: gD[j] = g[j-1]
    return wm


def _build(low, high):
    import concourse.bass as bass
    import concourse.bacc as bacc
    import concourse.mybir as mybir
    from concourse.tile import TileContext

    f32 = mybir.dt.float32
    f16 = mybir.dt.float16
    u8 = mybir.dt.uint8
    Alu = mybir.AluOpType
    Act = mybir.ActivationFunctionType

    g5 = np.exp(-0.5 * (np.arange(5) - 2.0) ** 2).astype(np.float32)
    ga = float(g5[0])
    gb = float(g5[1])
    t1c = float(np.float32(np.tan(np.deg2rad(np.float64(22.5)))))
    t2c = float(np.float32(np.tan(np.deg2rad(np.float64(67.5)))))
    lowx = float(np.nextafter(np.float32(low), np.float32(0.0)))

    nc = bacc.Bacc()
    xr = nc.dram_tensor("xr", [3, SHARD_ROWS, WPAD], f32, kind="ExternalInput")
    xt = nc.dram_tensor("xt", [3, 128, 24, 26], f32, kind="ExternalInput")
    wm = nc.dram_tensor("wm", [8, 128, 128], f32, kind="ExternalInput")
    wm16 = nc.dram_tensor("wm16", [128, 128], f16, kind="ExternalInput")
    out = nc.dram_tensor("out", [NR, W], f16, kind="ExternalOutput")
    out_t = nc.dram_tensor("out_t", [128, NT, 16], f32, kind="ExternalOutput")

    with TileContext(nc) as tc:
        with tc.tile_pool(name="io", bufs=2) as iop, \
             tc.tile_pool(name="pl", bufs=1) as pool, \
             tc.tile_pool(name="ps", bufs=3, space="PSUM") as psp:

            V = nc.vector
            S = nc.scalar

            imgs = {}

            def dma_img(c, nm, row0, nrows):
                img = iop.tile([128, WPAD], f32, tag="img", name=f"img{c}{nm}",
                               bufs=3)
                off0 = c * SHARD_ROWS * WPAD + row0 * WPAD
                nc.sync.dma_start(
                    out=img[0:nrows, :],
                    in_=bass.AP(xr, off0, [[WPAD, nrows], [343, 6], [1, 343]]))
                imgs[(c, nm)] = img

            timgs = []
            for c_ in range(3):
                ti = pool.tile([128, NT + 10, 26], f32, tag="timg",
                               name=f"timg{c_}", bufs=2)
                nc.sync.dma_start(out=ti[:], in_=bass.AP(
                    xt, c_ * 128 * 24 * 26, [[24 * 26, 128], [26, NT + 10], [1, 26]]))
                timgs.append(ti)

            dma_img(0, "a", 0, 128)
            dma_img(0, "b", 124, 128)

            wt_ = []
            for i in range(8):
                if i in (2, 5):
                    wt_.append(None)
                    continue
                wti = pool.tile([128, 128], f32, tag=f"w{i}", name=f"wt{i}")
                nc.sync.dma_start(out=wti[:], in_=bass.AP(wm, i * 128 * 128,
                                                          [[128, 128], [1, 128]]))
                wt_.append(wti)
            m3t = pool.tile([128, 128], f16, tag="m3", name="m3t")
            nc.sync.dma_start(out=m3t[:], in_=bass.AP(wm16, 0, [[128, 128], [1, 128]]))

            # accumulators per chunk
            acc = {}
            for q in ("a", "b"):
                acc[q] = (
                    pool.tile([128, WG], f32, tag=f"g_{q}", name=f"g_{q}"),
                    pool.tile([128, WG], f32, tag=f"sgx_{q}", name=f"sgx_{q}"),
                    pool.tile([128, WG], f32, tag=f"sgy_{q}", name=f"sgy_{q}"),
                )

            # ---- tail: columns-mode v1-style pass for out rows [248,256) ----
            N, R, RV, RT, RN = NT, NT + 10, NT + 6, NT + 4, NT + 2
            tg = pool.tile([128, RT, 20], f32, tag="tg", name="tg")
            tsgx = pool.tile([128, RT, 20], f32, tag="tsgx", name="tsgx")
            tsgy = pool.tile([128, RT, 20], f32, tag="tsgy", name="tsgy")
            for c in range(3):
                img = timgs[c]
                tA = pool.tile([128, R, 22], f32, tag="tlA", name=f"tlA{c}")
                tB = pool.tile([128, R, 22], f32, tag="tlB", name=f"tlB{c}")
                tC = pool.tile([128, R, 22], f32, tag="tlC", name=f"tlC{c}")
                tD = pool.tile([128, RV, 22], f32, tag="tlD", name=f"tlD{c}")
                V.tensor_tensor(tA[:], img[:, :, 1:23], img[:, :, 3:25], Alu.add)
                V.tensor_tensor(tB[:], img[:, :, 0:22], img[:, :, 4:26], Alu.add)
                V.scalar_tensor_tensor(tC[:], tA[:], gb, img[:, :, 2:24], Alu.mult, Alu.add)
                V.scalar_tensor_tensor(tC[:], tB[:], ga, tC[:], Alu.mult, Alu.add)
                V.tensor_tensor(tA[:, 0:RV, :], tC[:, 1:RV + 1, :], tC[:, 3:RV + 3, :], Alu.add)
                V.tensor_tensor(tB[:, 0:RV, :], tC[:, 0:RV, :], tC[:, 4:RV + 4, :], Alu.add)
                V.scalar_tensor_tensor(tD[:], tA[:, 0:RV, :], gb, tC[:, 2:RV + 2, :],
                                       Alu.mult, Alu.add)
                V.scalar_tensor_tensor(tD[:], tB[:, 0:RV, :], ga, tD[:], Alu.mult, Alu.add)
                V.tensor_tensor(tA[:, 0:RT, :], tD[:, 0:RT, :], tD[:, 2:RT + 2, :], Alu.add)
                V.scalar_tensor_tensor(tA[:, 0:RT, :], tD[:, 1:RT + 1, :], 2.0,
                                       tA[:, 0:RT, :], Alu.mult, Alu.add)   # t1
                V.tensor_tensor(tB[:, 0:RT, :], tD[:, 0:RT, :], tD[:, 2:RT + 2, :],
                                Alu.subtract)                                # t2
                gx = tsgx[:] if c == 0 else tC[:, 0:RT, 0:20]
                gy = tsgy[:] if c == 0 else tD[:, 0:RT, 0:20]
                w2 = tD[:, 0:RT, 0:20]
                V.tensor_tensor(gx, tA[:, 0:RT, 0:20], tA[:, 0:RT, 2:22], Alu.subtract)
                V.tensor_tensor(w2, tB[:, 0:RT, 0:20], tB[:, 0:RT, 2:22], Alu.add)
                V.scalar_tensor_tensor(gy, tB[:, 0:RT, 1:21], 2.0, w2, Alu.mult, Alu.add)
                q1 = tA[:, 0:RT, 0:20]
                q2 = tB[:, 0:RT, 0:20]
                S.activation(q1, gx, Act.Square)
                S.activation(q2, gy, Act.Square)
                V.tensor_tensor(q1, q1, q2, Alu.add)
                m = tg[:] if c == 0 else tB[:, 0:RT, 0:20]
                S.activation(m, q1, Act.Sqrt)
                if c != 0:
                    V.tensor_tensor(tg[:], tg[:], m, Alu.add)
                    V.tensor_tensor(tsgx[:], tsgx[:], gx, Alu.add)
                    V.tensor_tensor(tsgy[:], tsgy[:], gy, Alu.add)




            for c_ in (1, 2):
                dma_img(c_, "a", 0, 128)
                dma_img(c_, "b", 124, 128)

            def emit_bh(c):
                d = {}
                for (nm, row0, nrows, tag) in (
                        ("a", 0, 128, "bhA"), ("b", 124, 128, "bhB")):
                    img = imgs[(c, nm)]
                    bht = pool.tile([128, WB], f32, tag="bht", name=f"bht{c}{nm}")
                    bh = pool.tile([128, WB], f32, tag=tag, name=f"bh{nm}{c}",
                                   bufs=2)
                    r = slice(0, nrows)
                    V.tensor_tensor(bht[r, :], img[r, 0:WB], img[r, 4:WB + 4], Alu.add)
                    V.tensor_tensor(bh[r, :], img[r, 1:WB + 1], img[r, 3:WB + 3], Alu.add)
                    V.scalar_tensor_tensor(bh[r, :], bh[r, :], gb, img[r, 2:WB + 2],
                                           Alu.mult, Alu.add)
                    V.scalar_tensor_tensor(bh[r, :], bht[r, :], ga, bh[r, :],
                                           Alu.mult, Alu.add)
                    d[nm] = bh
                return d

            shifts = {}
            bhs_all = {0: emit_bh(0)}
            for c in range(3):
                if c + 1 < 3:
                    bhs_all[c + 1] = emit_bh(c + 1)
                bhs = bhs_all.pop(c) if False else bhs_all[c]

                # ---- per chunk: PE T1/T2, then hsobel + mag on DVE ----
                for q in ("a", "b"):
                    if q == "a":
                        main_src, halo_src, halo_k, wia, wib = bhs["a"], bhs["b"], 128, 1, 4
                    else:
                        main_src, halo_src, halo_k, wia, wib = bhs["b"], None, 0, 2, 5
                    S1 = pool.tile([128, WB], f32, tag="S1", name=f"S1_{c}{q}")
                    S2 = pool.tile([128, WB], f32, tag="S2", name=f"S2_{c}{q}")
                    gq, sgxq, sgyq = acc[q]
                    gxt = sgxq if c == 0 else pool.tile(
                        [128, WG], f32, tag="gxT", name=f"gx{c}{q}")
                    gyt = sgyq if c == 0 else pool.tile(
                        [128, WG], f32, tag="gyT", name=f"gy{c}{q}")
                    gx = gxt[:]
                    gy = gyt[:]
                    for (b0, bw) in BLOCKS:
                        has_halo = halo_src is not None
                        pt1 = psp.tile([128, bw], f32, tag="pT1", name=f"pt1_{c}{q}{b0}")
                        nc.tensor.matmul(pt1[:], wt_[0][:], main_src[:, b0:b0 + bw],
                                         start=True, stop=not has_halo)
                        if has_halo:
                            nc.tensor.matmul(pt1[:], wt_[wia][0:halo_k, :],
                                             halo_src[0:halo_k, b0:b0 + bw],
                                             start=False, stop=True)
                        S.copy(S1[:, b0:b0 + bw], pt1[:])
                        pt2 = psp.tile([128, bw], f32, tag="pT2", name=f"pt2_{c}{q}{b0}")
                        nc.tensor.matmul(pt2[:], wt_[3][:], main_src[:, b0:b0 + bw],
                                         start=True, stop=not has_halo)
                        if has_halo:
                            nc.tensor.matmul(pt2[:], wt_[wib][0:halo_k, :],
                                             halo_src[0:halo_k, b0:b0 + bw],
                                             start=False, stop=True)
                        S.copy(S2[:, b0:b0 + bw], pt2[:])
                        # hsobel for the column range this block completes
                        g0 = max(0, b0 - 2)
                        g1 = min(WG, b0 + bw - 2)
                        V.tensor_tensor(gxt[:, g0:g1], S1[:, g0:g1],
                                        S1[:, g0 + 2:g1 + 2], Alu.subtract)
                        V.tensor_tensor(gyt[:, g0:g1], S2[:, g0:g1],
                                        S2[:, g0 + 2:g1 + 2], Alu.add)
                        V.scalar_tensor_tensor(gyt[:, g0:g1], S2[:, g0 + 1:g1 + 1],
                                               2.0, gyt[:, g0:g1], Alu.mult, Alu.add)
                    # final sliver: gx/gy cols [WB-8..WG) done, but blocks end at
                    # b0+bw-2 = WB-2 = WG+2 > WG, so full range is covered.
                    S.activation(S1[:, 0:WG], gx, Act.Square)
                    S.activation(S2[:, 0:WG], gy, Act.Square)
                    V.tensor_tensor(S1[:, 0:WG], S1[:, 0:WG], S2[:, 0:WG], Alu.add)
                    m = gq[:] if c == 0 else S2[:, 0:WG]
                    S.activation(m, S1[:, 0:WG], Act.Sqrt)
                    if c != 0:
                        V.tensor_tensor(gq[:], gq[:], m, Alu.add)
                        V.tensor_tensor(sgxq[:], sgxq[:], gx, Alu.add)
                        V.tensor_tensor(sgyq[:], sgyq[:], gy, Alu.add)


            # ---- NMS per chunk ----
            def nms_chunk(q):
                gq, sgxq, sgyq = acc[q]
                rr = pool.tile([128, WG], f32, tag="bhA", name=f"rr{q}", bufs=2)
                ss = pool.tile([128, WG], f32, tag="bhB", name=f"ss{q}", bufs=2)
                m0 = pool.tile([128, WG], u8, tag="mk0", name=f"m0{q}")
                m2 = pool.tile([128, WG], u8, tag="mk1", name=f"m2{q}")
                d = pool.tile([128, WG], f32, tag="bhX", name=f"d{q}")
                dpos = pool.tile([128, WG], u8, tag="mk2", name=f"dp{q}")
                S.activation(rr[:], sgyq[:], Act.Abs)
                S.activation(ss[:], sgxq[:], Act.Abs)
                V.scalar_tensor_tensor(m0[:], ss[:], t1c, rr[:], Alu.mult, Alu.is_ge)
                V.scalar_tensor_tensor(m2[:], ss[:], t2c, rr[:], Alu.mult, Alu.is_le)
                V.tensor_tensor(d[:], sgxq[:], sgyq[:], Alu.mult)
                V.tensor_scalar(dpos[:], d[:], 0.0, None, Alu.is_ge)

                gU, gD = shifts[q]

                cand1 = pool.tile([128, WC], f32, tag="bht", name=f"c1{q}")
                cc = pool.tile([128, WC], f32, tag="gxT", name=f"cc{q}")
                cand2 = pool.tile([128, WC], f32, tag="gyT", name=f"c2{q}")
                V.tensor_tensor(cand1[:], gU[:, 2:WG], gD[:, 0:WC], Alu.max)   # SE/NW
                V.tensor_tensor(cc[:], gU[:, 0:WC], gD[:, 2:WG], Alu.max)      # SW/NE
                V.copy_predicated(cc[:], dpos[:, 1:WC + 1], cand1[:])
                V.tensor_tensor(cand2[:], gU[:, 1:WC + 1], gD[:, 1:WC + 1], Alu.max)  # S/N
                V.copy_predicated(cc[:], m2[:, 1:WC + 1], cand2[:])
                cand0 = pool.tile([128, WC], f32, tag="bht", name=f"c0{q}")
                V.tensor_tensor(cand0[:], gq[:, 2:WG], gq[:, 0:WC], Alu.max)   # E/W
                V.copy_predicated(cc[:], m0[:, 1:WC + 1], cand0[:])

                hp = pool.tile([128, WC], f16, tag="hp", name=f"hp{q}")
                lm = pool.tile([128, W], f16, tag="lm", name=f"lm{q}")
                V.scalar_tensor_tensor(hp[:], cc[:], high, gq[:, 1:WC + 1],
                                       Alu.max, Alu.is_lt)
                V.scalar_tensor_tensor(lm[:], cc[:, 1:W + 1], lowx, gq[:, 2:W + 2],
                                       Alu.max, Alu.is_lt)

                rm1 = pool.tile([128, W], f16, tag="rm1", name=f"rm1{q}")
                rm = pool.tile([128, W], f16, tag="rm", name=f"rm{q}")
                V.tensor_tensor(rm1[:], hp[:, 0:W], hp[:, 2:W + 2], Alu.max)
                V.tensor_tensor(rm[:], rm1[:], hp[:, 1:W + 1], Alu.max)
                # vertical 3-row OR via tridiagonal-ones matmul on PE
                zp = pool.tile([128, W], f16, tag="rm1", name=f"zp{q}")
                for zb in range(0, W, 512):
                    pz = psp.tile([128, 512], f32, tag="pT1", name=f"pz{q}{zb}")
                    nc.tensor.matmul(pz[:], m3t[:], rm[:, zb:zb + 512],
                                     start=True, stop=True)
                    S.copy(zp[:, zb:zb + 512], pz[:])
                outt = iop.tile([128, W], f16, tag="out", name=f"out{q}")
                r0, nrows_out = (0, 124) if q == "a" else (124, NR - 124)
                hw3 = W // 2
                for h0 in (0, hw3):
                    V.scalar_tensor_tensor(outt[:, h0:h0 + hw3], zp[:, h0:h0 + hw3],
                                           0.5, lm[:, h0:h0 + hw3],
                                           Alu.is_ge, Alu.mult)
                    nc.sync.dma_start(
                        out=bass.AP(out, r0 * W + h0, [[W, nrows_out], [1, hw3]]),
                        in_=outt[2:2 + nrows_out, h0:h0 + hw3])



            for q in ("a", "b"):
                gq = acc[q][0]
                gU = pool.tile([128, WG], f32, tag="S1", name=f"gU{q}")
                gD = pool.tile([128, WG], f32, tag="S2", name=f"gD{q}")
                for sb in range(0, WG, 512):
                    sw = min(512, WG - sb)
                    pu = psp.tile([128, sw], f32, tag="pT1", name=f"pu{q}{sb}")
                    nc.tensor.matmul(pu[:], wt_[6][:], gq[:, sb:sb + sw],
                                     start=True, stop=True)
                    S.copy(gU[:, sb:sb + sw], pu[:])
                    pd = psp.tile([128, sw], f32, tag="pT2", name=f"pd{q}{sb}")
                    nc.tensor.matmul(pd[:], wt_[7][:], gq[:, sb:sb + sw],
                                     start=True, stop=True)
                    S.copy(gD[:, sb:sb + sw], pd[:])
                shifts[q] = (gU, gD)

            nms_chunk("a")

            rr = pool.tile([128, RN, 18], f32, tag="tlA", name="trr")
            ss = pool.tile([128, RN, 18], f32, tag="tlB", name="tss")
            m0 = pool.tile([128, RN, 18], u8, tag="tmk0", name="tm0")
            m2 = pool.tile([128, RN, 18], u8, tag="tmk1", name="tm2")
            d = pool.tile([128, RN, 18], f32, tag="tlC", name="td")
            dpos = pool.tile([128, RN, 18], u8, tag="tmk2", name="tdp")
            S.activation(rr[:], tsgy[:, 1:RN + 1, 1:19], Act.Abs)
            S.activation(ss[:], tsgx[:, 1:RN + 1, 1:19], Act.Abs)
            V.scalar_tensor_tensor(m0[:], ss[:], t1c, rr[:], Alu.mult, Alu.is_ge)
            V.scalar_tensor_tensor(m2[:], ss[:], t2c, rr[:], Alu.mult, Alu.is_le)
            V.tensor_tensor(d[:], tsgx[:, 1:RN + 1, 1:19], tsgy[:, 1:RN + 1, 1:19], Alu.mult)
            V.tensor_scalar(dpos[:], d[:], 0.0, None, Alu.is_ge)
            cand = pool.tile([128, RN, 18], f32, tag="tlD", name="tc1")
            cc = pool.tile([128, RN, 18], f32, tag="tlA", name="tcc")
            cand2 = pool.tile([128, RN, 18], f32, tag="tlB", name="tc2")
            cand3 = pool.tile([128, RN, 18], f32, tag="tlC", name="tc0")
            V.tensor_tensor(cand[:], tg[:, 2:RN + 2, 2:20], tg[:, 0:RN, 0:18], Alu.max)
            V.tensor_tensor(cc[:], tg[:, 2:RN + 2, 0:18], tg[:, 0:RN, 2:20], Alu.max)
            V.copy_predicated(cc[:], dpos[:], cand[:])
            V.tensor_tensor(cand2[:], tg[:, 2:RN + 2, 1:19], tg[:, 0:RN, 1:19], Alu.max)
            V.copy_predicated(cc[:], m2[:], cand2[:])
            V.tensor_tensor(cand3[:], tg[:, 1:RN + 1, 2:20], tg[:, 1:RN + 1, 0:18], Alu.max)
            V.copy_predicated(cc[:], m0[:], cand3[:])
            hp = pool.tile([128, RN, 18], f16, tag="thp", name="thp")
            lm = pool.tile([128, N, 16], f32, tag="tlC", name="tlm")
            V.scalar_tensor_tensor(hp[:], cc[:], high, tg[:, 1:RN + 1, 1:19],
                                   Alu.max, Alu.is_lt)
            V.scalar_tensor_tensor(lm[:], cc[:, 1:N + 1, 1:17], lowx,
                                   tg[:, 2:RN, 2:18], Alu.max, Alu.is_lt)
            rm1 = pool.tile([128, RN, 16], f16, tag="trm1", name="trm1")
            rm = pool.tile([128, RN, 16], f16, tag="trm", name="trm")
            cm = pool.tile([128, N, 16], f16, tag="tcm", name="tcm")
            V.tensor_tensor(rm1[:], hp[:, :, 0:16], hp[:, :, 2:18], Alu.max)
            V.tensor_tensor(rm[:], rm1[:], hp[:, :, 1:17], Alu.max)
            V.tensor_tensor(cm[:], rm[:, 0:N, :], rm[:, 2:RN, :], Alu.max)
            V.tensor_tensor(cm[:], cm[:], rm[:, 1:N + 1, :], Alu.max)
            outt = pool.tile([128, N, 16], f32, tag="tlD", name="touts")
            V.tensor_tensor(outt[:], lm[:], cm[:], Alu.mult)
            nc.sync.dma_start(
                out=bass.AP(out_t, 0, [[NT * 16, 128], [16, N], [1, 16]]),
                in_=outt[:])

            nms_chunk("b")

    nc.finalize()
    return nc


def _get_compiled(low, high):
    key = (low, high)
    if key not in _COMPILED:
        _COMPILED[key] = _build(low, high)
    return _COMPILED[key]


def kernel(img, threshold1, threshold2, _trace=False):
    from concourse import bass_utils

    t1 = float(np.asarray(threshold1))
    t2 = float(np.asarray(threshold2))
    low, high = min(t1, t2), max(t1, t2)

    x = np.ascontiguousarray(np.asarray(img, dtype=np.float32)[0])  # [3,H,W]
    xp = np.zeros((3, H + 2 * HALO, W + 2 * HALO), dtype=np.float32)
    xp[:, HALO:HALO + H, HALO:HALO + W] = x

    wmv = _weights()
    m3v = np.zeros((128, 128), np.float16)
    for p in range(128):
        for j in range(max(0, p - 1), min(128, p + 2)):
            m3v[p, j] = 1.0
    win = np.lib.stride_tricks.sliding_window_view(xp, 26, axis=2)[:, :, ::16, :]
    in_maps = []
    for k in range(8):
        rows = np.ascontiguousarray(xp[:, k * RPC:k * RPC + SHARD_ROWS, :])
        tw = win[:, k * RPC + NR:k * RPC + NR + NT + 10]     # [3,24,128,26]
        packed = np.ascontiguousarray(tw.transpose(0, 2, 1, 3))
        in_maps.append({"xr": rows, "xt": packed, "wm": wmv, "wm16": m3v})

    nc = _get_compiled(low, high)
    res = bass_utils.run_bass_kernel_spmd(nc, in_maps, core_ids=list(range(8)),
                                          trace=_trace)

    full = np.zeros((1, 1, H, W), dtype=np.float32)
    for k in range(8):
        o = res.results[k]["out"]          # [248, 2048] f16
        ot = res.results[k]["out_t"]       # [128, 8, 16] f32
        full[0, 0, k * RPC:k * RPC + NR, :] = o.astype(np.float32)
        full[0, 0, k * RPC + NR:(k + 1) * RPC, :] = (
            ot.transpose(1, 0, 2).reshape(NT, W))
    full[:, :, 0, :] = 0.0
    full[:, :, -1, :] = 0.0
    full[:, :, :, 0] = 0.0
    full[:, :, :, -1] = 0.0
    if _trace:
        kernel._last_results = res
    return full
